# revision 1
# baseline (speedup 1.0000x reference)
import numpy as np

# nn_LocalDynamics GNN message passing.
# delta[n] = sum_e tanh(fMLP(inp_e))[addr_from=n] + tanh(tMLP(inp_e))[addr_to=n]
# out = tanh(delta).  inp_e = [h[from], h[to], x_e, hg, xg, t] (153 dims).
N = 100_000
E = 800_000
D = 64
H = 128
NCORES = 8
PAIR = 1024                      # edges per device iteration
EPC = E // NCORES                # 100000 edges per core
EPAD = ((EPC + PAIR - 1) // PAIR) * PAIR   # 100352


def _scatter_add(delta, idx, vals):
    o = np.argsort(idx, kind="stable")
    si = idx[o]
    sv = vals[o]
    starts = np.flatnonzero(np.r_[True, si[1:] != si[:-1]])
    sums = np.add.reduceat(sv, starts, axis=0)
    np.add.at(delta, si[starts], sums)


def _host_post(af, at, d_from, d_to, n_nodes):
    delta = np.zeros((n_nodes, D), dtype=np.float32)
    _scatter_add(delta, af, d_from)
    _scatter_add(delta, at, d_to)
    return np.tanh(delta).astype(np.float32)


def _mlp_np(x, W0, b0, W1, b1, W2, b2):
    h = np.tanh(x @ W0 + b0)
    h = np.tanh(h @ W1 + b1)
    return h @ W2 + b2


def _kernel_numpy(addr_from, addr_to, h_local, h_global, x_local, x_global, t,
                  f_W0, f_b0, f_W1, f_b1, f_W2, f_b2,
                  t_W0, t_b0, t_W1, t_b1, t_W2, t_b2):
    af = np.asarray(addr_from).astype(np.int64)
    at = np.asarray(addr_to).astype(np.int64)
    h_local = np.asarray(h_local, dtype=np.float32)
    x_local = np.asarray(x_local, dtype=np.float32)
    const = np.concatenate([np.asarray(h_global, np.float32).ravel(),
                            np.asarray(x_global, np.float32).ravel(),
                            np.asarray(t, np.float32).ravel()])
    ne = af.shape[0]
    d_from = np.empty((ne, D), np.float32)
    d_to = np.empty((ne, D), np.float32)
    CH = 100_000
    for s in range(0, ne, CH):
        e = min(s + CH, ne)
        inp = np.concatenate([h_local[af[s:e]], h_local[at[s:e]], x_local[s:e],
                              np.broadcast_to(const, (e - s, 21))], axis=1).astype(np.float32)
        d_from[s:e] = np.tanh(_mlp_np(inp, f_W0, f_b0, f_W1, f_b1, f_W2, f_b2))
        d_to[s:e] = np.tanh(_mlp_np(inp, t_W0, t_b0, t_W1, t_b1, t_W2, t_b2))
    return _host_post(af, at, d_from, d_to, h_local.shape[0])


_BASS_CACHE = {}


def _build_bass():
    import concourse.bass as bass
    import concourse.mybir as mybir
    import concourse.tile as tile

    # walrus in this env rejects Drain instructions carrying >1 sem wait;
    # move each wait onto its own sync nop before the drain.
    def _patched(self, tick_clock, wait_clock):
        from concourse.tile import ScopedClock
        nop0 = self.nc.sync.nop(nofuse=True)
        wait_clock.add_sem_waits(nop0.ins, ScopedClock({None: tick_clock.global_clock}))
        si = nop0.ins.sync_info
        if si is not None and si.on_wait and len(si.on_wait) > 1:
            waits = list(si.on_wait)
            si.on_wait = waits[:1]
            for w in waits[1:]:
                n = self.nc.sync.nop(nofuse=True)
                n.ins.sync_info = mybir.SyncInfo(on_wait=[w], on_update=[])
        self.nc.sync.drain()
        self.nc.all_engine_barrier()
        popped = self.nc._tile_sem_poison_stack.pop()
        assert popped is self._sem_poison
        self.nc.clear_and_free_semaphores(list(self.sems.allocated().values()))
        self.nc.all_engine_barrier()

    tile.TileContext._drain_and_barrier = _patched

    f32 = mybir.dt.float32
    f32r = mybir.dt.float32r
    nc = bass.Bass()
    inpa = nc.dram_tensor("inpa", [128, EPAD], f32, kind="ExternalInput")
    inpb = nc.dram_tensor("inpb", [4, EPAD], f32, kind="ExternalInput")
    wts = {}
    for p in ("f", "t"):
        wts[p + "w0a"] = nc.dram_tensor(p + "w0a", [128, H], f32, kind="ExternalInput")
        wts[p + "w0b"] = nc.dram_tensor(p + "w0b", [4, H], f32, kind="ExternalInput")
        wts[p + "w1"] = nc.dram_tensor(p + "w1", [H, H], f32, kind="ExternalInput")
        wts[p + "w2"] = nc.dram_tensor(p + "w2", [H, D], f32, kind="ExternalInput")
        wts[p + "b0"] = nc.dram_tensor(p + "b0", [H, 1], f32, kind="ExternalInput")
        wts[p + "b1"] = nc.dram_tensor(p + "b1", [H, 1], f32, kind="ExternalInput")
        wts[p + "b2"] = nc.dram_tensor(p + "b2", [D, 1], f32, kind="ExternalInput")
    douts = {p: nc.dram_tensor("d" + p, [D, EPAD], f32, kind="ExternalOutput")
             for p in ("f", "t")}

    Tanh = mybir.ActivationFunctionType.Tanh
    with tile.TileContext(nc) as tc:
        with tc.tile_pool(name="wpool", bufs=1) as wp, \
             tc.tile_pool(name="io", bufs=3) as io, \
             tc.tile_pool(name="act", bufs=2) as ap_, \
             tc.tile_pool(name="ps01", bufs=1, space="PSUM") as ps01, \
             tc.tile_pool(name="ps2", bufs=2, space="PSUM") as ps2:
            wt = {}
            for k, dr in wts.items():
                sh = list(dr.shape)
                tl = wp.tile(sh, f32, tag="w" + k)
                nc.sync.dma_start(out=tl[:], in_=dr[:])
                wt[k] = tl
            for it in range(EPAD // PAIR):
                sl = slice(it * PAIR, (it + 1) * PAIR)
                ra = io.tile([128, PAIR], f32, tag="ra")
                rb = io.tile([4, PAIR], f32, tag="rb")
                nc.sync.dma_start(out=ra[:], in_=inpa[:, sl])
                nc.sync.dma_start(out=rb[:], in_=inpb[:, sl])
                for p in ("f", "t"):
                    ps0 = ps01.tile([128, PAIR], f32, tag="ps0")
                    for hh in range(2):
                        hs = slice(hh * 512, (hh + 1) * 512)
                        nc.tensor.matmul(out=ps0[:, hs],
                                         lhsT=wt[p + "w0a"][:],
                                         rhs=ra[:, hs],
                                         start=True, stop=False)
                        nc.tensor.matmul(out=ps0[:, hs],
                                         lhsT=wt[p + "w0b"][:],
                                         rhs=rb[:, hs],
                                         start=False, stop=True)
                    h1 = ap_.tile([128, PAIR], f32, tag="h1")
                    nc.scalar.activation(h1[:], ps0[:], Tanh, bias=wt[p + "b0"][:, 0:1])
                    ps1 = ps01.tile([128, PAIR], f32, tag="ps1")
                    for hh in range(2):
                        hs = slice(hh * 512, (hh + 1) * 512)
                        nc.tensor.matmul(out=ps1[:, hs],
                                         lhsT=wt[p + "w1"][:],
                                         rhs=h1[:, hs],
                                         start=True, stop=True)
                    h2 = ap_.tile([128, PAIR], f32, tag="h2")
                    nc.scalar.activation(h2[:], ps1[:], Tanh, bias=wt[p + "b1"][:, 0:1])
                    psd = ps2.tile([D, PAIR], f32, tag="psd")
                    for hh in range(2):
                        hs = slice(hh * 512, (hh + 1) * 512)
                        nc.tensor.matmul(out=psd[:, hs],
                                         lhsT=wt[p + "w2"][:],
                                         rhs=h2[:, hs],
                                         start=True, stop=True)
                    dt_ = ap_.tile([D, PAIR], f32, tag="dt")
                    nc.scalar.activation(dt_[:], psd[:], Tanh, bias=wt[p + "b2"][:, 0:1])
                    nc.sync.dma_start(out=douts[p][:, sl], in_=dt_[:])

    # this walrus rejects any compute instruction carrying >1 sem wait;
    # hoist extra waits onto same-engine nops placed just before it.
    ctr = 0
    for bb in nc.main_func.blocks:
        new = []
        for ins in bb.instructions:
            si = getattr(ins, "sync_info", None)
            if si is not None and si.on_wait and len(si.on_wait) > 1:
                waits = list(si.on_wait)
                si.on_wait = [waits[-1]]
                for w in waits[:-1]:
                    ctr += 1
                    nop = mybir.InstNoOp(
                        name=f"wsplit-{ctr}", engine=ins.engine, ins=[], outs=[],
                        sync_info=mybir.SyncInfo(on_wait=[w], on_update=[]))
                    new.append(nop)
            new.append(ins)
        bb.instructions[:] = new
    return nc


def _kernel_bass(addr_from, addr_to, h_local, h_global, x_local, x_global, t,
                 f_W0, f_b0, f_W1, f_b1, f_W2, f_b2,
                 t_W0, t_b0, t_W1, t_b1, t_W2, t_b2, trace=False):
    import sys
    if "/opt/trn_rl_repo" not in sys.path:
        sys.path.insert(0, "/opt/trn_rl_repo")
    from concourse.bass_utils import run_bass_kernel_spmd

    af = np.asarray(addr_from).astype(np.int64)
    at = np.asarray(addr_to).astype(np.int64)
    h_local = np.ascontiguousarray(np.asarray(h_local, np.float32))
    x_local = np.asarray(x_local, np.float32)
    const = np.concatenate([np.asarray(h_global, np.float32).ravel(),
                            np.asarray(x_global, np.float32).ravel(),
                            np.asarray(t, np.float32).ravel()])  # [21]

    if "nc" not in _BASS_CACHE:
        _BASS_CACHE["nc"] = _build_bass()
    nc = _BASS_CACHE["nc"]

    weights = {}
    for p, W0, b0, W1, b1, W2, b2 in (
        ("f", f_W0, f_b0, f_W1, f_b1, f_W2, f_b2),
        ("t", t_W0, t_b0, t_W1, t_b1, t_W2, t_b2),
    ):
        W0 = np.asarray(W0, np.float32)
        b0eff = np.asarray(b0, np.float32) + const @ W0[132:153]
        weights[p + "w0a"] = np.ascontiguousarray(W0[0:128])
        weights[p + "w0b"] = np.ascontiguousarray(W0[128:132])
        weights[p + "w1"] = np.asarray(W1, np.float32)
        weights[p + "w2"] = np.asarray(W2, np.float32)
        weights[p + "b0"] = b0eff.reshape(H, 1)
        weights[p + "b1"] = np.asarray(b1, np.float32).reshape(H, 1)
        weights[p + "b2"] = np.asarray(b2, np.float32).reshape(D, 1)

    in_maps = []
    for c in range(NCORES):
        s, e = c * EPC, (c + 1) * EPC
        ia = np.zeros((128, EPAD), np.float32)
        ia[0:64, :EPC] = h_local[af[s:e]].T
        ia[64:128, :EPC] = h_local[at[s:e]].T
        ib = np.zeros((4, EPAD), np.float32)
        ib[:, :EPC] = x_local[s:e].T
        m = {"inpa": ia, "inpb": ib}
        m.update(weights)
        in_maps.append(m)

    res = run_bass_kernel_spmd(nc, in_maps, core_ids=list(range(NCORES)),
                               trace=trace)
    d_from = np.concatenate(
        [res.results[c]["df"][:, :EPC].T for c in range(NCORES)], axis=0)
    d_to = np.concatenate(
        [res.results[c]["dt"][:, :EPC].T for c in range(NCORES)], axis=0)
    out = _host_post(af, at, d_from, d_to, h_local.shape[0])
    if trace:
        return out, res
    return out


def kernel(**inputs):
    try:
        return _kernel_bass(**inputs)
    except Exception:
        import traceback
        traceback.print_exc()
        return _kernel_numpy(**inputs)



# revision 8
# speedup vs baseline: 5.6290x; 5.6290x over previous
import numpy as np

# nn_LocalDynamics GNN message passing.
#   delta[n] = sum_e tanh(fMLP(inp_e))[addr_from=n] + tanh(tMLP(inp_e))[addr_to=n]
#   out = tanh(delta).  inp_e = [h[from], h[to], x_e, hg, xg, t] (153 dims).
#
# Destination-sharded design: each core owns nodes [c*12500, (c+1)*12500).
# Every edge yields two "slots": an f-slot on the core owning addr_from and a
# t-slot on the core owning addr_to.  Slots are grouped by 128-node windows of
# the owning core; each window holds a fixed K tiles of 128 slots per
# population (f/t), host-padded.  On device, per 512-slot batch:
#   indirect-DMA gather of (h[from], h[to]) row pairs -> XBAR transpose to
#   feature-major -> fp16 MLP -> slot-major final layer -> one-hot matmul
#   accumulates the window's delta in PSUM -> contiguous fp32 writes.
# Host applies the final tanh (and the overflow spill path, normally empty).

N = 100_000
E = 800_000
D = 64
H = 128
NCORES = 8
NS = N // NCORES            # nodes per core (12500)
NWIN = (NS + 127) // 128    # windows per core (98)
LASTROWS = NS - (NWIN - 1) * 128   # rows in last window (84)
KTILES = 12                 # 128-slot tiles per population per window
BT = 4                      # tiles per batch
SB = BT * 128               # slots per batch (512)
NBW = KTILES // BT          # batches per population-window (3)
NB = NWIN * 2 * NBW         # batches per core (588)
SLOTS = NB * SB             # padded slots per core (301056)


def _scatter_add(delta, idx, vals):
    o = np.argsort(idx, kind="stable")
    si = idx[o]
    sv = vals[o]
    starts = np.flatnonzero(np.r_[True, si[1:] != si[:-1]])
    sums = np.add.reduceat(sv, starts, axis=0)
    np.add.at(delta, si[starts], sums)


def _kernel_numpy(addr_from, addr_to, h_local, h_global, x_local, x_global, t,
                  f_W0, f_b0, f_W1, f_b1, f_W2, f_b2,
                  t_W0, t_b0, t_W1, t_b1, t_W2, t_b2):
    af = np.asarray(addr_from).astype(np.int64)
    at = np.asarray(addr_to).astype(np.int64)
    h_local = np.asarray(h_local, dtype=np.float32)
    x_local = np.asarray(x_local, dtype=np.float32)
    const = np.concatenate([np.asarray(h_global, np.float32).ravel(),
                            np.asarray(x_global, np.float32).ravel(),
                            np.asarray(t, np.float32).ravel()])
    ne = af.shape[0]
    delta = np.zeros((h_local.shape[0], D), np.float32)
    CH = 100_000
    for s in range(0, ne, CH):
        e = min(s + CH, ne)
        inp = np.concatenate([h_local[af[s:e]], h_local[at[s:e]], x_local[s:e],
                              np.broadcast_to(const, (e - s, 21))], axis=1).astype(np.float32)
        d_f = np.tanh(np.tanh(np.tanh(inp @ f_W0 + f_b0) @ f_W1 + f_b1) @ f_W2 + f_b2)
        d_t = np.tanh(np.tanh(np.tanh(inp @ t_W0 + t_b0) @ t_W1 + t_b1) @ t_W2 + t_b2)
        _scatter_add(delta, af[s:e], d_f.astype(np.float32))
        _scatter_add(delta, at[s:e], d_t.astype(np.float32))
    return np.tanh(delta).astype(np.float32)


_BASS_CACHE = {}


def _build_bass(ns, nwin, lastrows, ktiles, bt, nb, ntot, walrus_fix=True):
    import concourse.bass as bass
    import concourse.mybir as mybir
    import concourse.tile as tile

    # walrus in this env rejects Drain instructions carrying >1 sem wait;
    # move each wait onto its own sync nop before the drain.
    def _patched(self, tick_clock, wait_clock):
        from concourse.tile import ScopedClock
        nop0 = self.nc.sync.nop(nofuse=True)
        wait_clock.add_sem_waits(nop0.ins, ScopedClock({None: tick_clock.global_clock}))
        si = nop0.ins.sync_info
        if si is not None and si.on_wait and len(si.on_wait) > 1:
            waits = list(si.on_wait)
            si.on_wait = waits[:1]
            for w in waits[1:]:
                n = self.nc.sync.nop(nofuse=True)
                n.ins.sync_info = mybir.SyncInfo(on_wait=[w], on_update=[])
        self.nc.sync.drain()
        self.nc.all_engine_barrier()
        popped = self.nc._tile_sem_poison_stack.pop()
        assert popped is self._sem_poison
        self.nc.clear_and_free_semaphores(list(self.sems.allocated().values()))
        self.nc.all_engine_barrier()

    tile.TileContext._drain_and_barrier = _patched

    f32 = mybir.dt.float32
    f16 = mybir.dt.float16
    i32 = mybir.dt.int32
    sb = bt * 128
    nbw = ktiles // bt

    nc = bass.Bass()
    gidx_d = nc.dram_tensor("gidx", [nb, 128, 2 * bt], i32, kind="ExternalInput")
    locv_d = nc.dram_tensor("locv", [nb, 128, bt], f16, kind="ExternalInput")
    xt_d = nc.dram_tensor("xt", [nb, 4, sb], f16, kind="ExternalInput")
    htab_d = nc.dram_tensor("htab", [ntot, D], f16, kind="ExternalInput")
    iota_d = nc.dram_tensor("iota", [128, 128], f16, kind="ExternalInput")
    wts = {}
    for p in ("f", "t"):
        wts[p + "w0h"] = nc.dram_tensor(p + "w0h", [128, H], f16, kind="ExternalInput")
        wts[p + "w0x"] = nc.dram_tensor(p + "w0x", [4, H], f16, kind="ExternalInput")
        wts[p + "w1"] = nc.dram_tensor(p + "w1", [H, H], f16, kind="ExternalInput")
        wts[p + "w2"] = nc.dram_tensor(p + "w2", [H, D], f16, kind="ExternalInput")
        wts[p + "b0"] = nc.dram_tensor(p + "b0", [H, 1], f32, kind="ExternalInput")
        wts[p + "b1"] = nc.dram_tensor(p + "b1", [H, 1], f32, kind="ExternalInput")
        wts[p + "b2r"] = nc.dram_tensor(p + "b2r", [128, bt * D], f16, kind="ExternalInput")
    delta_d = nc.dram_tensor("delta", [ns, D], f32, kind="ExternalOutput")

    Tanh = mybir.ActivationFunctionType.Tanh
    with tile.TileContext(nc) as tc:
        with tc.tile_pool(name="wpool", bufs=1) as wp, \
             tc.tile_pool(name="io", bufs=3) as io, \
             tc.tile_pool(name="act", bufs=2) as ap_, \
             tc.tile_pool(name="ps01", bufs=1, space="PSUM") as ps01, \
             tc.tile_pool(name="psd", bufs=2, space="PSUM") as psdp, \
             tc.tile_pool(name="win", bufs=2, space="PSUM") as winp:
            wt = {}
            for k, dr in wts.items():
                tl = wp.tile(list(dr.shape), dr.dtype, tag="w" + k)
                nc.sync.dma_start(out=tl[:], in_=dr[:])
                wt[k] = tl
            iota = wp.tile([128, 128], f16, tag="iota")
            nc.sync.dma_start(out=iota[:], in_=iota_d[:])

            for w in range(nwin):
                win = winp.tile([128, D], f32, tag="win")
                for pi, p in enumerate(("f", "t")):
                    for bi in range(nbw):
                        b = (w * 2 + pi) * nbw + bi
                        gi = io.tile([128, 2 * bt], i32, tag="gi")
                        lo = io.tile([128, bt], f16, tag="lo")
                        xb = io.tile([4, sb], f16, tag="xb")
                        nc.sync.dma_start(out=gi[:], in_=gidx_d[b])
                        nc.sync.dma_start(out=lo[:], in_=locv_d[b])
                        nc.sync.dma_start(out=xb[:], in_=xt_d[b])
                        gp = io.tile([128, 2 * bt, D], f16, tag="gp")
                        # HW DGE handles one offset per partition per
                        # instruction; fan out over the 2*bt columns.
                        for j in range(2 * bt):
                            nc.gpsimd.indirect_dma_start(
                                out=gp[:, j, :],
                                out_offset=None,
                                in_=htab_d[:],
                                in_offset=bass.IndirectOffsetOnAxis(
                                    ap=gi[:, j:j + 1], axis=0),
                            )
                        rhs = ap_.tile([128, sb], f16, tag="rhs")
                        for t in range(bt):
                            nc.sync.dma_start_transpose(
                                out=rhs[:, t * 128:(t + 1) * 128],
                                in_=gp[:, 2 * t:2 * t + 2, :])
                        ps0 = ps01.tile([128, sb], f32, tag="ps0")
                        nc.tensor.matmul(out=ps0[:], lhsT=wt[p + "w0h"][:],
                                         rhs=rhs[:], start=True, stop=False)
                        nc.tensor.matmul(out=ps0[:], lhsT=wt[p + "w0x"][:],
                                         rhs=xb[:], start=False, stop=True)
                        h1 = ap_.tile([128, sb], f16, tag="h1")
                        nc.scalar.activation(h1[:], ps0[:], Tanh,
                                             bias=wt[p + "b0"][:, 0:1])
                        ps1 = ps01.tile([128, sb], f32, tag="ps1")
                        nc.tensor.matmul(out=ps1[:], lhsT=wt[p + "w1"][:],
                                         rhs=h1[:], start=True, stop=True)
                        h2 = ap_.tile([128, sb], f16, tag="h2")
                        nc.scalar.activation(h2[:], ps1[:], Tanh,
                                             bias=wt[p + "b1"][:, 0:1])
                        psd = psdp.tile([128, bt * D], f32, tag="psd")
                        for t in range(bt):
                            nc.tensor.matmul(out=psd[:, t * D:(t + 1) * D],
                                             lhsT=h2[:, t * 128:(t + 1) * 128],
                                             rhs=wt[p + "w2"][:],
                                             start=True, stop=True)
                        dsb = ap_.tile([128, bt * D], f16, tag="dsb")
                        nc.vector.tensor_tensor(out=dsb[:], in0=psd[:],
                                                in1=wt[p + "b2r"][:],
                                                op=mybir.AluOpType.add)
                        dtl = ap_.tile([128, bt * D], f16, tag="dtl")
                        nc.scalar.activation(dtl[:], dsb[:], Tanh)
                        oh = ap_.tile([128, bt, 128], f16, tag="oh")
                        for t in range(bt):
                            nc.vector.tensor_tensor(
                                out=oh[:, t, :],
                                in0=lo[:, t:t + 1].to_broadcast([128, 128]),
                                in1=iota[:],
                                op=mybir.AluOpType.is_equal)
                        for t in range(bt):
                            nc.tensor.matmul(
                                out=win[:],
                                lhsT=oh[:, t, :],
                                rhs=dtl[:, t * D:(t + 1) * D],
                                start=(pi == 0 and bi == 0 and t == 0),
                                stop=(pi == 1 and bi == nbw - 1 and t == bt - 1))
                rows = lastrows if w == nwin - 1 else 128
                wout = ap_.tile([128, D], f32, tag="wout")
                nc.vector.tensor_copy(out=wout[:], in_=win[:])
                nc.sync.dma_start(out=delta_d[w * 128:w * 128 + rows, :],
                                  in_=wout[0:rows, :])

    # this walrus rejects any compute instruction carrying >1 sem wait;
    # hoist extra waits onto same-engine nops placed just before it.
    if not walrus_fix:
        return nc
    ctr = 0
    for bb in nc.main_func.blocks:
        new = []
        for ins in bb.instructions:
            si = getattr(ins, "sync_info", None)
            if si is not None and si.on_wait and len(si.on_wait) > 1:
                waits = list(si.on_wait)
                si.on_wait = [waits[-1]]
                for wv in waits[:-1]:
                    ctr += 1
                    nop = mybir.InstNoOp(
                        name=f"wsplit-{ctr}", engine=ins.engine, ins=[], outs=[],
                        sync_info=mybir.SyncInfo(on_wait=[wv], on_update=[]))
                    new.append(nop)
            new.append(ins)
        bb.instructions[:] = new
    return nc


def _prep_slots(af, at, x_local, ncores, ns, nwin, ktiles, bt, nb):
    """Build per-core padded slot arrays. Returns (gidx, locv, xt, spills)
    with gidx [ncores, NB, 128, 2bt] i32, locv [...bt] f16, xt [..., 4, SB] f16,
    spills = list of (pop, edge_indices) that overflowed window capacity."""
    sb = bt * 128
    nbw = ktiles // bt
    cap = ktiles * 128
    gidx = np.zeros((ncores, nb, 128, 2 * bt), np.int32)
    locv = np.full((ncores, nb, 128, bt), 128.0, np.float16)
    xt = np.zeros((ncores, nb, 4, sb), np.float16)
    xtv = np.ascontiguousarray(x_local.astype(np.float16))
    spills = []
    for pi, dest in enumerate((af, at)):
        core = dest // ns
        node_l = dest - core * ns
        w = node_l >> 7
        loc = node_l & 127
        cw = core * nwin + w
        order = np.argsort(cw, kind="stable")
        cws = cw[order]
        counts = np.bincount(cws, minlength=ncores * nwin)
        starts = np.concatenate([[0], np.cumsum(counts)[:-1]])
        rank = np.arange(len(cws)) - np.repeat(starts, counts)
        ok = rank < cap
        if not ok.all():
            spills.append((pi, order[~ok]))
        e_ok = order[ok]
        r = rank[ok]
        c_ok = core[e_ok]
        w_ok = w[e_ok]
        # slot within core: window block of 2*cap, population block of cap
        s = w_ok * (2 * cap) + pi * cap + r
        b = s // sb
        t = (s % sb) // 128
        pp = s % 128
        gidx[c_ok, b, pp, 2 * t] = af[e_ok]
        gidx[c_ok, b, pp, 2 * t + 1] = at[e_ok]
        locv[c_ok, b, pp, t] = loc[e_ok].astype(np.float16)
        xt[c_ok, b, :, t * 128 + pp] = xtv[e_ok]
    return gidx, locv, xt, spills


def _prep_weights(inputs, bt):
    const = np.concatenate([np.asarray(inputs["h_global"], np.float32).ravel(),
                            np.asarray(inputs["x_global"], np.float32).ravel(),
                            np.asarray(inputs["t"], np.float32).ravel()])
    wm = {}
    for p in ("f", "t"):
        W0 = np.asarray(inputs[p + "_W0"], np.float32)
        b0 = np.asarray(inputs[p + "_b0"], np.float32)
        b0eff = b0 + const @ W0[132:153]
        wm[p + "w0h"] = np.ascontiguousarray(W0[0:128]).astype(np.float16)
        wm[p + "w0x"] = np.ascontiguousarray(W0[128:132]).astype(np.float16)
        wm[p + "w1"] = np.asarray(inputs[p + "_W1"], np.float32).astype(np.float16)
        wm[p + "w2"] = np.asarray(inputs[p + "_W2"], np.float32).astype(np.float16)
        wm[p + "b0"] = b0eff.reshape(H, 1).astype(np.float32)
        wm[p + "b1"] = np.asarray(inputs[p + "_b1"], np.float32).reshape(H, 1)
        b2 = np.asarray(inputs[p + "_b2"], np.float32)
        wm[p + "b2r"] = np.tile(b2.reshape(1, D), (128, bt)).astype(np.float16)
    wm["iota"] = np.broadcast_to(np.arange(128, dtype=np.float16), (128, 128)).copy()
    return wm


def _spill_delta(spills, inputs, delta):
    if not spills:
        return
    af = np.asarray(inputs["addr_from"]).astype(np.int64)
    at = np.asarray(inputs["addr_to"]).astype(np.int64)
    h = np.asarray(inputs["h_local"], np.float32)
    x = np.asarray(inputs["x_local"], np.float32)
    const = np.concatenate([np.asarray(inputs["h_global"], np.float32).ravel(),
                            np.asarray(inputs["x_global"], np.float32).ravel(),
                            np.asarray(inputs["t"], np.float32).ravel()])
    for pi, edges in spills:
        p = "f" if pi == 0 else "t"
        inp = np.concatenate([h[af[edges]], h[at[edges]], x[edges],
                              np.broadcast_to(const, (len(edges), 21))], axis=1)
        d = np.tanh(np.tanh(np.tanh(
            inp @ inputs[p + "_W0"] + inputs[p + "_b0"]) @ inputs[p + "_W1"]
            + inputs[p + "_b1"]) @ inputs[p + "_W2"] + inputs[p + "_b2"])
        dest = af[edges] if pi == 0 else at[edges]
        _scatter_add(delta, dest, d.astype(np.float32))


def _kernel_bass(addr_from, addr_to, h_local, h_global, x_local, x_global, t,
                 f_W0, f_b0, f_W1, f_b1, f_W2, f_b2,
                 t_W0, t_b0, t_W1, t_b1, t_W2, t_b2, trace=False):
    import sys
    if "/opt/trn_rl_repo" not in sys.path:
        sys.path.insert(0, "/opt/trn_rl_repo")
    from concourse.bass_utils import run_bass_kernel_spmd

    inputs = dict(addr_from=addr_from, addr_to=addr_to, h_local=h_local,
                  h_global=h_global, x_local=x_local, x_global=x_global, t=t,
                  f_W0=f_W0, f_b0=f_b0, f_W1=f_W1, f_b1=f_b1, f_W2=f_W2,
                  f_b2=f_b2, t_W0=t_W0, t_b0=t_b0, t_W1=t_W1, t_b1=t_b1,
                  t_W2=t_W2, t_b2=t_b2)
    af = np.asarray(addr_from).astype(np.int64)
    at = np.asarray(addr_to).astype(np.int64)
    h = np.asarray(h_local, np.float32)
    x = np.asarray(x_local, np.float32)

    key = (N, E)
    if key not in _BASS_CACHE:
        _BASS_CACHE[key] = _build_bass(NS, NWIN, LASTROWS, KTILES, BT, NB, N)
    nc = _BASS_CACHE[key]

    gidx, locv, xt, spills = _prep_slots(af, at, x, NCORES, NS, NWIN,
                                         KTILES, BT, NB)
    wm = _prep_weights(inputs, BT)
    htab = np.ascontiguousarray(h.astype(np.float16))

    in_maps = []
    for c in range(NCORES):
        m = {"gidx": gidx[c], "locv": locv[c], "xt": xt[c], "htab": htab}
        m.update(wm)
        in_maps.append(m)

    res = run_bass_kernel_spmd(nc, in_maps, core_ids=list(range(NCORES)),
                               trace=trace)
    delta = np.concatenate([res.results[c]["delta"] for c in range(NCORES)],
                           axis=0)
    _spill_delta(spills, inputs, delta)
    out = np.tanh(delta).astype(np.float32)
    if trace:
        return out, res
    return out


def kernel(**inputs):
    try:
        return _kernel_bass(**inputs)
    except Exception:
        import traceback
        traceback.print_exc()
        return _kernel_numpy(**inputs)


# revision 13
# speedup vs baseline: 6.6605x; 1.1833x over previous
import numpy as np

# nn_LocalDynamics GNN message passing.
#   delta[n] = sum_e tanh(fMLP(inp_e))[addr_from=n] + tanh(tMLP(inp_e))[addr_to=n]
#   out = tanh(delta).  inp_e = [h[from], h[to], x_e, hg, xg, t] (153 dims).
#
# Destination-sharded design: each core owns nodes [c*12500, (c+1)*12500).
# Every edge yields two "slots": an f-slot on the core owning addr_from and a
# t-slot on the core owning addr_to.  Slots are grouped by 128-node windows of
# the owning core; each window holds a fixed K tiles of 128 slots per
# population (f/t), host-padded.  On device, per 512-slot batch:
#   indirect-DMA gather of (h[from], h[to]) row pairs -> XBAR transpose to
#   feature-major -> fp16 MLP -> slot-major final layer -> one-hot matmul
#   accumulates the window's delta in PSUM -> contiguous fp32 writes.
# Host applies the final tanh (and the overflow spill path, normally empty).

N = 100_000
E = 800_000
D = 64
H = 128
NCORES = 8
NS = N // NCORES            # nodes per core (12500)
NWIN = (NS + 127) // 128    # windows per core (98)
LASTROWS = NS - (NWIN - 1) * 128   # rows in last window (84)
KTILES = 12                 # 128-slot tiles per population per window
BT = 4                      # tiles per batch
SB = BT * 128               # slots per batch (512)
NBW = KTILES // BT          # batches per population-window (3)
NB = NWIN * 2 * NBW         # batches per core (588)
SLOTS = NB * SB             # padded slots per core (301056)


def _scatter_add(delta, idx, vals):
    o = np.argsort(idx, kind="stable")
    si = idx[o]
    sv = vals[o]
    starts = np.flatnonzero(np.r_[True, si[1:] != si[:-1]])
    sums = np.add.reduceat(sv, starts, axis=0)
    np.add.at(delta, si[starts], sums)


def _kernel_numpy(addr_from, addr_to, h_local, h_global, x_local, x_global, t,
                  f_W0, f_b0, f_W1, f_b1, f_W2, f_b2,
                  t_W0, t_b0, t_W1, t_b1, t_W2, t_b2):
    af = np.asarray(addr_from).astype(np.int64)
    at = np.asarray(addr_to).astype(np.int64)
    h_local = np.asarray(h_local, dtype=np.float32)
    x_local = np.asarray(x_local, dtype=np.float32)
    const = np.concatenate([np.asarray(h_global, np.float32).ravel(),
                            np.asarray(x_global, np.float32).ravel(),
                            np.asarray(t, np.float32).ravel()])
    ne = af.shape[0]
    delta = np.zeros((h_local.shape[0], D), np.float32)
    CH = 100_000
    for s in range(0, ne, CH):
        e = min(s + CH, ne)
        inp = np.concatenate([h_local[af[s:e]], h_local[at[s:e]], x_local[s:e],
                              np.broadcast_to(const, (e - s, 21))], axis=1).astype(np.float32)
        d_f = np.tanh(np.tanh(np.tanh(inp @ f_W0 + f_b0) @ f_W1 + f_b1) @ f_W2 + f_b2)
        d_t = np.tanh(np.tanh(np.tanh(inp @ t_W0 + t_b0) @ t_W1 + t_b1) @ t_W2 + t_b2)
        _scatter_add(delta, af[s:e], d_f.astype(np.float32))
        _scatter_add(delta, at[s:e], d_t.astype(np.float32))
    return np.tanh(delta).astype(np.float32)


_BASS_CACHE = {}


def _build_bass(ns, nwin, lastrows, ktiles, bt, nb, ntot, ncores,
                walrus_fix=True):
    import concourse.bass as bass
    import concourse.mybir as mybir
    import concourse.tile as tile

    # walrus in this env rejects Drain instructions carrying >1 sem wait;
    # move each wait onto its own sync nop before the drain.
    def _patched(self, tick_clock, wait_clock):
        from concourse.tile import ScopedClock
        nop0 = self.nc.sync.nop(nofuse=True)
        wait_clock.add_sem_waits(nop0.ins, ScopedClock({None: tick_clock.global_clock}))
        si = nop0.ins.sync_info
        if si is not None and si.on_wait and len(si.on_wait) > 1:
            waits = list(si.on_wait)
            si.on_wait = waits[:1]
            for w in waits[1:]:
                n = self.nc.sync.nop(nofuse=True)
                n.ins.sync_info = mybir.SyncInfo(on_wait=[w], on_update=[])
        self.nc.sync.drain()
        self.nc.all_engine_barrier()
        popped = self.nc._tile_sem_poison_stack.pop()
        assert popped is self._sem_poison
        self.nc.clear_and_free_semaphores(list(self.sems.allocated().values()))
        self.nc.all_engine_barrier()

    tile.TileContext._drain_and_barrier = _patched

    f32 = mybir.dt.float32
    f16 = mybir.dt.float16
    i32 = mybir.dt.int32
    sb = bt * 128
    nbw = ktiles // bt

    nc = bass.Bass(num_devices=ncores)
    gidx_d = nc.dram_tensor("gidx", [nb, 128, 2 * bt], i32, kind="ExternalInput")
    locv_d = nc.dram_tensor("locv", [nb, 128, bt], f16, kind="ExternalInput")
    xt_d = nc.dram_tensor("xt", [nb, 4, sb], f16, kind="ExternalInput")
    hshard_d = nc.dram_tensor("hshard", [ns, D], f16, kind="ExternalInput")
    hsh_b = nc.dram_tensor("hsh_b", [ns, D], f16)
    htab_d = nc.dram_tensor("hfull", [ntot, D], f16, addr_space="Shared")
    iota_d = nc.dram_tensor("iota", [128, 128], f16, kind="ExternalInput")
    wts = {}
    for p in ("f", "t"):
        wts[p + "w0h"] = nc.dram_tensor(p + "w0h", [128, H], f16, kind="ExternalInput")
        wts[p + "w0x"] = nc.dram_tensor(p + "w0x", [4, H], f16, kind="ExternalInput")
        wts[p + "w1"] = nc.dram_tensor(p + "w1", [H, H], f16, kind="ExternalInput")
        wts[p + "w2"] = nc.dram_tensor(p + "w2", [H, D], f16, kind="ExternalInput")
        wts[p + "b0"] = nc.dram_tensor(p + "b0", [H, 1], f32, kind="ExternalInput")
        wts[p + "b1"] = nc.dram_tensor(p + "b1", [H, 1], f32, kind="ExternalInput")
        wts[p + "b2r"] = nc.dram_tensor(p + "b2r", [128, bt * D], f16, kind="ExternalInput")
    delta_d = nc.dram_tensor("delta", [ns, D], f32, kind="ExternalOutput")

    # all-gather the h shards into a full replicated table before the main
    # body; runs on the gpsimd stream, which also issues the gathers later,
    # so engine program order guarantees completion before first use.
    with nc.Block() as blk, \
         nc.semaphore("ag_dma") as ag_dma, \
         nc.semaphore("ag_cc") as ag_cc:

        @blk.gpsimd
        def _(g):
            g.dma_start(out=hsh_b[:, :], in_=hshard_d[:, :]).then_inc(ag_dma, 16)
            g.wait_ge(ag_dma, 16)
            g.collective_compute(
                "AllGather",
                mybir.AluOpType.bypass,
                replica_groups=[list(range(ncores))],
                ins=[hsh_b.ap().opt()],
                outs=[htab_d.ap().opt()],
            ).then_inc(ag_cc)
            g.wait_ge(ag_cc, 1)

    Tanh = mybir.ActivationFunctionType.Tanh
    with tile.TileContext(nc) as tc:
        with tc.tile_pool(name="wpool", bufs=1) as wp, \
             tc.tile_pool(name="io", bufs=3) as io, \
             tc.tile_pool(name="act", bufs=2) as ap_, \
             tc.tile_pool(name="ps01", bufs=1, space="PSUM") as ps01, \
             tc.tile_pool(name="psd", bufs=2, space="PSUM") as psdp, \
             tc.tile_pool(name="win", bufs=2, space="PSUM") as winp:
            wt = {}
            for k, dr in wts.items():
                tl = wp.tile(list(dr.shape), dr.dtype, tag="w" + k)
                nc.sync.dma_start(out=tl[:], in_=dr[:])
                wt[k] = tl
            iota = wp.tile([128, 128], f16, tag="iota")
            nc.sync.dma_start(out=iota[:], in_=iota_d[:])

            for w in range(nwin):
                win = winp.tile([128, D], f32, tag="win")
                for pi, p in enumerate(("f", "t")):
                    for bi in range(nbw):
                        b = (w * 2 + pi) * nbw + bi
                        gi = io.tile([128, 2 * bt], i32, tag="gi")
                        lo = io.tile([128, bt], f16, tag="lo")
                        xb = io.tile([4, sb], f16, tag="xb")
                        nc.sync.dma_start(out=gi[:], in_=gidx_d[b])
                        nc.sync.dma_start(out=lo[:], in_=locv_d[b])
                        nc.sync.dma_start(out=xb[:], in_=xt_d[b])
                        gp = io.tile([128, 2 * bt, D], f16, tag="gp")
                        # HW DGE handles one offset per partition per
                        # instruction; fan out over the 2*bt columns.
                        for j in range(2 * bt):
                            nc.gpsimd.indirect_dma_start(
                                out=gp[:, j, :],
                                out_offset=None,
                                in_=htab_d[:],
                                in_offset=bass.IndirectOffsetOnAxis(
                                    ap=gi[:, j:j + 1], axis=0),
                            )
                        rhs = ap_.tile([128, sb], f16, tag="rhs")
                        for t in range(bt):
                            nc.sync.dma_start_transpose(
                                out=rhs[:, t * 128:(t + 1) * 128],
                                in_=gp[:, 2 * t:2 * t + 2, :])
                        ps0 = ps01.tile([128, sb], f32, tag="ps0")
                        nc.tensor.matmul(out=ps0[:], lhsT=wt[p + "w0h"][:],
                                         rhs=rhs[:], start=True, stop=False)
                        nc.tensor.matmul(out=ps0[:], lhsT=wt[p + "w0x"][:],
                                         rhs=xb[:], start=False, stop=True)
                        h1 = ap_.tile([128, sb], f16, tag="h1")
                        nc.scalar.activation(h1[:], ps0[:], Tanh,
                                             bias=wt[p + "b0"][:, 0:1])
                        ps1 = ps01.tile([128, sb], f32, tag="ps1")
                        nc.tensor.matmul(out=ps1[:], lhsT=wt[p + "w1"][:],
                                         rhs=h1[:], start=True, stop=True)
                        h2 = ap_.tile([128, sb], f16, tag="h2")
                        nc.scalar.activation(h2[:], ps1[:], Tanh,
                                             bias=wt[p + "b1"][:, 0:1])
                        psd = psdp.tile([128, bt * D], f32, tag="psd")
                        for t in range(bt):
                            nc.tensor.matmul(out=psd[:, t * D:(t + 1) * D],
                                             lhsT=h2[:, t * 128:(t + 1) * 128],
                                             rhs=wt[p + "w2"][:],
                                             start=True, stop=True)
                        dsb = ap_.tile([128, bt * D], f16, tag="dsb")
                        nc.vector.tensor_tensor(out=dsb[:], in0=psd[:],
                                                in1=wt[p + "b2r"][:],
                                                op=mybir.AluOpType.add)
                        dtl = ap_.tile([128, bt * D], f16, tag="dtl")
                        nc.scalar.activation(dtl[:], dsb[:], Tanh)
                        oh = ap_.tile([128, bt, 128], f16, tag="oh")
                        for t in range(bt):
                            nc.vector.tensor_tensor(
                                out=oh[:, t, :],
                                in0=lo[:, t:t + 1].to_broadcast([128, 128]),
                                in1=iota[:],
                                op=mybir.AluOpType.is_equal)
                        for t in range(bt):
                            nc.tensor.matmul(
                                out=win[:],
                                lhsT=oh[:, t, :],
                                rhs=dtl[:, t * D:(t + 1) * D],
                                start=(pi == 0 and bi == 0 and t == 0),
                                stop=(pi == 1 and bi == nbw - 1 and t == bt - 1))
                rows = lastrows if w == nwin - 1 else 128
                wout = ap_.tile([128, D], f32, tag="wout")
                nc.vector.tensor_copy(out=wout[:], in_=win[:])
                nc.sync.dma_start(out=delta_d[w * 128:w * 128 + rows, :],
                                  in_=wout[0:rows, :])

    # this walrus rejects any compute instruction carrying >1 sem wait;
    # hoist extra waits onto same-engine nops placed just before it.
    if not walrus_fix:
        return nc
    ctr = 0
    for bb in nc.main_func.blocks:
        new = []
        for ins in bb.instructions:
            si = getattr(ins, "sync_info", None)
            if si is not None and si.on_wait and len(si.on_wait) > 1:
                waits = list(si.on_wait)
                si.on_wait = [waits[-1]]
                for wv in waits[:-1]:
                    ctr += 1
                    nop = mybir.InstNoOp(
                        name=f"wsplit-{ctr}", engine=ins.engine, ins=[], outs=[],
                        sync_info=mybir.SyncInfo(on_wait=[wv], on_update=[]))
                    new.append(nop)
            new.append(ins)
        bb.instructions[:] = new
    return nc


def _prep_slots(af, at, x_local, ncores, ns, nwin, ktiles, bt, nb):
    """Build per-core padded slot arrays. Returns (gidx, locv, xt, spills)
    with gidx [ncores, NB, 128, 2bt] i32, locv [...bt] f16, xt [..., 4, SB] f16,
    spills = list of (pop, edge_indices) that overflowed window capacity."""
    sb = bt * 128
    nbw = ktiles // bt
    cap = ktiles * 128
    gidx = np.zeros((ncores, nb, 128, 2 * bt), np.int32)
    locv = np.full((ncores, nb, 128, bt), 128.0, np.float16)
    xt = np.zeros((ncores, nb, 4, sb), np.float16)
    xtv = np.ascontiguousarray(x_local.astype(np.float16))
    spills = []
    for pi, dest in enumerate((af, at)):
        core = dest // ns
        node_l = dest - core * ns
        w = node_l >> 7
        loc = node_l & 127
        cw = core * nwin + w
        order = np.argsort(cw, kind="stable")
        cws = cw[order]
        counts = np.bincount(cws, minlength=ncores * nwin)
        starts = np.concatenate([[0], np.cumsum(counts)[:-1]])
        rank = np.arange(len(cws)) - np.repeat(starts, counts)
        ok = rank < cap
        if not ok.all():
            spills.append((pi, order[~ok]))
        e_ok = order[ok]
        r = rank[ok]
        c_ok = core[e_ok]
        w_ok = w[e_ok]
        # slot within core: window block of 2*cap, population block of cap
        s = w_ok * (2 * cap) + pi * cap + r
        b = s // sb
        t = (s % sb) // 128
        pp = s % 128
        gidx[c_ok, b, pp, 2 * t] = af[e_ok]
        gidx[c_ok, b, pp, 2 * t + 1] = at[e_ok]
        locv[c_ok, b, pp, t] = loc[e_ok].astype(np.float16)
        xt[c_ok, b, :, t * 128 + pp] = xtv[e_ok]
    return gidx, locv, xt, spills


def _prep_weights(inputs, bt):
    const = np.concatenate([np.asarray(inputs["h_global"], np.float32).ravel(),
                            np.asarray(inputs["x_global"], np.float32).ravel(),
                            np.asarray(inputs["t"], np.float32).ravel()])
    wm = {}
    for p in ("f", "t"):
        W0 = np.asarray(inputs[p + "_W0"], np.float32)
        b0 = np.asarray(inputs[p + "_b0"], np.float32)
        b0eff = b0 + const @ W0[132:153]
        wm[p + "w0h"] = np.ascontiguousarray(W0[0:128]).astype(np.float16)
        wm[p + "w0x"] = np.ascontiguousarray(W0[128:132]).astype(np.float16)
        wm[p + "w1"] = np.asarray(inputs[p + "_W1"], np.float32).astype(np.float16)
        wm[p + "w2"] = np.asarray(inputs[p + "_W2"], np.float32).astype(np.float16)
        wm[p + "b0"] = b0eff.reshape(H, 1).astype(np.float32)
        wm[p + "b1"] = np.asarray(inputs[p + "_b1"], np.float32).reshape(H, 1)
        b2 = np.asarray(inputs[p + "_b2"], np.float32)
        wm[p + "b2r"] = np.tile(b2.reshape(1, D), (128, bt)).astype(np.float16)
    wm["iota"] = np.broadcast_to(np.arange(128, dtype=np.float16), (128, 128)).copy()
    return wm


def _spill_delta(spills, inputs, delta):
    if not spills:
        return
    af = np.asarray(inputs["addr_from"]).astype(np.int64)
    at = np.asarray(inputs["addr_to"]).astype(np.int64)
    h = np.asarray(inputs["h_local"], np.float32)
    x = np.asarray(inputs["x_local"], np.float32)
    const = np.concatenate([np.asarray(inputs["h_global"], np.float32).ravel(),
                            np.asarray(inputs["x_global"], np.float32).ravel(),
                            np.asarray(inputs["t"], np.float32).ravel()])
    for pi, edges in spills:
        p = "f" if pi == 0 else "t"
        inp = np.concatenate([h[af[edges]], h[at[edges]], x[edges],
                              np.broadcast_to(const, (len(edges), 21))], axis=1)
        d = np.tanh(np.tanh(np.tanh(
            inp @ inputs[p + "_W0"] + inputs[p + "_b0"]) @ inputs[p + "_W1"]
            + inputs[p + "_b1"]) @ inputs[p + "_W2"] + inputs[p + "_b2"])
        dest = af[edges] if pi == 0 else at[edges]
        _scatter_add(delta, dest, d.astype(np.float32))


def _kernel_bass(addr_from, addr_to, h_local, h_global, x_local, x_global, t,
                 f_W0, f_b0, f_W1, f_b1, f_W2, f_b2,
                 t_W0, t_b0, t_W1, t_b1, t_W2, t_b2, trace=False):
    import sys
    if "/opt/trn_rl_repo" not in sys.path:
        sys.path.insert(0, "/opt/trn_rl_repo")
    from concourse.bass_utils import run_bass_kernel_spmd

    inputs = dict(addr_from=addr_from, addr_to=addr_to, h_local=h_local,
                  h_global=h_global, x_local=x_local, x_global=x_global, t=t,
                  f_W0=f_W0, f_b0=f_b0, f_W1=f_W1, f_b1=f_b1, f_W2=f_W2,
                  f_b2=f_b2, t_W0=t_W0, t_b0=t_b0, t_W1=t_W1, t_b1=t_b1,
                  t_W2=t_W2, t_b2=t_b2)
    af = np.asarray(addr_from).astype(np.int64)
    at = np.asarray(addr_to).astype(np.int64)
    h = np.asarray(h_local, np.float32)
    x = np.asarray(x_local, np.float32)

    key = (N, E)
    if key not in _BASS_CACHE:
        _BASS_CACHE[key] = _build_bass(NS, NWIN, LASTROWS, KTILES, BT, NB, N,
                                       NCORES)
    nc = _BASS_CACHE[key]

    gidx, locv, xt, spills = _prep_slots(af, at, x, NCORES, NS, NWIN,
                                         KTILES, BT, NB)
    wm = _prep_weights(inputs, BT)
    htab = np.ascontiguousarray(h.astype(np.float16))

    in_maps = []
    for c in range(NCORES):
        m = {"gidx": gidx[c], "locv": locv[c], "xt": xt[c],
             "hshard": htab[c * NS:(c + 1) * NS]}
        m.update(wm)
        in_maps.append(m)

    res = run_bass_kernel_spmd(nc, in_maps, core_ids=list(range(NCORES)),
                               trace=trace)
    delta = np.concatenate([res.results[c]["delta"] for c in range(NCORES)],
                           axis=0)
    _spill_delta(spills, inputs, delta)
    out = np.tanh(delta).astype(np.float32)
    if trace:
        return out, res
    return out


def kernel(**inputs):
    try:
        return _kernel_bass(**inputs)
    except Exception:
        import traceback
        traceback.print_exc()
        return _kernel_numpy(**inputs)


# revision 17
# speedup vs baseline: 7.7746x; 1.1673x over previous
import numpy as np

# nn_LocalDynamics GNN message passing.
#   delta[n] = sum_e tanh(fMLP(inp_e))[addr_from=n] + tanh(tMLP(inp_e))[addr_to=n]
#   out = tanh(delta).  inp_e = [h[from], h[to], x_e, hg, xg, t] (153 dims).
#
# Destination-sharded design: each core owns nodes [c*12500, (c+1)*12500).
# Every edge yields two "slots": an f-slot on the core owning addr_from and a
# t-slot on the core owning addr_to.  Slots are grouped by 128-node windows of
# the owning core; each window holds a fixed K tiles of 128 slots per
# population (f/t), host-padded.  On device, per 512-slot batch:
#   indirect-DMA gather of (h[from], h[to]) row pairs -> XBAR transpose to
#   feature-major -> fp16 MLP -> slot-major final layer -> one-hot matmul
#   accumulates the window's delta in PSUM -> contiguous fp32 writes.
# Host applies the final tanh (and the overflow spill path, normally empty).

N = 100_000
E = 800_000
D = 64
H = 128
NCORES = 8
NS = N // NCORES            # nodes per core (12500)
NWIN = (NS + 127) // 128    # windows per core (98)
LASTROWS = NS - (NWIN - 1) * 128   # rows in last window (84)
KTILES = 12                 # 128-slot tiles per population per window
BT = 4                      # tiles per batch
SB = BT * 128               # slots per batch (512)
NBW = KTILES // BT          # batches per population-window (3)
NB = NWIN * 2 * NBW         # batches per core (588)
SLOTS = NB * SB             # padded slots per core (301056)


def _scatter_add(delta, idx, vals):
    o = np.argsort(idx, kind="stable")
    si = idx[o]
    sv = vals[o]
    starts = np.flatnonzero(np.r_[True, si[1:] != si[:-1]])
    sums = np.add.reduceat(sv, starts, axis=0)
    np.add.at(delta, si[starts], sums)


def _kernel_numpy(addr_from, addr_to, h_local, h_global, x_local, x_global, t,
                  f_W0, f_b0, f_W1, f_b1, f_W2, f_b2,
                  t_W0, t_b0, t_W1, t_b1, t_W2, t_b2):
    af = np.asarray(addr_from).astype(np.int64)
    at = np.asarray(addr_to).astype(np.int64)
    h_local = np.asarray(h_local, dtype=np.float32)
    x_local = np.asarray(x_local, dtype=np.float32)
    const = np.concatenate([np.asarray(h_global, np.float32).ravel(),
                            np.asarray(x_global, np.float32).ravel(),
                            np.asarray(t, np.float32).ravel()])
    ne = af.shape[0]
    delta = np.zeros((h_local.shape[0], D), np.float32)
    CH = 100_000
    for s in range(0, ne, CH):
        e = min(s + CH, ne)
        inp = np.concatenate([h_local[af[s:e]], h_local[at[s:e]], x_local[s:e],
                              np.broadcast_to(const, (e - s, 21))], axis=1).astype(np.float32)
        d_f = np.tanh(np.tanh(np.tanh(inp @ f_W0 + f_b0) @ f_W1 + f_b1) @ f_W2 + f_b2)
        d_t = np.tanh(np.tanh(np.tanh(inp @ t_W0 + t_b0) @ t_W1 + t_b1) @ t_W2 + t_b2)
        _scatter_add(delta, af[s:e], d_f.astype(np.float32))
        _scatter_add(delta, at[s:e], d_t.astype(np.float32))
    return np.tanh(delta).astype(np.float32)


_BASS_CACHE = {}


def _build_bass(ns, nwin, lastrows, ktiles, bt, nb, ntot, ncores,
                walrus_fix=True):
    import concourse.bass as bass
    import concourse.mybir as mybir
    import concourse.tile as tile

    # walrus in this env rejects Drain instructions carrying >1 sem wait;
    # move each wait onto its own sync nop before the drain.
    def _patched(self, tick_clock, wait_clock):
        from concourse.tile import ScopedClock
        nop0 = self.nc.sync.nop(nofuse=True)
        wait_clock.add_sem_waits(nop0.ins, ScopedClock({None: tick_clock.global_clock}))
        si = nop0.ins.sync_info
        if si is not None and si.on_wait and len(si.on_wait) > 1:
            waits = list(si.on_wait)
            si.on_wait = waits[:1]
            for w in waits[1:]:
                n = self.nc.sync.nop(nofuse=True)
                n.ins.sync_info = mybir.SyncInfo(on_wait=[w], on_update=[])
        self.nc.sync.drain()
        self.nc.all_engine_barrier()
        popped = self.nc._tile_sem_poison_stack.pop()
        assert popped is self._sem_poison
        self.nc.clear_and_free_semaphores(list(self.sems.allocated().values()))
        self.nc.all_engine_barrier()

    tile.TileContext._drain_and_barrier = _patched

    f32 = mybir.dt.float32
    f16 = mybir.dt.float16
    i32 = mybir.dt.int32
    sb = bt * 128
    nbw = ktiles // bt

    nc = bass.Bass(num_devices=ncores)
    gidx_d = nc.dram_tensor("gidx", [nb, 128, 2 * bt], i32, kind="ExternalInput")
    locv_d = nc.dram_tensor("locv", [nb, 128, bt], f16, kind="ExternalInput")
    xt_d = nc.dram_tensor("xt", [nb, 4, sb], f16, kind="ExternalInput")
    hshard_d = nc.dram_tensor("hshard", [ns, D], f16, kind="ExternalInput")
    hsh_b = nc.dram_tensor("hsh_b", [ns, D], f16)
    htab_d = nc.dram_tensor("hfull", [ntot, D], f16, addr_space="Shared")
    iota_d = nc.dram_tensor("iota", [128, 128], f16, kind="ExternalInput")
    wts = {}
    for p in ("f", "t"):
        wts[p + "w0h"] = nc.dram_tensor(p + "w0h", [128, H], f16, kind="ExternalInput")
        wts[p + "w0x"] = nc.dram_tensor(p + "w0x", [4, H], f16, kind="ExternalInput")
        wts[p + "w1"] = nc.dram_tensor(p + "w1", [H, H], f16, kind="ExternalInput")
        wts[p + "w2"] = nc.dram_tensor(p + "w2", [H, D], f16, kind="ExternalInput")
        wts[p + "b0"] = nc.dram_tensor(p + "b0", [H, 1], f32, kind="ExternalInput")
        wts[p + "b1"] = nc.dram_tensor(p + "b1", [H, 1], f32, kind="ExternalInput")
        wts[p + "b2r"] = nc.dram_tensor(p + "b2r", [128, bt * D], f16, kind="ExternalInput")
    delta_d = nc.dram_tensor("delta", [ns, D], f16, kind="ExternalOutput")

    # all-gather the h shards into a full replicated table before the main
    # body; runs on the gpsimd stream, which also issues the gathers later,
    # so engine program order guarantees completion before first use.
    with nc.Block() as blk, \
         nc.semaphore("ag_dma") as ag_dma, \
         nc.semaphore("ag_cc") as ag_cc:

        @blk.gpsimd
        def _(g):
            g.dma_start(out=hsh_b[:, :], in_=hshard_d[:, :]).then_inc(ag_dma, 16)
            g.wait_ge(ag_dma, 16)
            g.collective_compute(
                "AllGather",
                mybir.AluOpType.bypass,
                replica_groups=[list(range(ncores))],
                ins=[hsh_b.ap().opt()],
                outs=[htab_d.ap().opt()],
            ).then_inc(ag_cc)
            g.wait_ge(ag_cc, 1)

    Tanh = mybir.ActivationFunctionType.Tanh
    with tile.TileContext(nc) as tc:
        with tc.tile_pool(name="wpool", bufs=1) as wp, \
             tc.tile_pool(name="io", bufs=3) as io, \
             tc.tile_pool(name="act", bufs=2) as ap_, \
             tc.tile_pool(name="ps01", bufs=1, space="PSUM") as ps01, \
             tc.tile_pool(name="psd", bufs=2, space="PSUM") as psdp, \
             tc.tile_pool(name="win", bufs=2, space="PSUM") as winp:
            wt = {}
            for k, dr in wts.items():
                tl = wp.tile(list(dr.shape), dr.dtype, tag="w" + k)
                nc.sync.dma_start(out=tl[:], in_=dr[:])
                wt[k] = tl
            iota = wp.tile([128, 128], f16, tag="iota")
            nc.sync.dma_start(out=iota[:], in_=iota_d[:])

            for w in range(nwin):
                win = winp.tile([128, D], f32, tag="win")
                for pi, p in enumerate(("f", "t")):
                    for bi in range(nbw):
                        b = (w * 2 + pi) * nbw + bi
                        gi = io.tile([128, 2 * bt], i32, tag="gi")
                        lo = io.tile([128, bt], f16, tag="lo")
                        xb = io.tile([4, sb], f16, tag="xb")
                        nc.sync.dma_start(out=gi[:], in_=gidx_d[b])
                        nc.sync.dma_start(out=lo[:], in_=locv_d[b])
                        nc.sync.dma_start(out=xb[:], in_=xt_d[b])
                        gp = io.tile([128, 2 * bt, D], f16, tag="gp")
                        # HW DGE handles one offset per partition per
                        # instruction; fan out over the 2*bt columns.
                        for j in range(2 * bt):
                            nc.gpsimd.indirect_dma_start(
                                out=gp[:, j, :],
                                out_offset=None,
                                in_=htab_d[:],
                                in_offset=bass.IndirectOffsetOnAxis(
                                    ap=gi[:, j:j + 1], axis=0),
                            )
                        rhs = ap_.tile([128, sb], f16, tag="rhs")
                        for t in range(bt):
                            nc.sync.dma_start_transpose(
                                out=rhs[:, t * 128:(t + 1) * 128],
                                in_=gp[:, 2 * t:2 * t + 2, :])
                        ps0 = ps01.tile([128, sb], f32, tag="ps0")
                        nc.tensor.matmul(out=ps0[:], lhsT=wt[p + "w0h"][:],
                                         rhs=rhs[:], start=True, stop=False)
                        nc.tensor.matmul(out=ps0[:], lhsT=wt[p + "w0x"][:],
                                         rhs=xb[:], start=False, stop=True)
                        h1 = ap_.tile([128, sb], f16, tag="h1")
                        nc.scalar.activation(h1[:], ps0[:], Tanh,
                                             bias=wt[p + "b0"][:, 0:1])
                        ps1 = ps01.tile([128, sb], f32, tag="ps1")
                        nc.tensor.matmul(out=ps1[:], lhsT=wt[p + "w1"][:],
                                         rhs=h1[:], start=True, stop=True)
                        h2 = ap_.tile([128, sb], f16, tag="h2")
                        nc.scalar.activation(h2[:], ps1[:], Tanh,
                                             bias=wt[p + "b1"][:, 0:1])
                        psd = psdp.tile([128, bt * D], f32, tag="psd")
                        for t in range(bt):
                            nc.tensor.matmul(out=psd[:, t * D:(t + 1) * D],
                                             lhsT=h2[:, t * 128:(t + 1) * 128],
                                             rhs=wt[p + "w2"][:],
                                             start=True, stop=True)
                        dsb = ap_.tile([128, bt * D], f16, tag="dsb")
                        nc.vector.tensor_tensor(out=dsb[:], in0=psd[:],
                                                in1=wt[p + "b2r"][:],
                                                op=mybir.AluOpType.add)
                        dtl = ap_.tile([128, bt * D], f16, tag="dtl")
                        nc.scalar.activation(dtl[:], dsb[:], Tanh)
                        oh = ap_.tile([128, bt, 128], f16, tag="oh")
                        for t in range(bt):
                            nc.vector.tensor_tensor(
                                out=oh[:, t, :],
                                in0=lo[:, t:t + 1].to_broadcast([128, 128]),
                                in1=iota[:],
                                op=mybir.AluOpType.is_equal)
                        for t in range(bt):
                            nc.tensor.matmul(
                                out=win[:],
                                lhsT=oh[:, t, :],
                                rhs=dtl[:, t * D:(t + 1) * D],
                                start=(pi == 0 and bi == 0 and t == 0),
                                stop=(pi == 1 and bi == nbw - 1 and t == bt - 1))
                rows = lastrows if w == nwin - 1 else 128
                wout = ap_.tile([128, D], f16, tag="wout")
                nc.scalar.activation(wout[:], win[:], Tanh)
                nc.sync.dma_start(out=delta_d[w * 128:w * 128 + rows, :],
                                  in_=wout[0:rows, :])

    # this walrus rejects any compute instruction carrying >1 sem wait;
    # hoist extra waits onto same-engine nops placed just before it.
    if not walrus_fix:
        return nc
    ctr = 0
    for bb in nc.main_func.blocks:
        new = []
        for ins in bb.instructions:
            si = getattr(ins, "sync_info", None)
            if si is not None and si.on_wait and len(si.on_wait) > 1:
                waits = list(si.on_wait)
                si.on_wait = [waits[-1]]
                for wv in waits[:-1]:
                    ctr += 1
                    nop = mybir.InstNoOp(
                        name=f"wsplit-{ctr}", engine=ins.engine, ins=[], outs=[],
                        sync_info=mybir.SyncInfo(on_wait=[wv], on_update=[]))
                    new.append(nop)
            new.append(ins)
        bb.instructions[:] = new
    return nc


def _prep_slots(af, at, x_local, ncores, ns, nwin, ktiles, bt, nb):
    """Build per-core padded slot arrays. Returns (gidx, locv, xt, spills)
    with gidx [ncores, NB, 128, 2bt] i32, locv [...bt] f16, xt [..., 4, SB] f16,
    spills = list of (pop, edge_indices) that overflowed window capacity."""
    sb = bt * 128
    nbw = ktiles // bt
    cap = ktiles * 128
    gidx = np.zeros((ncores, nb, 128, 2 * bt), np.int32)
    locv = np.full((ncores, nb, 128, bt), 128.0, np.float16)
    xt = np.zeros((ncores, nb, 4, sb), np.float16)
    xtv = np.ascontiguousarray(x_local.astype(np.float16))
    spills = []
    for pi, dest in enumerate((af, at)):
        core = dest // ns
        node_l = dest - core * ns
        w = node_l >> 7
        loc = node_l & 127
        cw = core * nwin + w
        order = np.argsort(cw, kind="stable")
        cws = cw[order]
        counts = np.bincount(cws, minlength=ncores * nwin)
        starts = np.concatenate([[0], np.cumsum(counts)[:-1]])
        rank = np.arange(len(cws)) - np.repeat(starts, counts)
        ok = rank < cap
        if not ok.all():
            spills.append((pi, order[~ok]))
        e_ok = order[ok]
        r = rank[ok]
        c_ok = core[e_ok]
        w_ok = w[e_ok]
        # slot within core: window block of 2*cap, population block of cap
        s = w_ok * (2 * cap) + pi * cap + r
        b = s // sb
        t = (s % sb) // 128
        pp = s % 128
        gidx[c_ok, b, pp, 2 * t] = af[e_ok]
        gidx[c_ok, b, pp, 2 * t + 1] = at[e_ok]
        locv[c_ok, b, pp, t] = loc[e_ok].astype(np.float16)
        xt[c_ok, b, :, t * 128 + pp] = xtv[e_ok]
    return gidx, locv, xt, spills


def _prep_weights(inputs, bt):
    const = np.concatenate([np.asarray(inputs["h_global"], np.float32).ravel(),
                            np.asarray(inputs["x_global"], np.float32).ravel(),
                            np.asarray(inputs["t"], np.float32).ravel()])
    wm = {}
    for p in ("f", "t"):
        W0 = np.asarray(inputs[p + "_W0"], np.float32)
        b0 = np.asarray(inputs[p + "_b0"], np.float32)
        b0eff = b0 + const @ W0[132:153]
        wm[p + "w0h"] = np.ascontiguousarray(W0[0:128]).astype(np.float16)
        wm[p + "w0x"] = np.ascontiguousarray(W0[128:132]).astype(np.float16)
        wm[p + "w1"] = np.asarray(inputs[p + "_W1"], np.float32).astype(np.float16)
        wm[p + "w2"] = np.asarray(inputs[p + "_W2"], np.float32).astype(np.float16)
        wm[p + "b0"] = b0eff.reshape(H, 1).astype(np.float32)
        wm[p + "b1"] = np.asarray(inputs[p + "_b1"], np.float32).reshape(H, 1)
        b2 = np.asarray(inputs[p + "_b2"], np.float32)
        wm[p + "b2r"] = np.tile(b2.reshape(1, D), (128, bt)).astype(np.float16)
    wm["iota"] = np.broadcast_to(np.arange(128, dtype=np.float16), (128, 128)).copy()
    return wm


def _fix_spill_nodes(spills, inputs, out):
    """Recompute on host (fp32) every node whose window overflowed device
    capacity; overwrite those rows of `out`. Empty for uniform edge data."""
    if not spills:
        return
    af = np.asarray(inputs["addr_from"]).astype(np.int64)
    at = np.asarray(inputs["addr_to"]).astype(np.int64)
    h = np.asarray(inputs["h_local"], np.float32)
    x = np.asarray(inputs["x_local"], np.float32)
    const = np.concatenate([np.asarray(inputs["h_global"], np.float32).ravel(),
                            np.asarray(inputs["x_global"], np.float32).ravel(),
                            np.asarray(inputs["t"], np.float32).ravel()])
    nodes = np.unique(np.concatenate(
        [(af if pi == 0 else at)[e] for pi, e in spills]))
    node_set = np.zeros(h.shape[0], bool)
    node_set[nodes] = True
    delta = np.zeros((len(nodes), D), np.float32)
    remap = np.full(h.shape[0], -1, np.int64)
    remap[nodes] = np.arange(len(nodes))
    for pi, idx_all in ((0, af), (1, at)):
        p = "f" if pi == 0 else "t"
        edges = np.flatnonzero(node_set[idx_all])
        if not len(edges):
            continue
        inp = np.concatenate([h[af[edges]], h[at[edges]], x[edges],
                              np.broadcast_to(const, (len(edges), 21))], axis=1)
        d = np.tanh(np.tanh(np.tanh(
            inp @ inputs[p + "_W0"] + inputs[p + "_b0"]) @ inputs[p + "_W1"]
            + inputs[p + "_b1"]) @ inputs[p + "_W2"] + inputs[p + "_b2"])
        _scatter_add(delta, remap[idx_all[edges]], d.astype(np.float32))
    out[nodes] = np.tanh(delta)


def _kernel_bass(addr_from, addr_to, h_local, h_global, x_local, x_global, t,
                 f_W0, f_b0, f_W1, f_b1, f_W2, f_b2,
                 t_W0, t_b0, t_W1, t_b1, t_W2, t_b2, trace=False):
    import sys
    if "/opt/trn_rl_repo" not in sys.path:
        sys.path.insert(0, "/opt/trn_rl_repo")
    from concourse.bass_utils import run_bass_kernel_spmd

    inputs = dict(addr_from=addr_from, addr_to=addr_to, h_local=h_local,
                  h_global=h_global, x_local=x_local, x_global=x_global, t=t,
                  f_W0=f_W0, f_b0=f_b0, f_W1=f_W1, f_b1=f_b1, f_W2=f_W2,
                  f_b2=f_b2, t_W0=t_W0, t_b0=t_b0, t_W1=t_W1, t_b1=t_b1,
                  t_W2=t_W2, t_b2=t_b2)
    af = np.asarray(addr_from).astype(np.int64)
    at = np.asarray(addr_to).astype(np.int64)
    h = np.asarray(h_local, np.float32)
    x = np.asarray(x_local, np.float32)

    key = (N, E)
    if key not in _BASS_CACHE:
        _BASS_CACHE[key] = _build_bass(NS, NWIN, LASTROWS, KTILES, BT, NB, N,
                                       NCORES)
    nc = _BASS_CACHE[key]

    gidx, locv, xt, spills = _prep_slots(af, at, x, NCORES, NS, NWIN,
                                         KTILES, BT, NB)
    wm = _prep_weights(inputs, BT)
    htab = np.ascontiguousarray(h.astype(np.float16))

    in_maps = []
    for c in range(NCORES):
        m = {"gidx": gidx[c], "locv": locv[c], "xt": xt[c],
             "hshard": htab[c * NS:(c + 1) * NS]}
        m.update(wm)
        in_maps.append(m)

    res = run_bass_kernel_spmd(nc, in_maps, core_ids=list(range(NCORES)),
                               trace=trace)
    out = np.concatenate([res.results[c]["delta"] for c in range(NCORES)],
                         axis=0).astype(np.float32)
    _fix_spill_nodes(spills, inputs, out)
    if trace:
        return out, res
    return out


def kernel(**inputs):
    try:
        return _kernel_bass(**inputs)
    except Exception:
        import traceback
        traceback.print_exc()
        return _kernel_numpy(**inputs)


# revision 19
# speedup vs baseline: 15.7334x; 2.0237x over previous
import numpy as np

# nn_LocalDynamics GNN message passing.
#   delta[n] = sum_e tanh(fMLP(inp_e))[addr_from=n] + tanh(tMLP(inp_e))[addr_to=n]
#   out = tanh(delta).  inp_e = [h[from], h[to], x_e, hg, xg, t] (153 dims).
#
# Destination-sharded design: each core owns nodes [c*12500, (c+1)*12500).
# Every edge yields two "slots": an f-slot on the core owning addr_from and a
# t-slot on the core owning addr_to.  Slots are grouped by 128-node windows of
# the owning core; each window holds a fixed K tiles of 128 slots per
# population (f/t), host-padded.  On device, per 512-slot batch:
#   indirect-DMA gather of (h[from], h[to]) row pairs -> XBAR transpose to
#   feature-major -> fp16 MLP -> slot-major final layer -> one-hot matmul
#   accumulates the window's delta in PSUM -> contiguous fp32 writes.
# Host applies the final tanh (and the overflow spill path, normally empty).

N = 100_000
E = 800_000
D = 64
H = 128
NCORES = 8
NS = N // NCORES            # nodes per core (12500)
NWIN = (NS + 127) // 128    # windows per core (98)
LASTROWS = NS - (NWIN - 1) * 128   # rows in last window (84)
KTILES = 12                 # 128-slot tiles per population per window
BT = 4                      # tiles per batch
SB = BT * 128               # slots per batch (512)
NBW = KTILES // BT          # batches per population-window (3)
NB = NWIN * 2 * NBW         # batches per core (588)
SLOTS = NB * SB             # padded slots per core (301056)


def _scatter_add(delta, idx, vals):
    o = np.argsort(idx, kind="stable")
    si = idx[o]
    sv = vals[o]
    starts = np.flatnonzero(np.r_[True, si[1:] != si[:-1]])
    sums = np.add.reduceat(sv, starts, axis=0)
    np.add.at(delta, si[starts], sums)


def _kernel_numpy(addr_from, addr_to, h_local, h_global, x_local, x_global, t,
                  f_W0, f_b0, f_W1, f_b1, f_W2, f_b2,
                  t_W0, t_b0, t_W1, t_b1, t_W2, t_b2):
    af = np.asarray(addr_from).astype(np.int64)
    at = np.asarray(addr_to).astype(np.int64)
    h_local = np.asarray(h_local, dtype=np.float32)
    x_local = np.asarray(x_local, dtype=np.float32)
    const = np.concatenate([np.asarray(h_global, np.float32).ravel(),
                            np.asarray(x_global, np.float32).ravel(),
                            np.asarray(t, np.float32).ravel()])
    ne = af.shape[0]
    delta = np.zeros((h_local.shape[0], D), np.float32)
    CH = 100_000
    for s in range(0, ne, CH):
        e = min(s + CH, ne)
        inp = np.concatenate([h_local[af[s:e]], h_local[at[s:e]], x_local[s:e],
                              np.broadcast_to(const, (e - s, 21))], axis=1).astype(np.float32)
        d_f = np.tanh(np.tanh(np.tanh(inp @ f_W0 + f_b0) @ f_W1 + f_b1) @ f_W2 + f_b2)
        d_t = np.tanh(np.tanh(np.tanh(inp @ t_W0 + t_b0) @ t_W1 + t_b1) @ t_W2 + t_b2)
        _scatter_add(delta, af[s:e], d_f.astype(np.float32))
        _scatter_add(delta, at[s:e], d_t.astype(np.float32))
    return np.tanh(delta).astype(np.float32)


_BASS_CACHE = {}


def _build_bass(ns, nwin, lastrows, ktiles, bt, nb, ntot, ncores,
                walrus_fix=True):
    import concourse.bass as bass
    import concourse.mybir as mybir
    import concourse.tile as tile

    # walrus in this env rejects Drain instructions carrying >1 sem wait;
    # move each wait onto its own sync nop before the drain.
    def _patched(self, tick_clock, wait_clock):
        from concourse.tile import ScopedClock
        nop0 = self.nc.sync.nop(nofuse=True)
        wait_clock.add_sem_waits(nop0.ins, ScopedClock({None: tick_clock.global_clock}))
        si = nop0.ins.sync_info
        if si is not None and si.on_wait and len(si.on_wait) > 1:
            waits = list(si.on_wait)
            si.on_wait = waits[:1]
            for w in waits[1:]:
                n = self.nc.sync.nop(nofuse=True)
                n.ins.sync_info = mybir.SyncInfo(on_wait=[w], on_update=[])
        self.nc.sync.drain()
        self.nc.all_engine_barrier()
        popped = self.nc._tile_sem_poison_stack.pop()
        assert popped is self._sem_poison
        self.nc.clear_and_free_semaphores(list(self.sems.allocated().values()))
        self.nc.all_engine_barrier()

    tile.TileContext._drain_and_barrier = _patched

    f32 = mybir.dt.float32
    f16 = mybir.dt.float16
    i32 = mybir.dt.int32
    sb = bt * 128
    nbw = ktiles // bt

    nc = bass.Bass(num_devices=ncores)
    gidx_d = nc.dram_tensor("gidx", [nb, 128, 2 * bt], i32, kind="ExternalInput")
    locv_d = nc.dram_tensor("locv", [nb, 128, bt], f16, kind="ExternalInput")
    xt_d = nc.dram_tensor("xt", [nb, 4, sb], f16, kind="ExternalInput")
    hshard_d = nc.dram_tensor("hshard", [ns, D], f16, kind="ExternalInput")
    hsh_b = nc.dram_tensor("hsh_b", [ns, D], f16)
    htab_d = nc.dram_tensor("hfull", [ntot, D], f16, addr_space="Shared")
    iota_d = nc.dram_tensor("iota", [128, 128], f16, kind="ExternalInput")
    wts = {}
    for p in ("f", "t"):
        wts[p + "w0h"] = nc.dram_tensor(p + "w0h", [128, H], f16, kind="ExternalInput")
        wts[p + "w0x"] = nc.dram_tensor(p + "w0x", [4, H], f16, kind="ExternalInput")
        wts[p + "w1"] = nc.dram_tensor(p + "w1", [H, H], f16, kind="ExternalInput")
        wts[p + "w2"] = nc.dram_tensor(p + "w2", [H, D], f16, kind="ExternalInput")
        wts[p + "b0"] = nc.dram_tensor(p + "b0", [H, 1], f32, kind="ExternalInput")
        wts[p + "b1"] = nc.dram_tensor(p + "b1", [H, 1], f32, kind="ExternalInput")
        wts[p + "b2r"] = nc.dram_tensor(p + "b2r", [128, bt * D], f16, kind="ExternalInput")
    delta_d = nc.dram_tensor("delta", [ns, D], f16, kind="ExternalOutput")

    # all-gather the h shards into a full replicated table before the main
    # body; runs on the gpsimd stream, which also issues the gathers later,
    # so engine program order guarantees completion before first use.
    with nc.Block() as blk, \
         nc.semaphore("ag_dma") as ag_dma, \
         nc.semaphore("ag_cc") as ag_cc:

        @blk.gpsimd
        def _(g):
            g.dma_start(out=hsh_b[:, :], in_=hshard_d[:, :]).then_inc(ag_dma, 16)
            g.wait_ge(ag_dma, 16)
            g.collective_compute(
                "AllGather",
                mybir.AluOpType.bypass,
                replica_groups=[list(range(ncores))],
                ins=[hsh_b.ap().opt()],
                outs=[htab_d.ap().opt()],
            ).then_inc(ag_cc)
            g.wait_ge(ag_cc, 1)

    Tanh = mybir.ActivationFunctionType.Tanh
    with tile.TileContext(nc) as tc:
        with tc.tile_pool(name="wpool", bufs=1) as wp, \
             tc.tile_pool(name="io", bufs=3) as io, \
             tc.tile_pool(name="act", bufs=2) as ap_, \
             tc.tile_pool(name="ps01", bufs=1, space="PSUM") as ps01, \
             tc.tile_pool(name="psd", bufs=2, space="PSUM") as psdp, \
             tc.tile_pool(name="win", bufs=2, space="PSUM") as winp:
            wt = {}
            for k, dr in wts.items():
                tl = wp.tile(list(dr.shape), dr.dtype, tag="w" + k)
                nc.sync.dma_start(out=tl[:], in_=dr[:])
                wt[k] = tl
            iota = wp.tile([128, 128], f16, tag="iota")
            nc.sync.dma_start(out=iota[:], in_=iota_d[:])

            for w in range(nwin):
                win = winp.tile([128, D], f32, tag="win")
                for pi, p in enumerate(("f", "t")):
                    for bi in range(nbw):
                        b = (w * 2 + pi) * nbw + bi
                        gi = io.tile([128, 2 * bt], i32, tag="gi")
                        lo = io.tile([128, bt], f16, tag="lo")
                        xb = io.tile([4, sb], f16, tag="xb")
                        nc.sync.dma_start(out=gi[:], in_=gidx_d[b])
                        nc.sync.dma_start(out=lo[:], in_=locv_d[b])
                        nc.sync.dma_start(out=xb[:], in_=xt_d[b])
                        gp = io.tile([128, 2 * bt, D], f16, tag="gp")
                        # HW DGE handles one offset per partition per
                        # instruction; fan out over the 2*bt columns.
                        for j in range(2 * bt):
                            nc.gpsimd.indirect_dma_start(
                                out=gp[:, j, :],
                                out_offset=None,
                                in_=htab_d[:],
                                in_offset=bass.IndirectOffsetOnAxis(
                                    ap=gi[:, j:j + 1], axis=0),
                            )
                        rhs = ap_.tile([128, sb], f16, tag="rhs")
                        for t in range(bt):
                            nc.sync.dma_start_transpose(
                                out=rhs[:, t * 128:(t + 1) * 128],
                                in_=gp[:, 2 * t:2 * t + 2, :])
                        ps0 = ps01.tile([128, sb], f32, tag="ps0")
                        nc.tensor.matmul(out=ps0[:], lhsT=wt[p + "w0h"][:],
                                         rhs=rhs[:], start=True, stop=False)
                        nc.tensor.matmul(out=ps0[:], lhsT=wt[p + "w0x"][:],
                                         rhs=xb[:], start=False, stop=True)
                        h1 = ap_.tile([128, sb], f16, tag="h1")
                        nc.scalar.activation(h1[:], ps0[:], Tanh,
                                             bias=wt[p + "b0"][:, 0:1])
                        ps1 = ps01.tile([128, sb], f32, tag="ps1")
                        nc.tensor.matmul(out=ps1[:], lhsT=wt[p + "w1"][:],
                                         rhs=h1[:], start=True, stop=True)
                        h2 = ap_.tile([128, sb], f16, tag="h2")
                        nc.scalar.activation(h2[:], ps1[:], Tanh,
                                             bias=wt[p + "b1"][:, 0:1])
                        psd = psdp.tile([128, bt * D], f32, tag="psd")
                        for t in range(bt):
                            nc.tensor.matmul(out=psd[:, t * D:(t + 1) * D],
                                             lhsT=h2[:, t * 128:(t + 1) * 128],
                                             rhs=wt[p + "w2"][:],
                                             start=True, stop=True)
                        dsb = ap_.tile([128, bt * D], f16, tag="dsb")
                        nc.vector.tensor_tensor(out=dsb[:], in0=psd[:],
                                                in1=wt[p + "b2r"][:],
                                                op=mybir.AluOpType.add)
                        dtl = ap_.tile([128, bt * D], f16, tag="dtl")
                        nc.scalar.activation(dtl[:], dsb[:], Tanh)
                        oh = ap_.tile([128, bt, 128], f16, tag="oh")
                        for t in range(bt):
                            nc.vector.tensor_tensor(
                                out=oh[:, t, :],
                                in0=lo[:, t:t + 1].to_broadcast([128, 128]),
                                in1=iota[:],
                                op=mybir.AluOpType.is_equal)
                        for t in range(bt):
                            nc.tensor.matmul(
                                out=win[:],
                                lhsT=oh[:, t, :],
                                rhs=dtl[:, t * D:(t + 1) * D],
                                start=(pi == 0 and bi == 0 and t == 0),
                                stop=(pi == 1 and bi == nbw - 1 and t == bt - 1))
                rows = lastrows if w == nwin - 1 else 128
                wout = ap_.tile([128, D], f16, tag="wout")
                nc.scalar.activation(wout[:], win[:], Tanh)
                nc.sync.dma_start(out=delta_d[w * 128:w * 128 + rows, :],
                                  in_=wout[0:rows, :])

    # this walrus rejects any compute instruction carrying >1 sem wait;
    # hoist extra waits onto same-engine nops placed just before it.
    if not walrus_fix:
        return nc
    ctr = 0
    for bb in nc.main_func.blocks:
        new = []
        for ins in bb.instructions:
            si = getattr(ins, "sync_info", None)
            if si is not None and si.on_wait and len(si.on_wait) > 1:
                waits = list(si.on_wait)
                si.on_wait = [waits[-1]]
                for wv in waits[:-1]:
                    ctr += 1
                    nop = mybir.InstNoOp(
                        name=f"wsplit-{ctr}", engine=ins.engine, ins=[], outs=[],
                        sync_info=mybir.SyncInfo(on_wait=[wv], on_update=[]))
                    new.append(nop)
            new.append(ins)
        bb.instructions[:] = new
    return nc


def _prep_slots(af, at, x_local, ncores, ns, nwin, ktiles, bt, nb):
    """Build per-core padded slot arrays. Returns (gidx, locv, xt, spills)
    with gidx [ncores, NB, 128, 2bt] i32, locv [...bt] f16, xt [..., 4, SB] f16,
    spills = list of (pop, edge_indices) that overflowed window capacity."""
    sb = bt * 128
    nbw = ktiles // bt
    cap = ktiles * 128
    gidx = np.zeros((ncores, nb, 128, 2 * bt), np.int32)
    locv = np.full((ncores, nb, 128, bt), 128.0, np.float16)
    xt = np.zeros((ncores, nb, 4, sb), np.float16)
    xtv = np.ascontiguousarray(x_local.astype(np.float16))
    spills = []
    for pi, dest in enumerate((af, at)):
        core = dest // ns
        node_l = dest - core * ns
        w = node_l >> 7
        loc = node_l & 127
        cw = core * nwin + w
        order = np.argsort(cw, kind="stable")
        cws = cw[order]
        counts = np.bincount(cws, minlength=ncores * nwin)
        starts = np.concatenate([[0], np.cumsum(counts)[:-1]])
        rank = np.arange(len(cws)) - np.repeat(starts, counts)
        ok = rank < cap
        if not ok.all():
            spills.append((pi, order[~ok]))
        e_ok = order[ok]
        r = rank[ok]
        c_ok = core[e_ok]
        w_ok = w[e_ok]
        # slot within core: window block of 2*cap, population block of cap
        s = w_ok * (2 * cap) + pi * cap + r
        b = s // sb
        t = (s % sb) // 128
        pp = s % 128
        gidx[c_ok, b, pp, 2 * t] = af[e_ok]
        gidx[c_ok, b, pp, 2 * t + 1] = at[e_ok]
        locv[c_ok, b, pp, t] = loc[e_ok].astype(np.float16)
        xt[c_ok, b, :, t * 128 + pp] = xtv[e_ok]
    return gidx, locv, xt, spills


def _prep_weights(inputs, bt):
    const = np.concatenate([np.asarray(inputs["h_global"], np.float32).ravel(),
                            np.asarray(inputs["x_global"], np.float32).ravel(),
                            np.asarray(inputs["t"], np.float32).ravel()])
    wm = {}
    for p in ("f", "t"):
        W0 = np.asarray(inputs[p + "_W0"], np.float32)
        b0 = np.asarray(inputs[p + "_b0"], np.float32)
        b0eff = b0 + const @ W0[132:153]
        wm[p + "w0h"] = np.ascontiguousarray(W0[0:128]).astype(np.float16)
        wm[p + "w0x"] = np.ascontiguousarray(W0[128:132]).astype(np.float16)
        wm[p + "w1"] = np.asarray(inputs[p + "_W1"], np.float32).astype(np.float16)
        wm[p + "w2"] = np.asarray(inputs[p + "_W2"], np.float32).astype(np.float16)
        wm[p + "b0"] = b0eff.reshape(H, 1).astype(np.float32)
        wm[p + "b1"] = np.asarray(inputs[p + "_b1"], np.float32).reshape(H, 1)
        b2 = np.asarray(inputs[p + "_b2"], np.float32)
        wm[p + "b2r"] = np.tile(b2.reshape(1, D), (128, bt)).astype(np.float16)
    wm["iota"] = np.broadcast_to(np.arange(128, dtype=np.float16), (128, 128)).copy()
    return wm


def _fix_spill_nodes(spills, inputs, out):
    """Recompute on host (fp32) every node whose window overflowed device
    capacity; overwrite those rows of `out`. Empty for uniform edge data."""
    if not spills:
        return
    af = np.asarray(inputs["addr_from"]).astype(np.int64)
    at = np.asarray(inputs["addr_to"]).astype(np.int64)
    h = np.asarray(inputs["h_local"], np.float32)
    x = np.asarray(inputs["x_local"], np.float32)
    const = np.concatenate([np.asarray(inputs["h_global"], np.float32).ravel(),
                            np.asarray(inputs["x_global"], np.float32).ravel(),
                            np.asarray(inputs["t"], np.float32).ravel()])
    nodes = np.unique(np.concatenate(
        [(af if pi == 0 else at)[e] for pi, e in spills]))
    node_set = np.zeros(h.shape[0], bool)
    node_set[nodes] = True
    delta = np.zeros((len(nodes), D), np.float32)
    remap = np.full(h.shape[0], -1, np.int64)
    remap[nodes] = np.arange(len(nodes))
    for pi, idx_all in ((0, af), (1, at)):
        p = "f" if pi == 0 else "t"
        edges = np.flatnonzero(node_set[idx_all])
        if not len(edges):
            continue
        inp = np.concatenate([h[af[edges]], h[at[edges]], x[edges],
                              np.broadcast_to(const, (len(edges), 21))], axis=1)
        d = np.tanh(np.tanh(np.tanh(
            inp @ inputs[p + "_W0"] + inputs[p + "_b0"]) @ inputs[p + "_W1"]
            + inputs[p + "_b1"]) @ inputs[p + "_W2"] + inputs[p + "_b2"])
        _scatter_add(delta, remap[idx_all[edges]], d.astype(np.float32))
    out[nodes] = np.tanh(delta)


def _get_exec(nc):
    """Build (once) a cached jitted executor for the bass module: the same
    _bass_exec_p/shard_map lowering run_bass_kernel_spmd uses under axon,
    but with the jitted callable memoized so repeat calls skip retracing."""
    if "exec" in _BASS_CACHE:
        return _BASS_CACHE["exec"]
    import jax
    import numpy as jnp_np
    import concourse.mybir as mybir
    from jax.sharding import Mesh, PartitionSpec
    from jax.experimental.shard_map import shard_map
    from concourse.bass2jax import (_bass_exec_p, install_neuronx_cc_hook,
                                    partition_id_tensor)
    install_neuronx_cc_hook()

    in_names, out_names, out_avals = [], [], []
    pname = nc.partition_id_tensor.name if nc.partition_id_tensor else None
    for alloc in nc.m.functions[0].allocations:
        if not isinstance(alloc, mybir.MemoryLocationSet):
            continue
        name = alloc.memorylocations[0].name
        if alloc.kind == "ExternalInput":
            if name != pname:
                in_names.append(name)
        elif alloc.kind == "ExternalOutput":
            out_names.append(name)
            out_avals.append(jax.core.ShapedArray(
                tuple(alloc.tensor_shape), mybir.dt.np(alloc.dtype)))
    n_params = len(in_names)
    n_outs = len(out_avals)
    all_names = in_names + out_names + ([pname] if pname else [])

    def _body(*args):
        ops = list(args)
        if pname:
            ops.append(partition_id_tensor())
        outs = _bass_exec_p.bind(
            *ops, out_avals=tuple(out_avals), in_names=tuple(all_names),
            out_names=tuple(out_names), lowering_input_output_aliases=(),
            sim_require_finite=True, sim_require_nnan=True, nc=nc)
        return tuple(outs)

    devices = jax.devices()[:NCORES]
    mesh = Mesh(np.asarray(devices), ("core",))
    in_specs = (PartitionSpec("core"),) * (n_params + n_outs)
    out_specs = (PartitionSpec("core"),) * n_outs
    donate = tuple(range(n_params, n_params + n_outs))
    sharded = jax.jit(
        shard_map(_body, mesh=mesh, in_specs=in_specs, out_specs=out_specs,
                  check_rep=False),
        donate_argnums=donate, keep_unused=True)
    ex = (sharded, in_names, out_names, out_avals)
    _BASS_CACHE["exec"] = ex
    return ex


def _kernel_bass(addr_from, addr_to, h_local, h_global, x_local, x_global, t,
                 f_W0, f_b0, f_W1, f_b1, f_W2, f_b2,
                 t_W0, t_b0, t_W1, t_b1, t_W2, t_b2, trace=False):
    import sys
    if "/opt/trn_rl_repo" not in sys.path:
        sys.path.insert(0, "/opt/trn_rl_repo")

    inputs = dict(addr_from=addr_from, addr_to=addr_to, h_local=h_local,
                  h_global=h_global, x_local=x_local, x_global=x_global, t=t,
                  f_W0=f_W0, f_b0=f_b0, f_W1=f_W1, f_b1=f_b1, f_W2=f_W2,
                  f_b2=f_b2, t_W0=t_W0, t_b0=t_b0, t_W1=t_W1, t_b1=t_b1,
                  t_W2=t_W2, t_b2=t_b2)
    af = np.asarray(addr_from).astype(np.int64)
    at = np.asarray(addr_to).astype(np.int64)
    h = np.asarray(h_local, np.float32)
    x = np.asarray(x_local, np.float32)

    key = (N, E)
    if key not in _BASS_CACHE:
        _BASS_CACHE[key] = _build_bass(NS, NWIN, LASTROWS, KTILES, BT, NB, N,
                                       NCORES)
    nc = _BASS_CACHE[key]

    gidx, locv, xt, spills = _prep_slots(af, at, x, NCORES, NS, NWIN,
                                         KTILES, BT, NB)
    wm = _prep_weights(inputs, BT)
    htab = np.ascontiguousarray(h.astype(np.float16))

    in_maps = []
    for c in range(NCORES):
        m = {"gidx": gidx[c], "locv": locv[c], "xt": xt[c],
             "hshard": htab[c * NS:(c + 1) * NS]}
        m.update(wm)
        in_maps.append(m)

    sharded, in_names, out_names, out_avals = _get_exec(nc)
    concat_in = [np.concatenate([np.asarray(in_maps[c][n])
                                 for c in range(NCORES)], axis=0)
                 for n in in_names]
    concat_zeros = [np.zeros((NCORES * a.shape[0], *a.shape[1:]), a.dtype)
                    for a in out_avals]
    out_arrs = sharded(*concat_in, *concat_zeros)
    out = np.asarray(out_arrs[out_names.index("delta")]).astype(np.float32)
    _fix_spill_nodes(spills, inputs, out)
    return out


def kernel(**inputs):
    try:
        return _kernel_bass(**inputs)
    except Exception:
        import traceback
        traceback.print_exc()
        return _kernel_numpy(**inputs)


# revision 21
# speedup vs baseline: 18.4695x; 1.1739x over previous
import numpy as np

# nn_LocalDynamics GNN message passing.
#   delta[n] = sum_e tanh(fMLP(inp_e))[addr_from=n] + tanh(tMLP(inp_e))[addr_to=n]
#   out = tanh(delta).  inp_e = [h[from], h[to], x_e, hg, xg, t] (153 dims).
#
# Destination-sharded design: each core owns nodes [c*12500, (c+1)*12500).
# Every edge yields two "slots": an f-slot on the core owning addr_from and a
# t-slot on the core owning addr_to.  Slots are grouped by 128-node windows of
# the owning core; each window holds a fixed KTILES tiles of 128 slots per
# population (f/t), host-padded.  On device, per batch of tiles:
#   AllGather h shards -> indirect-DMA gather of (h[from], h[to]) row pairs ->
#   XBAR transpose to feature-major -> fp16 MLP -> slot-major final layer ->
#   one-hot matmul accumulates the window's delta in PSUM -> tanh -> fp16 out.
# Host recomputes any overflowed windows (empty for uniform edges).

N = 100_000
E = 800_000
D = 64
H = 128
NCORES = 8
NS = N // NCORES            # nodes per core (12500)
NWIN = (NS + 127) // 128    # windows per core (98)
LASTROWS = NS - (NWIN - 1) * 128   # rows in last window (84)
KTILES = 10                 # 128-slot tiles per population per window
NT = NWIN * 2 * KTILES      # tiles per core (1960)
SLOTS = NT * 128            # padded slots per core (250880)
MAXBT = 4                   # max tiles per batch


def _batch_tiles(ktiles):
    out = []
    k = ktiles
    while k > 0:
        out.append(min(MAXBT, k))
        k -= out[-1]
    return out


def _scatter_add(delta, idx, vals):
    o = np.argsort(idx, kind="stable")
    si = idx[o]
    sv = vals[o]
    starts = np.flatnonzero(np.r_[True, si[1:] != si[:-1]])
    sums = np.add.reduceat(sv, starts, axis=0)
    np.add.at(delta, si[starts], sums)


def _kernel_numpy(addr_from, addr_to, h_local, h_global, x_local, x_global, t,
                  f_W0, f_b0, f_W1, f_b1, f_W2, f_b2,
                  t_W0, t_b0, t_W1, t_b1, t_W2, t_b2):
    af = np.asarray(addr_from).astype(np.int64)
    at = np.asarray(addr_to).astype(np.int64)
    h_local = np.asarray(h_local, dtype=np.float32)
    x_local = np.asarray(x_local, dtype=np.float32)
    const = np.concatenate([np.asarray(h_global, np.float32).ravel(),
                            np.asarray(x_global, np.float32).ravel(),
                            np.asarray(t, np.float32).ravel()])
    ne = af.shape[0]
    delta = np.zeros((h_local.shape[0], D), np.float32)
    CH = 100_000
    for s in range(0, ne, CH):
        e = min(s + CH, ne)
        inp = np.concatenate([h_local[af[s:e]], h_local[at[s:e]], x_local[s:e],
                              np.broadcast_to(const, (e - s, 21))], axis=1).astype(np.float32)
        d_f = np.tanh(np.tanh(np.tanh(inp @ f_W0 + f_b0) @ f_W1 + f_b1) @ f_W2 + f_b2)
        d_t = np.tanh(np.tanh(np.tanh(inp @ t_W0 + t_b0) @ t_W1 + t_b1) @ t_W2 + t_b2)
        _scatter_add(delta, af[s:e], d_f.astype(np.float32))
        _scatter_add(delta, at[s:e], d_t.astype(np.float32))
    return np.tanh(delta).astype(np.float32)


_BASS_CACHE = {}


def _build_bass(ns, nwin, lastrows, ktiles, ntot, ncores, walrus_fix=True):
    import concourse.bass as bass
    import concourse.mybir as mybir
    import concourse.tile as tile

    # walrus in this env rejects Drain instructions carrying >1 sem wait;
    # move each wait onto its own sync nop before the drain.
    def _patched(self, tick_clock, wait_clock):
        from concourse.tile import ScopedClock
        nop0 = self.nc.sync.nop(nofuse=True)
        wait_clock.add_sem_waits(nop0.ins, ScopedClock({None: tick_clock.global_clock}))
        si = nop0.ins.sync_info
        if si is not None and si.on_wait and len(si.on_wait) > 1:
            waits = list(si.on_wait)
            si.on_wait = waits[:1]
            for w in waits[1:]:
                n = self.nc.sync.nop(nofuse=True)
                n.ins.sync_info = mybir.SyncInfo(on_wait=[w], on_update=[])
        self.nc.sync.drain()
        self.nc.all_engine_barrier()
        popped = self.nc._tile_sem_poison_stack.pop()
        assert popped is self._sem_poison
        self.nc.clear_and_free_semaphores(list(self.sems.allocated().values()))
        self.nc.all_engine_barrier()

    tile.TileContext._drain_and_barrier = _patched

    f32 = mybir.dt.float32
    f16 = mybir.dt.float16
    i32 = mybir.dt.int32
    i8 = mybir.dt.int8
    nt_tot = nwin * 2 * ktiles
    slots = nt_tot * 128
    bts = _batch_tiles(ktiles)

    nc = bass.Bass(num_devices=ncores)
    gidx_d = nc.dram_tensor("gidx", [128, 2 * nt_tot], i32, kind="ExternalInput")
    locv_d = nc.dram_tensor("locv", [128, nt_tot], i8, kind="ExternalInput")
    xt_d = nc.dram_tensor("xt", [4, slots], f16, kind="ExternalInput")
    hshard_d = nc.dram_tensor("hshard", [ns, D], f16, kind="ExternalInput")
    hsh_b = nc.dram_tensor("hsh_b", [ns, D], f16)
    htab_d = nc.dram_tensor("hfull", [ntot, D], f16, addr_space="Shared")
    iota_d = nc.dram_tensor("iota", [128, 128], f16, kind="ExternalInput")
    wts = {}
    for p in ("f", "t"):
        wts[p + "w0h"] = nc.dram_tensor(p + "w0h", [128, H], f16, kind="ExternalInput")
        wts[p + "w0x"] = nc.dram_tensor(p + "w0x", [4, H], f16, kind="ExternalInput")
        wts[p + "w1"] = nc.dram_tensor(p + "w1", [H, H], f16, kind="ExternalInput")
        wts[p + "w2"] = nc.dram_tensor(p + "w2", [H, D], f16, kind="ExternalInput")
        wts[p + "b0"] = nc.dram_tensor(p + "b0", [H, 1], f32, kind="ExternalInput")
        wts[p + "b1"] = nc.dram_tensor(p + "b1", [H, 1], f32, kind="ExternalInput")
        wts[p + "b2r"] = nc.dram_tensor(p + "b2r", [128, MAXBT * D], f16, kind="ExternalInput")
    delta_d = nc.dram_tensor("delta", [ns, D], f16, kind="ExternalOutput")

    # all-gather the h shards into a full replicated table before the main
    # body; runs on the gpsimd stream, which also issues the gathers later,
    # so engine program order guarantees completion before first use.
    with nc.Block() as blk, \
         nc.semaphore("ag_dma") as ag_dma, \
         nc.semaphore("ag_cc") as ag_cc:

        @blk.gpsimd
        def _(g):
            g.dma_start(out=hsh_b[:, :], in_=hshard_d[:, :]).then_inc(ag_dma, 16)
            g.wait_ge(ag_dma, 16)
            g.collective_compute(
                "AllGather",
                mybir.AluOpType.bypass,
                replica_groups=[list(range(ncores))],
                ins=[hsh_b.ap().opt()],
                outs=[htab_d.ap().opt()],
            ).then_inc(ag_cc)
            g.wait_ge(ag_cc, 1)

    Tanh = mybir.ActivationFunctionType.Tanh
    MB = MAXBT
    with tile.TileContext(nc) as tc:
        with tc.tile_pool(name="wpool", bufs=1) as wp, \
             tc.tile_pool(name="io", bufs=3) as io, \
             tc.tile_pool(name="act", bufs=2) as ap_, \
             tc.tile_pool(name="ps01", bufs=1, space="PSUM") as ps01, \
             tc.tile_pool(name="psd", bufs=2, space="PSUM") as psdp, \
             tc.tile_pool(name="win", bufs=2, space="PSUM") as winp:
            wt = {}
            for k, dr in wts.items():
                tl = wp.tile(list(dr.shape), dr.dtype, tag="w" + k)
                nc.sync.dma_start(out=tl[:], in_=dr[:])
                wt[k] = tl
            iota = wp.tile([128, 128], f16, tag="iota")
            nc.sync.dma_start(out=iota[:], in_=iota_d[:])

            for w in range(nwin):
                win = winp.tile([128, D], f32, tag="win")
                for pi, p in enumerate(("f", "t")):
                    tbase = (w * 2 + pi) * ktiles
                    off = 0
                    for bi, bt in enumerate(bts):
                        t0 = tbase + off
                        s0 = t0 * 128
                        nsl = bt * 128
                        off += bt
                        gi = io.tile([128, 2 * MB], i32, tag="gi")
                        lo8 = io.tile([128, MB], i8, tag="lo8")
                        xb = io.tile([4, MB * 128], f16, tag="xb")
                        nc.sync.dma_start(out=gi[:, :2 * bt],
                                          in_=gidx_d[:, 2 * t0:2 * (t0 + bt)])
                        nc.sync.dma_start(out=lo8[:, :bt],
                                          in_=locv_d[:, t0:t0 + bt])
                        nc.sync.dma_start(out=xb[:, :nsl],
                                          in_=xt_d[:, s0:s0 + nsl])
                        lo = io.tile([128, MB], f16, tag="lo")
                        nc.vector.tensor_copy(out=lo[:, :bt], in_=lo8[:, :bt])
                        gp = io.tile([128, 2 * MB, D], f16, tag="gp")
                        # HW DGE handles one offset per partition per
                        # instruction; fan out over the 2*bt columns.
                        for j in range(2 * bt):
                            nc.gpsimd.indirect_dma_start(
                                out=gp[:, j, :],
                                out_offset=None,
                                in_=htab_d[:],
                                in_offset=bass.IndirectOffsetOnAxis(
                                    ap=gi[:, j:j + 1], axis=0),
                            )
                        rhs = ap_.tile([128, MB * 128], f16, tag="rhs")
                        for t in range(bt):
                            nc.sync.dma_start_transpose(
                                out=rhs[:, t * 128:(t + 1) * 128],
                                in_=gp[:, 2 * t:2 * t + 2, :])
                        ps0 = ps01.tile([128, MB * 128], f32, tag="ps0")
                        nc.tensor.matmul(out=ps0[:, :nsl], lhsT=wt[p + "w0h"][:],
                                         rhs=rhs[:, :nsl], start=True, stop=False)
                        nc.tensor.matmul(out=ps0[:, :nsl], lhsT=wt[p + "w0x"][:],
                                         rhs=xb[:, :nsl], start=False, stop=True)
                        h1 = ap_.tile([128, MB * 128], f16, tag="h1")
                        nc.scalar.activation(h1[:, :nsl], ps0[:, :nsl], Tanh,
                                             bias=wt[p + "b0"][:, 0:1])
                        ps1 = ps01.tile([128, MB * 128], f32, tag="ps1")
                        nc.tensor.matmul(out=ps1[:, :nsl], lhsT=wt[p + "w1"][:],
                                         rhs=h1[:, :nsl], start=True, stop=True)
                        h2 = ap_.tile([128, MB * 128], f16, tag="h2")
                        nc.scalar.activation(h2[:, :nsl], ps1[:, :nsl], Tanh,
                                             bias=wt[p + "b1"][:, 0:1])
                        psd = psdp.tile([128, MB * D], f32, tag="psd")
                        for t in range(bt):
                            nc.tensor.matmul(out=psd[:, t * D:(t + 1) * D],
                                             lhsT=h2[:, t * 128:(t + 1) * 128],
                                             rhs=wt[p + "w2"][:],
                                             start=True, stop=True)
                        dsb = ap_.tile([128, MB * D], f16, tag="dsb")
                        nc.vector.tensor_tensor(out=dsb[:, :bt * D],
                                                in0=psd[:, :bt * D],
                                                in1=wt[p + "b2r"][:, :bt * D],
                                                op=mybir.AluOpType.add)
                        dtl = ap_.tile([128, MB * D], f16, tag="dtl")
                        nc.scalar.activation(dtl[:, :bt * D], dsb[:, :bt * D],
                                             Tanh)
                        oh = ap_.tile([128, MB, 128], f16, tag="oh")
                        for t in range(bt):
                            nc.vector.tensor_tensor(
                                out=oh[:, t, :],
                                in0=lo[:, t:t + 1].to_broadcast([128, 128]),
                                in1=iota[:],
                                op=mybir.AluOpType.is_equal)
                        for t in range(bt):
                            nc.tensor.matmul(
                                out=win[:],
                                lhsT=oh[:, t, :],
                                rhs=dtl[:, t * D:(t + 1) * D],
                                start=(pi == 0 and bi == 0 and t == 0),
                                stop=(pi == 1 and bi == len(bts) - 1
                                      and t == bt - 1))
                rows = lastrows if w == nwin - 1 else 128
                wout = ap_.tile([128, D], f16, tag="wout")
                nc.scalar.activation(wout[:], win[:], Tanh)
                nc.sync.dma_start(out=delta_d[w * 128:w * 128 + rows, :],
                                  in_=wout[0:rows, :])

    # this walrus rejects any compute instruction carrying >1 sem wait;
    # hoist extra waits onto same-engine nops placed just before it.
    if not walrus_fix:
        return nc
    ctr = 0
    for bb in nc.main_func.blocks:
        new = []
        for ins in bb.instructions:
            si = getattr(ins, "sync_info", None)
            if si is not None and si.on_wait and len(si.on_wait) > 1:
                waits = list(si.on_wait)
                si.on_wait = [waits[-1]]
                for wv in waits[:-1]:
                    ctr += 1
                    nop = mybir.InstNoOp(
                        name=f"wsplit-{ctr}", engine=ins.engine, ins=[], outs=[],
                        sync_info=mybir.SyncInfo(on_wait=[wv], on_update=[]))
                    new.append(nop)
            new.append(ins)
        bb.instructions[:] = new
    return nc


def _prep_slots(af, at, x_local, ncores, ns, nwin, ktiles):
    """Build per-core padded slot arrays in tile-major layout.
    gidx [ncores, 128, 2*NT] i32 (from/to pairs per tile column),
    locv [ncores, 128, NT] i8 (in-window node offset, -1 = pad),
    xt   [ncores, 4, SLOTS] f16 (x features, slot-major),
    spills = list of (pop, edge_indices) that overflowed window capacity."""
    cap = ktiles * 128
    nt_tot = nwin * 2 * ktiles
    slots = nt_tot * 128
    af32 = af.astype(np.int32)
    at32 = at.astype(np.int32)
    gidx = np.zeros((ncores, 128, 2 * nt_tot), np.int32)
    locv = np.full((ncores, 128, nt_tot), -1, np.int8)
    xt = np.zeros((ncores, 4, slots), np.float16)
    xtv = np.ascontiguousarray(x_local.astype(np.float16))
    gflat = gidx.reshape(-1)
    lflat = locv.reshape(-1)
    xflat = xt.reshape(-1)
    spills = []
    for pi, dest in enumerate((af32, at32)):
        core = dest // np.int32(ns)
        node_l = dest - core * np.int32(ns)
        w = node_l >> 7
        loc = (node_l & 127).astype(np.int8)
        cw = core * np.int32(nwin) + w
        order = np.argsort(cw, kind="stable").astype(np.int32)
        counts = np.bincount(cw, minlength=ncores * nwin)
        starts = np.concatenate([[0], np.cumsum(counts)[:-1]])
        rank = (np.arange(len(cw), dtype=np.int32)
                - np.repeat(starts, counts).astype(np.int32))
        ok = rank < cap
        if not ok.all():
            spills.append((pi, order[~ok].astype(np.int64)))
            e_ok = order[ok]
            r = rank[ok]
        else:
            e_ok = order
            r = rank
        # within-core slot: window block of 2*cap, population block of cap
        sc = w[e_ok] * np.int32(2 * cap) + np.int32(pi * cap) + r
        T = sc >> 7
        pp = sc & 127
        c_ok = core[e_ok]
        gbase = ((c_ok * 128 + pp) * (2 * nt_tot)) + 2 * T
        gflat[gbase] = af32[e_ok]
        gflat[gbase + 1] = at32[e_ok]
        lflat[(c_ok * 128 + pp) * nt_tot + T] = loc[e_ok]
        xbase = (c_ok * 4) * slots + sc
        xflat[xbase[:, None] + (np.arange(4, dtype=np.int32) * slots)[None, :]] \
            = xtv[e_ok]
    return gidx, locv, xt, spills


def _prep_weights(inputs):
    const = np.concatenate([np.asarray(inputs["h_global"], np.float32).ravel(),
                            np.asarray(inputs["x_global"], np.float32).ravel(),
                            np.asarray(inputs["t"], np.float32).ravel()])
    wm = {}
    for p in ("f", "t"):
        W0 = np.asarray(inputs[p + "_W0"], np.float32)
        b0 = np.asarray(inputs[p + "_b0"], np.float32)
        b0eff = b0 + const @ W0[132:153]
        wm[p + "w0h"] = np.ascontiguousarray(W0[0:128]).astype(np.float16)
        wm[p + "w0x"] = np.ascontiguousarray(W0[128:132]).astype(np.float16)
        wm[p + "w1"] = np.asarray(inputs[p + "_W1"], np.float32).astype(np.float16)
        wm[p + "w2"] = np.asarray(inputs[p + "_W2"], np.float32).astype(np.float16)
        wm[p + "b0"] = b0eff.reshape(H, 1).astype(np.float32)
        wm[p + "b1"] = np.asarray(inputs[p + "_b1"], np.float32).reshape(H, 1)
        b2 = np.asarray(inputs[p + "_b2"], np.float32)
        wm[p + "b2r"] = np.tile(b2.reshape(1, D), (128, MAXBT)).astype(np.float16)
    wm["iota"] = np.broadcast_to(np.arange(128, dtype=np.float16), (128, 128)).copy()
    return wm


def _fix_spill_nodes(spills, inputs, out):
    """Recompute on host (fp32) every node whose window overflowed device
    capacity; overwrite those rows of `out`. Empty for uniform edge data."""
    if not spills:
        return
    af = np.asarray(inputs["addr_from"]).astype(np.int64)
    at = np.asarray(inputs["addr_to"]).astype(np.int64)
    h = np.asarray(inputs["h_local"], np.float32)
    x = np.asarray(inputs["x_local"], np.float32)
    const = np.concatenate([np.asarray(inputs["h_global"], np.float32).ravel(),
                            np.asarray(inputs["x_global"], np.float32).ravel(),
                            np.asarray(inputs["t"], np.float32).ravel()])
    nodes = np.unique(np.concatenate(
        [(af if pi == 0 else at)[e] for pi, e in spills]))
    node_set = np.zeros(h.shape[0], bool)
    node_set[nodes] = True
    delta = np.zeros((len(nodes), D), np.float32)
    remap = np.full(h.shape[0], -1, np.int64)
    remap[nodes] = np.arange(len(nodes))
    for pi, idx_all in ((0, af), (1, at)):
        p = "f" if pi == 0 else "t"
        edges = np.flatnonzero(node_set[idx_all])
        if not len(edges):
            continue
        inp = np.concatenate([h[af[edges]], h[at[edges]], x[edges],
                              np.broadcast_to(const, (len(edges), 21))], axis=1)
        d = np.tanh(np.tanh(np.tanh(
            inp @ inputs[p + "_W0"] + inputs[p + "_b0"]) @ inputs[p + "_W1"]
            + inputs[p + "_b1"]) @ inputs[p + "_W2"] + inputs[p + "_b2"])
        _scatter_add(delta, remap[idx_all[edges]], d.astype(np.float32))
    out[nodes] = np.tanh(delta)


def _get_exec(nc):
    """Build (once) a cached jitted executor for the bass module: the same
    _bass_exec_p/shard_map lowering run_bass_kernel_spmd uses under axon,
    but with the jitted callable memoized so repeat calls skip retracing."""
    if "exec" in _BASS_CACHE:
        return _BASS_CACHE["exec"]
    import jax
    import concourse.mybir as mybir
    from jax.sharding import Mesh, PartitionSpec
    from jax.experimental.shard_map import shard_map
    from concourse.bass2jax import (_bass_exec_p, install_neuronx_cc_hook,
                                    partition_id_tensor)
    install_neuronx_cc_hook()

    in_names, out_names, out_avals = [], [], []
    pname = nc.partition_id_tensor.name if nc.partition_id_tensor else None
    for alloc in nc.m.functions[0].allocations:
        if not isinstance(alloc, mybir.MemoryLocationSet):
            continue
        name = alloc.memorylocations[0].name
        if alloc.kind == "ExternalInput":
            if name != pname:
                in_names.append(name)
        elif alloc.kind == "ExternalOutput":
            out_names.append(name)
            out_avals.append(jax.core.ShapedArray(
                tuple(alloc.tensor_shape), mybir.dt.np(alloc.dtype)))
    n_params = len(in_names)
    n_outs = len(out_avals)
    all_names = in_names + out_names + ([pname] if pname else [])

    def _body(*args):
        ops = list(args)
        if pname:
            ops.append(partition_id_tensor())
        outs = _bass_exec_p.bind(
            *ops, out_avals=tuple(out_avals), in_names=tuple(all_names),
            out_names=tuple(out_names), lowering_input_output_aliases=(),
            sim_require_finite=True, sim_require_nnan=True, nc=nc)
        return tuple(outs)

    devices = jax.devices()[:NCORES]
    mesh = Mesh(np.asarray(devices), ("core",))
    in_specs = (PartitionSpec("core"),) * (n_params + n_outs)
    out_specs = (PartitionSpec("core"),) * n_outs
    donate = tuple(range(n_params, n_params + n_outs))
    sharded = jax.jit(
        shard_map(_body, mesh=mesh, in_specs=in_specs, out_specs=out_specs,
                  check_rep=False),
        donate_argnums=donate, keep_unused=True)
    ex = (sharded, in_names, out_names, out_avals)
    _BASS_CACHE["exec"] = ex
    return ex


def _kernel_bass(addr_from, addr_to, h_local, h_global, x_local, x_global, t,
                 f_W0, f_b0, f_W1, f_b1, f_W2, f_b2,
                 t_W0, t_b0, t_W1, t_b1, t_W2, t_b2, trace=False):
    import sys
    if "/opt/trn_rl_repo" not in sys.path:
        sys.path.insert(0, "/opt/trn_rl_repo")

    inputs = dict(addr_from=addr_from, addr_to=addr_to, h_local=h_local,
                  h_global=h_global, x_local=x_local, x_global=x_global, t=t,
                  f_W0=f_W0, f_b0=f_b0, f_W1=f_W1, f_b1=f_b1, f_W2=f_W2,
                  f_b2=f_b2, t_W0=t_W0, t_b0=t_b0, t_W1=t_W1, t_b1=t_b1,
                  t_W2=t_W2, t_b2=t_b2)
    af = np.asarray(addr_from).astype(np.int64)
    at = np.asarray(addr_to).astype(np.int64)
    h = np.asarray(h_local, np.float32)
    x = np.asarray(x_local, np.float32)

    key = (N, E)
    if key not in _BASS_CACHE:
        _BASS_CACHE[key] = _build_bass(NS, NWIN, LASTROWS, KTILES, N, NCORES)
    nc = _BASS_CACHE[key]

    gidx, locv, xt, spills = _prep_slots(af, at, x, NCORES, NS, NWIN, KTILES)
    wm = _prep_weights(inputs)
    htab = np.ascontiguousarray(h.astype(np.float16))

    in_maps = []
    for c in range(NCORES):
        m = {"gidx": gidx[c], "locv": locv[c], "xt": xt[c],
             "hshard": htab[c * NS:(c + 1) * NS]}
        m.update(wm)
        in_maps.append(m)

    sharded, in_names, out_names, out_avals = _get_exec(nc)
    concat_in = [np.concatenate([np.asarray(in_maps[c][n])
                                 for c in range(NCORES)], axis=0)
                 for n in in_names]
    concat_zeros = [np.zeros((NCORES * a.shape[0], *a.shape[1:]), a.dtype)
                    for a in out_avals]
    out_arrs = sharded(*concat_in, *concat_zeros)
    out = np.asarray(out_arrs[out_names.index("delta")]).astype(np.float32)
    _fix_spill_nodes(spills, inputs, out)
    return out


def kernel(**inputs):
    try:
        return _kernel_bass(**inputs)
    except Exception:
        import traceback
        traceback.print_exc()
        return _kernel_numpy(**inputs)


# revision 24
# speedup vs baseline: 22.4296x; 1.2144x over previous
import numpy as np

# nn_LocalDynamics GNN message passing.
#   delta[n] = sum_e tanh(fMLP(inp_e))[addr_from=n] + tanh(tMLP(inp_e))[addr_to=n]
#   out = tanh(delta).  inp_e = [h[from], h[to], x_e, hg, xg, t] (153 dims).
#
# Destination-sharded design: each core owns nodes [c*12500, (c+1)*12500).
# Every edge yields two "slots": an f-slot on the core owning addr_from and a
# t-slot on the core owning addr_to.  Slots are grouped by 128-node windows of
# the owning core; each window holds a fixed KTILES tiles of 128 slots per
# population (f/t), host-padded.  On device, per batch of tiles:
#   AllGather h shards -> indirect-DMA gather of (h[from], h[to]) row pairs ->
#   XBAR transpose to feature-major -> fp16 MLP -> slot-major final layer ->
#   one-hot matmul accumulates the window's delta in PSUM -> tanh -> fp16 out.
# Host recomputes any overflowed windows (empty for uniform edges).

N = 100_000
E = 800_000
D = 64
H = 128
NCORES = 8
NS = N // NCORES            # nodes per core (12500)
NWIN = (NS + 127) // 128    # windows per core (98)
LASTROWS = NS - (NWIN - 1) * 128   # rows in last window (84)
KTILES = 10                 # 128-slot tiles per population per window
NT = NWIN * 2 * KTILES      # tiles per core (1960)
SLOTS = NT * 128            # padded slots per core (250880)
MAXBT = 4                   # max tiles per batch


def _batch_tiles(ktiles):
    out = []
    k = ktiles
    while k > 0:
        out.append(min(MAXBT, k))
        k -= out[-1]
    return out


def _scatter_add(delta, idx, vals):
    o = np.argsort(idx, kind="stable")
    si = idx[o]
    sv = vals[o]
    starts = np.flatnonzero(np.r_[True, si[1:] != si[:-1]])
    sums = np.add.reduceat(sv, starts, axis=0)
    np.add.at(delta, si[starts], sums)


def _kernel_numpy(addr_from, addr_to, h_local, h_global, x_local, x_global, t,
                  f_W0, f_b0, f_W1, f_b1, f_W2, f_b2,
                  t_W0, t_b0, t_W1, t_b1, t_W2, t_b2):
    af = np.asarray(addr_from).astype(np.int64)
    at = np.asarray(addr_to).astype(np.int64)
    h_local = np.asarray(h_local, dtype=np.float32)
    x_local = np.asarray(x_local, dtype=np.float32)
    const = np.concatenate([np.asarray(h_global, np.float32).ravel(),
                            np.asarray(x_global, np.float32).ravel(),
                            np.asarray(t, np.float32).ravel()])
    ne = af.shape[0]
    delta = np.zeros((h_local.shape[0], D), np.float32)
    CH = 100_000
    for s in range(0, ne, CH):
        e = min(s + CH, ne)
        inp = np.concatenate([h_local[af[s:e]], h_local[at[s:e]], x_local[s:e],
                              np.broadcast_to(const, (e - s, 21))], axis=1).astype(np.float32)
        d_f = np.tanh(np.tanh(np.tanh(inp @ f_W0 + f_b0) @ f_W1 + f_b1) @ f_W2 + f_b2)
        d_t = np.tanh(np.tanh(np.tanh(inp @ t_W0 + t_b0) @ t_W1 + t_b1) @ t_W2 + t_b2)
        _scatter_add(delta, af[s:e], d_f.astype(np.float32))
        _scatter_add(delta, at[s:e], d_t.astype(np.float32))
    return np.tanh(delta).astype(np.float32)


_BASS_CACHE = {}


def _build_bass(ns, nwin, lastrows, ktiles, ntot, ncores, walrus_fix=True):
    import concourse.bass as bass
    import concourse.mybir as mybir
    import concourse.tile as tile

    # walrus in this env rejects Drain instructions carrying >1 sem wait;
    # move each wait onto its own sync nop before the drain.
    def _patched(self, tick_clock, wait_clock):
        from concourse.tile import ScopedClock
        nop0 = self.nc.sync.nop(nofuse=True)
        wait_clock.add_sem_waits(nop0.ins, ScopedClock({None: tick_clock.global_clock}))
        si = nop0.ins.sync_info
        if si is not None and si.on_wait and len(si.on_wait) > 1:
            waits = list(si.on_wait)
            si.on_wait = waits[:1]
            for w in waits[1:]:
                n = self.nc.sync.nop(nofuse=True)
                n.ins.sync_info = mybir.SyncInfo(on_wait=[w], on_update=[])
        self.nc.sync.drain()
        self.nc.all_engine_barrier()
        popped = self.nc._tile_sem_poison_stack.pop()
        assert popped is self._sem_poison
        self.nc.clear_and_free_semaphores(list(self.sems.allocated().values()))
        self.nc.all_engine_barrier()

    tile.TileContext._drain_and_barrier = _patched

    f32 = mybir.dt.float32
    f16 = mybir.dt.float16
    i32 = mybir.dt.int32
    i8 = mybir.dt.int8
    nt_tot = nwin * 2 * ktiles
    slots = nt_tot * 128
    bts = _batch_tiles(ktiles)

    nc = bass.Bass(num_devices=ncores)
    gidx_d = nc.dram_tensor("gidx", [128, 2 * nt_tot], i32, kind="ExternalInput")
    locv_d = nc.dram_tensor("locv", [128, nt_tot], i8, kind="ExternalInput")
    xt_d = nc.dram_tensor("xt", [4, slots], f16, kind="ExternalInput")
    hshard_d = nc.dram_tensor("hshard", [ns, D], f16, kind="ExternalInput")
    hsh_b = nc.dram_tensor("hsh_b", [ns, D], f16)
    htab_d = nc.dram_tensor("hfull", [ntot, D], f16, addr_space="Shared")
    iota_d = nc.dram_tensor("iota", [128, 128], f16, kind="ExternalInput")
    wts = {}
    for p in ("f", "t"):
        wts[p + "w0h"] = nc.dram_tensor(p + "w0h", [128, H], f16, kind="ExternalInput")
        wts[p + "w0x"] = nc.dram_tensor(p + "w0x", [4, H], f16, kind="ExternalInput")
        wts[p + "w1"] = nc.dram_tensor(p + "w1", [H, H], f16, kind="ExternalInput")
        wts[p + "w2"] = nc.dram_tensor(p + "w2", [H, D], f16, kind="ExternalInput")
        wts[p + "b0"] = nc.dram_tensor(p + "b0", [H, 1], f32, kind="ExternalInput")
        wts[p + "b1"] = nc.dram_tensor(p + "b1", [H, 1], f32, kind="ExternalInput")
        wts[p + "b2r"] = nc.dram_tensor(p + "b2r", [128, MAXBT * D], f16, kind="ExternalInput")
    delta_d = nc.dram_tensor("delta", [ns, D], f16, kind="ExternalOutput")

    # all-gather the h shards into a full replicated table before the main
    # body; runs on the gpsimd stream, which also issues the gathers later,
    # so engine program order guarantees completion before first use.
    with nc.Block() as blk, \
         nc.semaphore("ag_dma") as ag_dma, \
         nc.semaphore("ag_cc") as ag_cc:

        @blk.gpsimd
        def _(g):
            g.dma_start(out=hsh_b[:, :], in_=hshard_d[:, :]).then_inc(ag_dma, 16)
            g.wait_ge(ag_dma, 16)
            g.collective_compute(
                "AllGather",
                mybir.AluOpType.bypass,
                replica_groups=[list(range(ncores))],
                ins=[hsh_b.ap().opt()],
                outs=[htab_d.ap().opt()],
            ).then_inc(ag_cc)
            g.wait_ge(ag_cc, 1)

    Tanh = mybir.ActivationFunctionType.Tanh
    MB = MAXBT
    with tile.TileContext(nc) as tc:
        with tc.tile_pool(name="wpool", bufs=1) as wp, \
             tc.tile_pool(name="io", bufs=3) as io, \
             tc.tile_pool(name="act", bufs=2) as ap_, \
             tc.tile_pool(name="ps01", bufs=1, space="PSUM") as ps01, \
             tc.tile_pool(name="psd", bufs=2, space="PSUM") as psdp, \
             tc.tile_pool(name="win", bufs=2, space="PSUM") as winp:
            wt = {}
            for k, dr in wts.items():
                tl = wp.tile(list(dr.shape), dr.dtype, tag="w" + k)
                nc.sync.dma_start(out=tl[:], in_=dr[:])
                wt[k] = tl
            iota = wp.tile([128, 128], f16, tag="iota")
            nc.sync.dma_start(out=iota[:], in_=iota_d[:])

            for w in range(nwin):
                win = winp.tile([128, D], f32, tag="win")
                for pi, p in enumerate(("f", "t")):
                    tbase = (w * 2 + pi) * ktiles
                    off = 0
                    for bi, bt in enumerate(bts):
                        t0 = tbase + off
                        s0 = t0 * 128
                        nsl = bt * 128
                        off += bt
                        gi = io.tile([128, 2 * MB], i32, tag="gi")
                        lo8 = io.tile([128, MB], i8, tag="lo8")
                        xb = io.tile([4, MB * 128], f16, tag="xb")
                        nc.sync.dma_start(out=gi[:, :2 * bt],
                                          in_=gidx_d[:, 2 * t0:2 * (t0 + bt)])
                        nc.sync.dma_start(out=lo8[:, :bt],
                                          in_=locv_d[:, t0:t0 + bt])
                        nc.sync.dma_start(out=xb[:, :nsl],
                                          in_=xt_d[:, s0:s0 + nsl])
                        lo = io.tile([128, MB], f16, tag="lo")
                        nc.vector.tensor_copy(out=lo[:, :bt], in_=lo8[:, :bt])
                        gp = io.tile([128, 2 * MB, D], f16, tag="gp")
                        # HW DGE handles one offset per partition per
                        # instruction; fan out over the 2*bt columns.
                        for j in range(2 * bt):
                            nc.gpsimd.indirect_dma_start(
                                out=gp[:, j, :],
                                out_offset=None,
                                in_=htab_d[:],
                                in_offset=bass.IndirectOffsetOnAxis(
                                    ap=gi[:, j:j + 1], axis=0),
                            )
                        rhs = ap_.tile([128, MB * 128], f16, tag="rhs")
                        for t in range(bt):
                            nc.sync.dma_start_transpose(
                                out=rhs[:, t * 128:(t + 1) * 128],
                                in_=gp[:, 2 * t:2 * t + 2, :])
                        ps0 = ps01.tile([128, MB * 128], f32, tag="ps0")
                        nc.tensor.matmul(out=ps0[:, :nsl], lhsT=wt[p + "w0h"][:],
                                         rhs=rhs[:, :nsl], start=True, stop=False)
                        nc.tensor.matmul(out=ps0[:, :nsl], lhsT=wt[p + "w0x"][:],
                                         rhs=xb[:, :nsl], start=False, stop=True)
                        h1 = ap_.tile([128, MB * 128], f16, tag="h1")
                        nc.scalar.activation(h1[:, :nsl], ps0[:, :nsl], Tanh,
                                             bias=wt[p + "b0"][:, 0:1])
                        ps1 = ps01.tile([128, MB * 128], f32, tag="ps1")
                        nc.tensor.matmul(out=ps1[:, :nsl], lhsT=wt[p + "w1"][:],
                                         rhs=h1[:, :nsl], start=True, stop=True)
                        h2 = ap_.tile([128, MB * 128], f16, tag="h2")
                        nc.scalar.activation(h2[:, :nsl], ps1[:, :nsl], Tanh,
                                             bias=wt[p + "b1"][:, 0:1])
                        psd = psdp.tile([128, MB * D], f32, tag="psd")
                        for t in range(bt):
                            nc.tensor.matmul(out=psd[:, t * D:(t + 1) * D],
                                             lhsT=h2[:, t * 128:(t + 1) * 128],
                                             rhs=wt[p + "w2"][:],
                                             start=True, stop=True)
                        dsb = ap_.tile([128, MB * D], f16, tag="dsb")
                        nc.vector.tensor_tensor(out=dsb[:, :bt * D],
                                                in0=psd[:, :bt * D],
                                                in1=wt[p + "b2r"][:, :bt * D],
                                                op=mybir.AluOpType.add)
                        dtl = ap_.tile([128, MB * D], f16, tag="dtl")
                        nc.scalar.activation(dtl[:, :bt * D], dsb[:, :bt * D],
                                             Tanh)
                        oh = ap_.tile([128, MB, 128], f16, tag="oh")
                        for t in range(bt):
                            nc.vector.tensor_tensor(
                                out=oh[:, t, :],
                                in0=lo[:, t:t + 1].to_broadcast([128, 128]),
                                in1=iota[:],
                                op=mybir.AluOpType.is_equal)
                        for t in range(bt):
                            nc.tensor.matmul(
                                out=win[:],
                                lhsT=oh[:, t, :],
                                rhs=dtl[:, t * D:(t + 1) * D],
                                start=(pi == 0 and bi == 0 and t == 0),
                                stop=(pi == 1 and bi == len(bts) - 1
                                      and t == bt - 1))
                rows = lastrows if w == nwin - 1 else 128
                wout = ap_.tile([128, D], f16, tag="wout")
                nc.scalar.activation(wout[:], win[:], Tanh)
                nc.sync.dma_start(out=delta_d[w * 128:w * 128 + rows, :],
                                  in_=wout[0:rows, :])

    # this walrus rejects any compute instruction carrying >1 sem wait;
    # hoist extra waits onto same-engine nops placed just before it.
    if not walrus_fix:
        return nc
    ctr = 0
    for bb in nc.main_func.blocks:
        new = []
        for ins in bb.instructions:
            si = getattr(ins, "sync_info", None)
            if si is not None and si.on_wait and len(si.on_wait) > 1:
                waits = list(si.on_wait)
                si.on_wait = [waits[-1]]
                for wv in waits[:-1]:
                    ctr += 1
                    nop = mybir.InstNoOp(
                        name=f"wsplit-{ctr}", engine=ins.engine, ins=[], outs=[],
                        sync_info=mybir.SyncInfo(on_wait=[wv], on_update=[]))
                    new.append(nop)
            new.append(ins)
        bb.instructions[:] = new
    return nc


def _prep_slots(af, at, x_local, ncores, ns, nwin, ktiles):
    """Build per-core padded slot arrays in tile-major layout.
    gidx [ncores, 128, 2*NT] i32 (from/to pairs per tile column),
    locv [ncores, 128, NT] i8 (in-window node offset, -1 = pad),
    xt   [ncores, 4, SLOTS] f16 (x features, slot-major),
    spills = list of (pop, edge_indices) that overflowed window capacity."""
    cap = ktiles * 128
    nt_tot = nwin * 2 * ktiles
    slots = nt_tot * 128
    af32 = af.astype(np.int32)
    at32 = at.astype(np.int32)
    gidx = np.zeros((ncores, 128, 2 * nt_tot), np.int32)
    locv = np.full((ncores, 128, nt_tot), -1, np.int8)
    xt = np.zeros((ncores, 4, slots), np.float16)
    xtv = np.ascontiguousarray(x_local.astype(np.float16))
    gflat = gidx.reshape(-1)
    lflat = locv.reshape(-1)
    xflat = xt.reshape(-1)
    spills = []
    for pi, dest in enumerate((af32, at32)):
        core = dest // np.int32(ns)
        node_l = dest - core * np.int32(ns)
        w = node_l >> 7
        loc = (node_l & 127).astype(np.int8)
        cw = core * np.int32(nwin) + w
        order = np.argsort(cw, kind="stable").astype(np.int32)
        counts = np.bincount(cw, minlength=ncores * nwin)
        starts = np.concatenate([[0], np.cumsum(counts)[:-1]])
        rank = (np.arange(len(cw), dtype=np.int32)
                - np.repeat(starts, counts).astype(np.int32))
        ok = rank < cap
        if not ok.all():
            spills.append((pi, order[~ok].astype(np.int64)))
            e_ok = order[ok]
            r = rank[ok]
        else:
            e_ok = order
            r = rank
        # within-core slot: window block of 2*cap, population block of cap
        sc = w[e_ok] * np.int32(2 * cap) + np.int32(pi * cap) + r
        T = sc >> 7
        pp = sc & 127
        c_ok = core[e_ok]
        gbase = ((c_ok * 128 + pp) * (2 * nt_tot)) + 2 * T
        gflat[gbase] = af32[e_ok]
        gflat[gbase + 1] = at32[e_ok]
        lflat[(c_ok * 128 + pp) * nt_tot + T] = loc[e_ok]
        xbase = (c_ok * 4) * slots + sc
        xflat[xbase[:, None] + (np.arange(4, dtype=np.int32) * slots)[None, :]] \
            = xtv[e_ok]
    return gidx, locv, xt, spills


def _prep_weights(inputs):
    const = np.concatenate([np.asarray(inputs["h_global"], np.float32).ravel(),
                            np.asarray(inputs["x_global"], np.float32).ravel(),
                            np.asarray(inputs["t"], np.float32).ravel()])
    wm = {}
    for p in ("f", "t"):
        W0 = np.asarray(inputs[p + "_W0"], np.float32)
        b0 = np.asarray(inputs[p + "_b0"], np.float32)
        b0eff = b0 + const @ W0[132:153]
        wm[p + "w0h"] = np.ascontiguousarray(W0[0:128]).astype(np.float16)
        wm[p + "w0x"] = np.ascontiguousarray(W0[128:132]).astype(np.float16)
        wm[p + "w1"] = np.asarray(inputs[p + "_W1"], np.float32).astype(np.float16)
        wm[p + "w2"] = np.asarray(inputs[p + "_W2"], np.float32).astype(np.float16)
        wm[p + "b0"] = b0eff.reshape(H, 1).astype(np.float32)
        wm[p + "b1"] = np.asarray(inputs[p + "_b1"], np.float32).reshape(H, 1)
        b2 = np.asarray(inputs[p + "_b2"], np.float32)
        wm[p + "b2r"] = np.tile(b2.reshape(1, D), (128, MAXBT)).astype(np.float16)
    wm["iota"] = np.broadcast_to(np.arange(128, dtype=np.float16), (128, 128)).copy()
    return wm


def _fix_spill_nodes(spills, inputs, out):
    """Recompute on host (fp32) every node whose window overflowed device
    capacity; overwrite those rows of `out`. Empty for uniform edge data."""
    if not spills:
        return
    af = np.asarray(inputs["addr_from"]).astype(np.int64)
    at = np.asarray(inputs["addr_to"]).astype(np.int64)
    h = np.asarray(inputs["h_local"], np.float32)
    x = np.asarray(inputs["x_local"], np.float32)
    const = np.concatenate([np.asarray(inputs["h_global"], np.float32).ravel(),
                            np.asarray(inputs["x_global"], np.float32).ravel(),
                            np.asarray(inputs["t"], np.float32).ravel()])
    nodes = np.unique(np.concatenate(
        [(af if pi == 0 else at)[e] for pi, e in spills]))
    node_set = np.zeros(h.shape[0], bool)
    node_set[nodes] = True
    delta = np.zeros((len(nodes), D), np.float32)
    remap = np.full(h.shape[0], -1, np.int64)
    remap[nodes] = np.arange(len(nodes))
    for pi, idx_all in ((0, af), (1, at)):
        p = "f" if pi == 0 else "t"
        edges = np.flatnonzero(node_set[idx_all])
        if not len(edges):
            continue
        inp = np.concatenate([h[af[edges]], h[at[edges]], x[edges],
                              np.broadcast_to(const, (len(edges), 21))], axis=1)
        d = np.tanh(np.tanh(np.tanh(
            inp @ inputs[p + "_W0"] + inputs[p + "_b0"]) @ inputs[p + "_W1"]
            + inputs[p + "_b1"]) @ inputs[p + "_W2"] + inputs[p + "_b2"])
        _scatter_add(delta, remap[idx_all[edges]], d.astype(np.float32))
    out[nodes] = np.tanh(delta)


def _get_exec(nc):
    """Build (once) a cached jitted executor for the bass module: the same
    _bass_exec_p/shard_map lowering run_bass_kernel_spmd uses under axon,
    but with the jitted callable memoized so repeat calls skip retracing."""
    if "exec" in _BASS_CACHE:
        return _BASS_CACHE["exec"]
    import jax
    import concourse.mybir as mybir
    from jax.sharding import Mesh, PartitionSpec
    from jax.experimental.shard_map import shard_map
    from concourse.bass2jax import (_bass_exec_p, install_neuronx_cc_hook,
                                    partition_id_tensor)
    install_neuronx_cc_hook()

    in_names, out_names, out_avals = [], [], []
    pname = nc.partition_id_tensor.name if nc.partition_id_tensor else None
    for alloc in nc.m.functions[0].allocations:
        if not isinstance(alloc, mybir.MemoryLocationSet):
            continue
        name = alloc.memorylocations[0].name
        if alloc.kind == "ExternalInput":
            if name != pname:
                in_names.append(name)
        elif alloc.kind == "ExternalOutput":
            out_names.append(name)
            out_avals.append(jax.core.ShapedArray(
                tuple(alloc.tensor_shape), mybir.dt.np(alloc.dtype)))
    n_params = len(in_names)
    n_outs = len(out_avals)
    all_names = in_names + out_names + ([pname] if pname else [])

    def _body(*args):
        ops = list(args)
        if pname:
            ops.append(partition_id_tensor())
        outs = _bass_exec_p.bind(
            *ops, out_avals=tuple(out_avals), in_names=tuple(all_names),
            out_names=tuple(out_names), lowering_input_output_aliases=(),
            sim_require_finite=True, sim_require_nnan=True, nc=nc)
        return tuple(outs)

    devices = jax.devices()[:NCORES]
    mesh = Mesh(np.asarray(devices), ("core",))
    in_specs = (PartitionSpec("core"),) * (n_params + n_outs)
    out_specs = (PartitionSpec("core"),) * n_outs
    donate = tuple(range(n_params, n_params + n_outs))
    sharded = jax.jit(
        shard_map(_body, mesh=mesh, in_specs=in_specs, out_specs=out_specs,
                  check_rep=False),
        donate_argnums=donate, keep_unused=True)

    from jax.sharding import NamedSharding
    sharding = NamedSharding(mesh, PartitionSpec("core"))
    import jax.numpy as jnp

    # donated output buffers made on-device (zeros never cross the tunnel)
    def _mk_zeros():
        return tuple(jnp.zeros((NCORES * a.shape[0], *a.shape[1:]), a.dtype)
                     for a in out_avals)
    zeros_fn = jax.jit(_mk_zeros, out_shardings=(sharding,) * n_outs)

    ex = (sharded, in_names, out_names, out_avals, sharding, zeros_fn)
    _BASS_CACHE["exec"] = ex
    return ex


def _kernel_bass(addr_from, addr_to, h_local, h_global, x_local, x_global, t,
                 f_W0, f_b0, f_W1, f_b1, f_W2, f_b2,
                 t_W0, t_b0, t_W1, t_b1, t_W2, t_b2, trace=False):
    import sys
    if "/opt/trn_rl_repo" not in sys.path:
        sys.path.insert(0, "/opt/trn_rl_repo")

    inputs = dict(addr_from=addr_from, addr_to=addr_to, h_local=h_local,
                  h_global=h_global, x_local=x_local, x_global=x_global, t=t,
                  f_W0=f_W0, f_b0=f_b0, f_W1=f_W1, f_b1=f_b1, f_W2=f_W2,
                  f_b2=f_b2, t_W0=t_W0, t_b0=t_b0, t_W1=t_W1, t_b1=t_b1,
                  t_W2=t_W2, t_b2=t_b2)
    af = np.asarray(addr_from).astype(np.int64)
    at = np.asarray(addr_to).astype(np.int64)
    h = np.asarray(h_local, np.float32)
    x = np.asarray(x_local, np.float32)

    key = (N, E)
    if key not in _BASS_CACHE:
        _BASS_CACHE[key] = _build_bass(NS, NWIN, LASTROWS, KTILES, N, NCORES)
    nc = _BASS_CACHE[key]
    sharded, in_names, out_names, out_avals, sharding, zeros_fn = _get_exec(nc)

    import jax

    # stage prep-independent inputs first: their h2d transfer overlaps the
    # host-side slot preparation below.
    staged = {}
    wm = _prep_weights(inputs)
    htab = np.ascontiguousarray(h.astype(np.float16))
    staged["hshard"] = jax.device_put(htab, sharding)
    for k, v in wm.items():
        staged[k] = jax.device_put(np.tile(v, (NCORES, 1)), sharding)
    zeros = zeros_fn()

    gidx, locv, xt, spills = _prep_slots(af, at, x, NCORES, NS, NWIN, KTILES)
    staged["gidx"] = jax.device_put(gidx.reshape(-1, gidx.shape[-1]), sharding)
    staged["locv"] = jax.device_put(locv.reshape(-1, locv.shape[-1]), sharding)
    staged["xt"] = jax.device_put(xt.reshape(-1, xt.shape[-1]), sharding)

    out_arrs = sharded(*[staged[n] for n in in_names], *zeros)
    out = np.asarray(out_arrs[out_names.index("delta")]).astype(np.float32)
    _fix_spill_nodes(spills, inputs, out)
    return out


def kernel(**inputs):
    try:
        return _kernel_bass(**inputs)
    except Exception:
        import traceback
        traceback.print_exc()
        return _kernel_numpy(**inputs)


# revision 25
# speedup vs baseline: 28.1685x; 1.2559x over previous
import numpy as np

# nn_LocalDynamics GNN message passing.
#   delta[n] = sum_e tanh(fMLP(inp_e))[addr_from=n] + tanh(tMLP(inp_e))[addr_to=n]
#   out = tanh(delta).  inp_e = [h[from], h[to], x_e, hg, xg, t] (153 dims).
#
# Destination-sharded design: each core owns nodes [c*12500, (c+1)*12500).
# Every edge yields two "slots": an f-slot on the core owning addr_from and a
# t-slot on the core owning addr_to.  Slots are grouped by 128-node windows of
# the owning core; each window holds a fixed KTILES tiles of 128 slots per
# population (f/t), host-padded.  On device, per batch of tiles:
#   AllGather h shards -> indirect-DMA gather of (h[from], h[to]) row pairs ->
#   XBAR transpose to feature-major -> fp16 MLP -> slot-major final layer ->
#   one-hot matmul accumulates the window's delta in PSUM -> tanh -> fp16 out.
# Host recomputes any overflowed windows (empty for uniform edges).

N = 100_000
E = 800_000
D = 64
H = 128
NCORES = 8
NS = N // NCORES            # nodes per core (12500)
NWIN = (NS + 127) // 128    # windows per core (98)
LASTROWS = NS - (NWIN - 1) * 128   # rows in last window (84)
KTILES = 10                 # 128-slot tiles per population per window
NT = NWIN * 2 * KTILES      # tiles per core (1960)
SLOTS = NT * 128            # padded slots per core (250880)
MAXBT = 4                   # max tiles per batch


def _batch_tiles(ktiles):
    out = []
    k = ktiles
    while k > 0:
        out.append(min(MAXBT, k))
        k -= out[-1]
    return out


def _scatter_add(delta, idx, vals):
    o = np.argsort(idx, kind="stable")
    si = idx[o]
    sv = vals[o]
    starts = np.flatnonzero(np.r_[True, si[1:] != si[:-1]])
    sums = np.add.reduceat(sv, starts, axis=0)
    np.add.at(delta, si[starts], sums)


def _kernel_numpy(addr_from, addr_to, h_local, h_global, x_local, x_global, t,
                  f_W0, f_b0, f_W1, f_b1, f_W2, f_b2,
                  t_W0, t_b0, t_W1, t_b1, t_W2, t_b2):
    af = np.asarray(addr_from).astype(np.int64)
    at = np.asarray(addr_to).astype(np.int64)
    h_local = np.asarray(h_local, dtype=np.float32)
    x_local = np.asarray(x_local, dtype=np.float32)
    const = np.concatenate([np.asarray(h_global, np.float32).ravel(),
                            np.asarray(x_global, np.float32).ravel(),
                            np.asarray(t, np.float32).ravel()])
    ne = af.shape[0]
    delta = np.zeros((h_local.shape[0], D), np.float32)
    CH = 100_000
    for s in range(0, ne, CH):
        e = min(s + CH, ne)
        inp = np.concatenate([h_local[af[s:e]], h_local[at[s:e]], x_local[s:e],
                              np.broadcast_to(const, (e - s, 21))], axis=1).astype(np.float32)
        d_f = np.tanh(np.tanh(np.tanh(inp @ f_W0 + f_b0) @ f_W1 + f_b1) @ f_W2 + f_b2)
        d_t = np.tanh(np.tanh(np.tanh(inp @ t_W0 + t_b0) @ t_W1 + t_b1) @ t_W2 + t_b2)
        _scatter_add(delta, af[s:e], d_f.astype(np.float32))
        _scatter_add(delta, at[s:e], d_t.astype(np.float32))
    return np.tanh(delta).astype(np.float32)


_BASS_CACHE = {}


def _build_bass(ns, nwin, lastrows, ktiles, ntot, ncores, walrus_fix=True):
    import concourse.bass as bass
    import concourse.mybir as mybir
    import concourse.tile as tile

    # walrus in this env rejects Drain instructions carrying >1 sem wait;
    # move each wait onto its own sync nop before the drain.
    def _patched(self, tick_clock, wait_clock):
        from concourse.tile import ScopedClock
        nop0 = self.nc.sync.nop(nofuse=True)
        wait_clock.add_sem_waits(nop0.ins, ScopedClock({None: tick_clock.global_clock}))
        si = nop0.ins.sync_info
        if si is not None and si.on_wait and len(si.on_wait) > 1:
            waits = list(si.on_wait)
            si.on_wait = waits[:1]
            for w in waits[1:]:
                n = self.nc.sync.nop(nofuse=True)
                n.ins.sync_info = mybir.SyncInfo(on_wait=[w], on_update=[])
        self.nc.sync.drain()
        self.nc.all_engine_barrier()
        popped = self.nc._tile_sem_poison_stack.pop()
        assert popped is self._sem_poison
        self.nc.clear_and_free_semaphores(list(self.sems.allocated().values()))
        self.nc.all_engine_barrier()

    tile.TileContext._drain_and_barrier = _patched

    f32 = mybir.dt.float32
    f16 = mybir.dt.float16
    i32 = mybir.dt.int32
    i8 = mybir.dt.int8
    nt_tot = nwin * 2 * ktiles
    slots = nt_tot * 128
    bts = _batch_tiles(ktiles)

    nc = bass.Bass(num_devices=ncores)
    gidx_d = nc.dram_tensor("gidx", [128, 2 * nt_tot], i32, kind="ExternalInput")
    locv_d = nc.dram_tensor("locv", [128, nt_tot], i8, kind="ExternalInput")
    xt_d = nc.dram_tensor("xt", [4, slots], f16, kind="ExternalInput")
    hshard_d = nc.dram_tensor("hshard", [ns, D], f16, kind="ExternalInput")
    hsh_b = nc.dram_tensor("hsh_b", [ns, D], f16)
    htab_d = nc.dram_tensor("hfull", [ntot, D], f16, addr_space="Shared")
    iota_d = nc.dram_tensor("iota", [128, 128], f16, kind="ExternalInput")
    wts = {}
    for p in ("f", "t"):
        wts[p + "w0h"] = nc.dram_tensor(p + "w0h", [128, H], f16, kind="ExternalInput")
        wts[p + "w0x"] = nc.dram_tensor(p + "w0x", [4, H], f16, kind="ExternalInput")
        wts[p + "w1"] = nc.dram_tensor(p + "w1", [H, H], f16, kind="ExternalInput")
        wts[p + "w2"] = nc.dram_tensor(p + "w2", [H, D], f16, kind="ExternalInput")
        wts[p + "b0"] = nc.dram_tensor(p + "b0", [H, 1], f32, kind="ExternalInput")
        wts[p + "b1"] = nc.dram_tensor(p + "b1", [H, 1], f32, kind="ExternalInput")
        wts[p + "b2r"] = nc.dram_tensor(p + "b2r", [128, MAXBT * D], f16, kind="ExternalInput")
    delta_d = nc.dram_tensor("delta", [ns, D], f16, kind="ExternalOutput")

    # all-gather the h shards into a full replicated table before the main
    # body; runs on the gpsimd stream, which also issues the gathers later,
    # so engine program order guarantees completion before first use.
    with nc.Block() as blk, \
         nc.semaphore("ag_dma") as ag_dma, \
         nc.semaphore("ag_cc") as ag_cc:

        @blk.gpsimd
        def _(g):
            g.dma_start(out=hsh_b[:, :], in_=hshard_d[:, :]).then_inc(ag_dma, 16)
            g.wait_ge(ag_dma, 16)
            g.collective_compute(
                "AllGather",
                mybir.AluOpType.bypass,
                replica_groups=[list(range(ncores))],
                ins=[hsh_b.ap().opt()],
                outs=[htab_d.ap().opt()],
            ).then_inc(ag_cc)
            g.wait_ge(ag_cc, 1)

    Tanh = mybir.ActivationFunctionType.Tanh
    MB = MAXBT
    with tile.TileContext(nc) as tc:
        with tc.tile_pool(name="wpool", bufs=1) as wp, \
             tc.tile_pool(name="io", bufs=3) as io, \
             tc.tile_pool(name="act", bufs=2) as ap_, \
             tc.tile_pool(name="ps01", bufs=1, space="PSUM") as ps01, \
             tc.tile_pool(name="psd", bufs=2, space="PSUM") as psdp, \
             tc.tile_pool(name="win", bufs=2, space="PSUM") as winp:
            wt = {}
            for k, dr in wts.items():
                tl = wp.tile(list(dr.shape), dr.dtype, tag="w" + k)
                nc.sync.dma_start(out=tl[:], in_=dr[:])
                wt[k] = tl
            iota = wp.tile([128, 128], f16, tag="iota")
            nc.sync.dma_start(out=iota[:], in_=iota_d[:])

            for w in range(nwin):
                win = winp.tile([128, D], f32, tag="win")
                for pi, p in enumerate(("f", "t")):
                    tbase = (w * 2 + pi) * ktiles
                    off = 0
                    for bi, bt in enumerate(bts):
                        t0 = tbase + off
                        s0 = t0 * 128
                        nsl = bt * 128
                        off += bt
                        gi = io.tile([128, 2 * MB], i32, tag="gi")
                        lo8 = io.tile([128, MB], i8, tag="lo8")
                        xb = io.tile([4, MB * 128], f16, tag="xb")
                        nc.sync.dma_start(out=gi[:, :2 * bt],
                                          in_=gidx_d[:, 2 * t0:2 * (t0 + bt)])
                        nc.sync.dma_start(out=lo8[:, :bt],
                                          in_=locv_d[:, t0:t0 + bt])
                        nc.sync.dma_start(out=xb[:, :nsl],
                                          in_=xt_d[:, s0:s0 + nsl])
                        lo = io.tile([128, MB], f16, tag="lo")
                        nc.vector.tensor_copy(out=lo[:, :bt], in_=lo8[:, :bt])
                        gp = io.tile([128, 2 * MB, D], f16, tag="gp")
                        # HW DGE handles one offset per partition per
                        # instruction; fan out over the 2*bt columns.
                        for j in range(2 * bt):
                            nc.gpsimd.indirect_dma_start(
                                out=gp[:, j, :],
                                out_offset=None,
                                in_=htab_d[:],
                                in_offset=bass.IndirectOffsetOnAxis(
                                    ap=gi[:, j:j + 1], axis=0),
                            )
                        rhs = ap_.tile([128, MB * 128], f16, tag="rhs")
                        for t in range(bt):
                            nc.sync.dma_start_transpose(
                                out=rhs[:, t * 128:(t + 1) * 128],
                                in_=gp[:, 2 * t:2 * t + 2, :])
                        ps0 = ps01.tile([128, MB * 128], f32, tag="ps0")
                        nc.tensor.matmul(out=ps0[:, :nsl], lhsT=wt[p + "w0h"][:],
                                         rhs=rhs[:, :nsl], start=True, stop=False)
                        nc.tensor.matmul(out=ps0[:, :nsl], lhsT=wt[p + "w0x"][:],
                                         rhs=xb[:, :nsl], start=False, stop=True)
                        h1 = ap_.tile([128, MB * 128], f16, tag="h1")
                        nc.scalar.activation(h1[:, :nsl], ps0[:, :nsl], Tanh,
                                             bias=wt[p + "b0"][:, 0:1])
                        ps1 = ps01.tile([128, MB * 128], f32, tag="ps1")
                        nc.tensor.matmul(out=ps1[:, :nsl], lhsT=wt[p + "w1"][:],
                                         rhs=h1[:, :nsl], start=True, stop=True)
                        h2 = ap_.tile([128, MB * 128], f16, tag="h2")
                        nc.scalar.activation(h2[:, :nsl], ps1[:, :nsl], Tanh,
                                             bias=wt[p + "b1"][:, 0:1])
                        psd = psdp.tile([128, MB * D], f32, tag="psd")
                        for t in range(bt):
                            nc.tensor.matmul(out=psd[:, t * D:(t + 1) * D],
                                             lhsT=h2[:, t * 128:(t + 1) * 128],
                                             rhs=wt[p + "w2"][:],
                                             start=True, stop=True)
                        dsb = ap_.tile([128, MB * D], f16, tag="dsb")
                        nc.vector.tensor_tensor(out=dsb[:, :bt * D],
                                                in0=psd[:, :bt * D],
                                                in1=wt[p + "b2r"][:, :bt * D],
                                                op=mybir.AluOpType.add)
                        dtl = ap_.tile([128, MB * D], f16, tag="dtl")
                        nc.scalar.activation(dtl[:, :bt * D], dsb[:, :bt * D],
                                             Tanh)
                        oh = ap_.tile([128, MB, 128], f16, tag="oh")
                        for t in range(bt):
                            nc.vector.tensor_tensor(
                                out=oh[:, t, :],
                                in0=lo[:, t:t + 1].to_broadcast([128, 128]),
                                in1=iota[:],
                                op=mybir.AluOpType.is_equal)
                        for t in range(bt):
                            nc.tensor.matmul(
                                out=win[:],
                                lhsT=oh[:, t, :],
                                rhs=dtl[:, t * D:(t + 1) * D],
                                start=(pi == 0 and bi == 0 and t == 0),
                                stop=(pi == 1 and bi == len(bts) - 1
                                      and t == bt - 1))
                rows = lastrows if w == nwin - 1 else 128
                wout = ap_.tile([128, D], f16, tag="wout")
                nc.scalar.activation(wout[:], win[:], Tanh)
                nc.sync.dma_start(out=delta_d[w * 128:w * 128 + rows, :],
                                  in_=wout[0:rows, :])

    # this walrus rejects any compute instruction carrying >1 sem wait;
    # hoist extra waits onto same-engine nops placed just before it.
    if not walrus_fix:
        return nc
    ctr = 0
    for bb in nc.main_func.blocks:
        new = []
        for ins in bb.instructions:
            si = getattr(ins, "sync_info", None)
            if si is not None and si.on_wait and len(si.on_wait) > 1:
                waits = list(si.on_wait)
                si.on_wait = [waits[-1]]
                for wv in waits[:-1]:
                    ctr += 1
                    nop = mybir.InstNoOp(
                        name=f"wsplit-{ctr}", engine=ins.engine, ins=[], outs=[],
                        sync_info=mybir.SyncInfo(on_wait=[wv], on_update=[]))
                    new.append(nop)
            new.append(ins)
        bb.instructions[:] = new
    return nc


def _get_fill_nb():
    if "fill_nb" in _BASS_CACHE:
        return _BASS_CACHE["fill_nb"]
    try:
        import numba
    except Exception:
        _BASS_CACHE["fill_nb"] = None
        return None

    @numba.njit(cache=True)
    def _fill(af32, at32, xu16, gflat, lflat, xflat, counters, spill_e,
              spill_pi, ns, nwin, cap, nt_tot, slots):
        ne = af32.size
        nsp = 0
        for e in range(ne):
            for pi in range(2):
                dest = af32[e] if pi == 0 else at32[e]
                c = dest // ns
                nl = dest - c * ns
                w = nl >> 7
                loc = nl & 127
                idx = (c * nwin + w) * 2 + pi
                r = counters[idx]
                counters[idx] = r + 1
                if r >= cap:
                    spill_e[nsp] = e
                    spill_pi[nsp] = pi
                    nsp += 1
                else:
                    sc = w * (2 * cap) + pi * cap + r
                    tt = sc >> 7
                    pp = sc & 127
                    gbase = (c * 128 + pp) * (2 * nt_tot) + 2 * tt
                    gflat[gbase] = af32[e]
                    gflat[gbase + 1] = at32[e]
                    lflat[(c * 128 + pp) * nt_tot + tt] = loc
                    xb = (c * 4) * slots + sc
                    for rr in range(4):
                        xflat[xb + rr * slots] = xu16[e, rr]
        return nsp

    _BASS_CACHE["fill_nb"] = _fill
    return _fill


def _prep_slots(af, at, x_local, ncores, ns, nwin, ktiles):
    """Build per-core padded slot arrays in tile-major layout.
    gidx [ncores, 128, 2*NT] i32 (from/to pairs per tile column),
    locv [ncores, 128, NT] i8 (in-window node offset, -1 = pad),
    xt   [ncores, 4, SLOTS] f16 (x features, slot-major),
    spills = list of (pop, edge_indices) that overflowed window capacity."""
    fill = _get_fill_nb()
    if fill is not None:
        cap = ktiles * 128
        nt_tot = nwin * 2 * ktiles
        slots = nt_tot * 128
        af32 = np.ascontiguousarray(af.astype(np.int32))
        at32 = np.ascontiguousarray(at.astype(np.int32))
        xu16 = np.ascontiguousarray(x_local.astype(np.float16)).view(np.uint16)
        gidx = np.zeros((ncores, 128, 2 * nt_tot), np.int32)
        locv = np.full((ncores, 128, nt_tot), -1, np.int8)
        xt16 = np.zeros((ncores, 4, slots), np.uint16)
        counters = np.zeros(ncores * nwin * 2, np.int32)
        spill_e = np.empty(af32.size * 2, np.int64)
        spill_pi = np.empty(af32.size * 2, np.int8)
        nsp = fill(af32, at32, xu16, gidx.reshape(-1), locv.reshape(-1),
                   xt16.reshape(-1), counters, spill_e, spill_pi,
                   ns, nwin, cap, nt_tot, slots)
        spills = []
        for pi in (0, 1):
            sel = spill_e[:nsp][spill_pi[:nsp] == pi]
            if len(sel):
                spills.append((pi, sel))
        return gidx, locv, xt16.view(np.float16), spills
    return _prep_slots_np(af, at, x_local, ncores, ns, nwin, ktiles)


def _prep_slots_np(af, at, x_local, ncores, ns, nwin, ktiles):
    cap = ktiles * 128
    nt_tot = nwin * 2 * ktiles
    slots = nt_tot * 128
    af32 = af.astype(np.int32)
    at32 = at.astype(np.int32)
    gidx = np.zeros((ncores, 128, 2 * nt_tot), np.int32)
    locv = np.full((ncores, 128, nt_tot), -1, np.int8)
    xt = np.zeros((ncores, 4, slots), np.float16)
    xtv = np.ascontiguousarray(x_local.astype(np.float16))
    gflat = gidx.reshape(-1)
    lflat = locv.reshape(-1)
    xflat = xt.reshape(-1)
    spills = []
    for pi, dest in enumerate((af32, at32)):
        core = dest // np.int32(ns)
        node_l = dest - core * np.int32(ns)
        w = node_l >> 7
        loc = (node_l & 127).astype(np.int8)
        cw = core * np.int32(nwin) + w
        order = np.argsort(cw, kind="stable").astype(np.int32)
        counts = np.bincount(cw, minlength=ncores * nwin)
        starts = np.concatenate([[0], np.cumsum(counts)[:-1]])
        rank = (np.arange(len(cw), dtype=np.int32)
                - np.repeat(starts, counts).astype(np.int32))
        ok = rank < cap
        if not ok.all():
            spills.append((pi, order[~ok].astype(np.int64)))
            e_ok = order[ok]
            r = rank[ok]
        else:
            e_ok = order
            r = rank
        # within-core slot: window block of 2*cap, population block of cap
        sc = w[e_ok] * np.int32(2 * cap) + np.int32(pi * cap) + r
        T = sc >> 7
        pp = sc & 127
        c_ok = core[e_ok]
        gbase = ((c_ok * 128 + pp) * (2 * nt_tot)) + 2 * T
        gflat[gbase] = af32[e_ok]
        gflat[gbase + 1] = at32[e_ok]
        lflat[(c_ok * 128 + pp) * nt_tot + T] = loc[e_ok]
        xbase = (c_ok * 4) * slots + sc
        xflat[xbase[:, None] + (np.arange(4, dtype=np.int32) * slots)[None, :]] \
            = xtv[e_ok]
    return gidx, locv, xt, spills


def _prep_weights(inputs):
    const = np.concatenate([np.asarray(inputs["h_global"], np.float32).ravel(),
                            np.asarray(inputs["x_global"], np.float32).ravel(),
                            np.asarray(inputs["t"], np.float32).ravel()])
    wm = {}
    for p in ("f", "t"):
        W0 = np.asarray(inputs[p + "_W0"], np.float32)
        b0 = np.asarray(inputs[p + "_b0"], np.float32)
        b0eff = b0 + const @ W0[132:153]
        wm[p + "w0h"] = np.ascontiguousarray(W0[0:128]).astype(np.float16)
        wm[p + "w0x"] = np.ascontiguousarray(W0[128:132]).astype(np.float16)
        wm[p + "w1"] = np.asarray(inputs[p + "_W1"], np.float32).astype(np.float16)
        wm[p + "w2"] = np.asarray(inputs[p + "_W2"], np.float32).astype(np.float16)
        wm[p + "b0"] = b0eff.reshape(H, 1).astype(np.float32)
        wm[p + "b1"] = np.asarray(inputs[p + "_b1"], np.float32).reshape(H, 1)
        b2 = np.asarray(inputs[p + "_b2"], np.float32)
        wm[p + "b2r"] = np.tile(b2.reshape(1, D), (128, MAXBT)).astype(np.float16)
    wm["iota"] = np.broadcast_to(np.arange(128, dtype=np.float16), (128, 128)).copy()
    return wm


def _fix_spill_nodes(spills, inputs, out):
    """Recompute on host (fp32) every node whose window overflowed device
    capacity; overwrite those rows of `out`. Empty for uniform edge data."""
    if not spills:
        return
    af = np.asarray(inputs["addr_from"]).astype(np.int64)
    at = np.asarray(inputs["addr_to"]).astype(np.int64)
    h = np.asarray(inputs["h_local"], np.float32)
    x = np.asarray(inputs["x_local"], np.float32)
    const = np.concatenate([np.asarray(inputs["h_global"], np.float32).ravel(),
                            np.asarray(inputs["x_global"], np.float32).ravel(),
                            np.asarray(inputs["t"], np.float32).ravel()])
    nodes = np.unique(np.concatenate(
        [(af if pi == 0 else at)[e] for pi, e in spills]))
    node_set = np.zeros(h.shape[0], bool)
    node_set[nodes] = True
    delta = np.zeros((len(nodes), D), np.float32)
    remap = np.full(h.shape[0], -1, np.int64)
    remap[nodes] = np.arange(len(nodes))
    for pi, idx_all in ((0, af), (1, at)):
        p = "f" if pi == 0 else "t"
        edges = np.flatnonzero(node_set[idx_all])
        if not len(edges):
            continue
        inp = np.concatenate([h[af[edges]], h[at[edges]], x[edges],
                              np.broadcast_to(const, (len(edges), 21))], axis=1)
        d = np.tanh(np.tanh(np.tanh(
            inp @ inputs[p + "_W0"] + inputs[p + "_b0"]) @ inputs[p + "_W1"]
            + inputs[p + "_b1"]) @ inputs[p + "_W2"] + inputs[p + "_b2"])
        _scatter_add(delta, remap[idx_all[edges]], d.astype(np.float32))
    out[nodes] = np.tanh(delta)


def _get_exec(nc):
    """Build (once) a cached jitted executor for the bass module: the same
    _bass_exec_p/shard_map lowering run_bass_kernel_spmd uses under axon,
    but with the jitted callable memoized so repeat calls skip retracing."""
    if "exec" in _BASS_CACHE:
        return _BASS_CACHE["exec"]
    import jax
    import concourse.mybir as mybir
    from jax.sharding import Mesh, PartitionSpec
    from jax.experimental.shard_map import shard_map
    from concourse.bass2jax import (_bass_exec_p, install_neuronx_cc_hook,
                                    partition_id_tensor)
    install_neuronx_cc_hook()

    in_names, out_names, out_avals = [], [], []
    pname = nc.partition_id_tensor.name if nc.partition_id_tensor else None
    for alloc in nc.m.functions[0].allocations:
        if not isinstance(alloc, mybir.MemoryLocationSet):
            continue
        name = alloc.memorylocations[0].name
        if alloc.kind == "ExternalInput":
            if name != pname:
                in_names.append(name)
        elif alloc.kind == "ExternalOutput":
            out_names.append(name)
            out_avals.append(jax.core.ShapedArray(
                tuple(alloc.tensor_shape), mybir.dt.np(alloc.dtype)))
    n_params = len(in_names)
    n_outs = len(out_avals)
    all_names = in_names + out_names + ([pname] if pname else [])

    def _body(*args):
        ops = list(args)
        if pname:
            ops.append(partition_id_tensor())
        outs = _bass_exec_p.bind(
            *ops, out_avals=tuple(out_avals), in_names=tuple(all_names),
            out_names=tuple(out_names), lowering_input_output_aliases=(),
            sim_require_finite=True, sim_require_nnan=True, nc=nc)
        return tuple(outs)

    devices = jax.devices()[:NCORES]
    mesh = Mesh(np.asarray(devices), ("core",))
    in_specs = (PartitionSpec("core"),) * (n_params + n_outs)
    out_specs = (PartitionSpec("core"),) * n_outs
    donate = tuple(range(n_params, n_params + n_outs))
    sharded = jax.jit(
        shard_map(_body, mesh=mesh, in_specs=in_specs, out_specs=out_specs,
                  check_rep=False),
        donate_argnums=donate, keep_unused=True)

    from jax.sharding import NamedSharding
    sharding = NamedSharding(mesh, PartitionSpec("core"))
    import jax.numpy as jnp

    # donated output buffers made on-device (zeros never cross the tunnel)
    def _mk_zeros():
        return tuple(jnp.zeros((NCORES * a.shape[0], *a.shape[1:]), a.dtype)
                     for a in out_avals)
    zeros_fn = jax.jit(_mk_zeros, out_shardings=(sharding,) * n_outs)

    ex = (sharded, in_names, out_names, out_avals, sharding, zeros_fn)
    _BASS_CACHE["exec"] = ex
    return ex


def _kernel_bass(addr_from, addr_to, h_local, h_global, x_local, x_global, t,
                 f_W0, f_b0, f_W1, f_b1, f_W2, f_b2,
                 t_W0, t_b0, t_W1, t_b1, t_W2, t_b2, trace=False):
    import sys
    if "/opt/trn_rl_repo" not in sys.path:
        sys.path.insert(0, "/opt/trn_rl_repo")

    inputs = dict(addr_from=addr_from, addr_to=addr_to, h_local=h_local,
                  h_global=h_global, x_local=x_local, x_global=x_global, t=t,
                  f_W0=f_W0, f_b0=f_b0, f_W1=f_W1, f_b1=f_b1, f_W2=f_W2,
                  f_b2=f_b2, t_W0=t_W0, t_b0=t_b0, t_W1=t_W1, t_b1=t_b1,
                  t_W2=t_W2, t_b2=t_b2)
    af = np.asarray(addr_from).astype(np.int64)
    at = np.asarray(addr_to).astype(np.int64)
    h = np.asarray(h_local, np.float32)
    x = np.asarray(x_local, np.float32)

    key = (N, E)
    if key not in _BASS_CACHE:
        _BASS_CACHE[key] = _build_bass(NS, NWIN, LASTROWS, KTILES, N, NCORES)
    nc = _BASS_CACHE[key]
    sharded, in_names, out_names, out_avals, sharding, zeros_fn = _get_exec(nc)

    import jax

    # stage prep-independent inputs first: their h2d transfer overlaps the
    # host-side slot preparation below.
    staged = {}
    wm = _prep_weights(inputs)
    htab = np.ascontiguousarray(h.astype(np.float16))
    staged["hshard"] = jax.device_put(htab, sharding)
    for k, v in wm.items():
        staged[k] = jax.device_put(np.tile(v, (NCORES, 1)), sharding)
    zeros = zeros_fn()

    gidx, locv, xt, spills = _prep_slots(af, at, x, NCORES, NS, NWIN, KTILES)
    staged["gidx"] = jax.device_put(gidx.reshape(-1, gidx.shape[-1]), sharding)
    staged["locv"] = jax.device_put(locv.reshape(-1, locv.shape[-1]), sharding)
    staged["xt"] = jax.device_put(xt.reshape(-1, xt.shape[-1]), sharding)

    out_arrs = sharded(*[staged[n] for n in in_names], *zeros)
    out = np.asarray(out_arrs[out_names.index("delta")]).astype(np.float32)
    _fix_spill_nodes(spills, inputs, out)
    return out


def kernel(**inputs):
    try:
        return _kernel_bass(**inputs)
    except Exception:
        import traceback
        traceback.print_exc()
        return _kernel_numpy(**inputs)


# revision 34
# speedup vs baseline: 32.7758x; 1.1636x over previous
import numpy as np

# nn_LocalDynamics GNN message passing.
#   delta[n] = sum_e tanh(fMLP(inp_e))[addr_from=n] + tanh(tMLP(inp_e))[addr_to=n]
#   out = tanh(delta).  inp_e = [h[from], h[to], x_e, hg, xg, t] (153 dims).
#
# Destination-sharded design: each core owns nodes [c*12500, (c+1)*12500).
# Every edge yields two "slots": an f-slot on the core owning addr_from and a
# t-slot on the core owning addr_to.  Slots are grouped by 128-node windows of
# the owning core; each window holds a fixed KTILES tiles of 128 slots per
# population (f/t), host-padded.  On device, per batch of tiles:
#   AllGather h shards -> indirect-DMA gather of (h[from], h[to]) row pairs ->
#   XBAR transpose to feature-major -> fp16 MLP -> slot-major final layer ->
#   one-hot matmul accumulates the window's delta in PSUM -> tanh -> fp16 out.
# Host recomputes any overflowed windows (empty for uniform edges).

N = 100_000
E = 800_000
D = 64
H = 128
NCORES = 8
NS = N // NCORES            # nodes per core (12500)
NWIN = (NS + 127) // 128    # windows per core (98)
LASTROWS = NS - (NWIN - 1) * 128   # rows in last window (84)
KTILES = 10                 # 128-slot tiles per population per window
NT = NWIN * 2 * KTILES      # tiles per core (1960)
SLOTS = NT * 128            # padded slots per core (250880)
MAXBT = 4                   # max tiles per batch


def _batch_tiles(ktiles):
    out = []
    k = ktiles
    while k > 0:
        out.append(min(MAXBT, k))
        k -= out[-1]
    return out


def _scatter_add(delta, idx, vals):
    o = np.argsort(idx, kind="stable")
    si = idx[o]
    sv = vals[o]
    starts = np.flatnonzero(np.r_[True, si[1:] != si[:-1]])
    sums = np.add.reduceat(sv, starts, axis=0)
    np.add.at(delta, si[starts], sums)


def _kernel_numpy(addr_from, addr_to, h_local, h_global, x_local, x_global, t,
                  f_W0, f_b0, f_W1, f_b1, f_W2, f_b2,
                  t_W0, t_b0, t_W1, t_b1, t_W2, t_b2):
    af = np.asarray(addr_from).astype(np.int64)
    at = np.asarray(addr_to).astype(np.int64)
    h_local = np.asarray(h_local, dtype=np.float32)
    x_local = np.asarray(x_local, dtype=np.float32)
    const = np.concatenate([np.asarray(h_global, np.float32).ravel(),
                            np.asarray(x_global, np.float32).ravel(),
                            np.asarray(t, np.float32).ravel()])
    ne = af.shape[0]
    delta = np.zeros((h_local.shape[0], D), np.float32)
    CH = 100_000
    for s in range(0, ne, CH):
        e = min(s + CH, ne)
        inp = np.concatenate([h_local[af[s:e]], h_local[at[s:e]], x_local[s:e],
                              np.broadcast_to(const, (e - s, 21))], axis=1).astype(np.float32)
        d_f = np.tanh(np.tanh(np.tanh(inp @ f_W0 + f_b0) @ f_W1 + f_b1) @ f_W2 + f_b2)
        d_t = np.tanh(np.tanh(np.tanh(inp @ t_W0 + t_b0) @ t_W1 + t_b1) @ t_W2 + t_b2)
        _scatter_add(delta, af[s:e], d_f.astype(np.float32))
        _scatter_add(delta, at[s:e], d_t.astype(np.float32))
    return np.tanh(delta).astype(np.float32)


_BASS_CACHE = {}


def _build_bass(ns, nwin, lastrows, ktiles, ntot, ncores, walrus_fix=True):
    import concourse.bass as bass
    import concourse.mybir as mybir
    import concourse.tile as tile

    # walrus in this env rejects Drain instructions carrying >1 sem wait;
    # move each wait onto its own sync nop before the drain.
    def _patched(self, tick_clock, wait_clock):
        from concourse.tile import ScopedClock
        nop0 = self.nc.sync.nop(nofuse=True)
        wait_clock.add_sem_waits(nop0.ins, ScopedClock({None: tick_clock.global_clock}))
        si = nop0.ins.sync_info
        if si is not None and si.on_wait and len(si.on_wait) > 1:
            waits = list(si.on_wait)
            si.on_wait = waits[:1]
            for w in waits[1:]:
                n = self.nc.sync.nop(nofuse=True)
                n.ins.sync_info = mybir.SyncInfo(on_wait=[w], on_update=[])
        self.nc.sync.drain()
        self.nc.all_engine_barrier()
        popped = self.nc._tile_sem_poison_stack.pop()
        assert popped is self._sem_poison
        self.nc.clear_and_free_semaphores(list(self.sems.allocated().values()))
        self.nc.all_engine_barrier()

    tile.TileContext._drain_and_barrier = _patched

    f32 = mybir.dt.float32
    f16 = mybir.dt.float16
    i32 = mybir.dt.int32
    i8 = mybir.dt.int8
    nt_tot = nwin * 2 * ktiles
    slots = nt_tot * 128
    bts = _batch_tiles(ktiles)

    i16 = mybir.dt.int16
    nc = bass.Bass(num_devices=ncores)
    goth_d = nc.dram_tensor("goth", [128, nt_tot], i32, kind="ExternalInput")
    gslf_d = nc.dram_tensor("gslf", [128, nt_tot], i16, kind="ExternalInput")
    cbase_d = nc.dram_tensor("cbase", [128, 1], i32, kind="ExternalInput")
    locv_d = nc.dram_tensor("locv", [128, nt_tot], i8, kind="ExternalInput")
    xt_d = nc.dram_tensor("xt", [4, slots], f16, kind="ExternalInput")
    hshard_d = nc.dram_tensor("hshard", [ns, D], f16, kind="ExternalInput")
    hsh_b = nc.dram_tensor("hsh_b", [ns, D], f16)
    htab_d = nc.dram_tensor("hfull", [ntot, D], f16, addr_space="Shared")
    iota_d = nc.dram_tensor("iota", [128, 128], f16, kind="ExternalInput")
    wts = {}
    for p in ("f", "t"):
        wts[p + "w0h"] = nc.dram_tensor(p + "w0h", [128, H], f16, kind="ExternalInput")
        wts[p + "w0x"] = nc.dram_tensor(p + "w0x", [4, H], f16, kind="ExternalInput")
        wts[p + "w1"] = nc.dram_tensor(p + "w1", [H, H], f16, kind="ExternalInput")
        wts[p + "w2"] = nc.dram_tensor(p + "w2", [H, D], f16, kind="ExternalInput")
        wts[p + "b0"] = nc.dram_tensor(p + "b0", [H, 1], f32, kind="ExternalInput")
        wts[p + "b1"] = nc.dram_tensor(p + "b1", [H, 1], f32, kind="ExternalInput")
        wts[p + "b2r"] = nc.dram_tensor(p + "b2r", [128, MAXBT * D], f16, kind="ExternalInput")
    delta_d = nc.dram_tensor("delta", [ns, D], i8, kind="ExternalOutput")

    # all-gather the h shards into a full replicated table before the main
    # body; runs on the gpsimd stream, which also issues the gathers later,
    # so engine program order guarantees completion before first use.
    with nc.Block() as blk, \
         nc.semaphore("ag_dma") as ag_dma, \
         nc.semaphore("ag_cc") as ag_cc:

        @blk.gpsimd
        def _(g):
            g.dma_start(out=hsh_b[:, :], in_=hshard_d[:, :]).then_inc(ag_dma, 16)
            g.wait_ge(ag_dma, 16)
            g.collective_compute(
                "AllGather",
                mybir.AluOpType.bypass,
                replica_groups=[list(range(ncores))],
                ins=[hsh_b.ap().opt()],
                outs=[htab_d.ap().opt()],
            ).then_inc(ag_cc)
            g.wait_ge(ag_cc, 1)

    Tanh = mybir.ActivationFunctionType.Tanh
    MB = MAXBT
    with tile.TileContext(nc) as tc:
        with tc.tile_pool(name="wpool", bufs=1) as wp, \
             tc.tile_pool(name="io", bufs=3) as io, \
             tc.tile_pool(name="act", bufs=2) as ap_, \
             tc.tile_pool(name="ps01", bufs=1, space="PSUM") as ps01, \
             tc.tile_pool(name="psd", bufs=2, space="PSUM") as psdp, \
             tc.tile_pool(name="win", bufs=2, space="PSUM") as winp:
            wt = {}
            for k, dr in wts.items():
                tl = wp.tile(list(dr.shape), dr.dtype, tag="w" + k)
                nc.sync.dma_start(out=tl[:], in_=dr[:])
                wt[k] = tl
            iota = wp.tile([128, 128], f16, tag="iota")
            nc.sync.dma_start(out=iota[:], in_=iota_d[:])
            cbase = wp.tile([128, 1], i32, tag="cbase")
            nc.sync.dma_start(out=cbase[:], in_=cbase_d[:])

            for w in range(nwin):
                win = winp.tile([128, D], f32, tag="win")
                for pi, p in enumerate(("f", "t")):
                    tbase = (w * 2 + pi) * ktiles
                    off = 0
                    for bi, bt in enumerate(bts):
                        t0 = tbase + off
                        s0 = t0 * 128
                        nsl = bt * 128
                        off += bt
                        go = io.tile([128, MB], i32, tag="go")
                        gs16 = io.tile([128, MB], i16, tag="gs16")
                        lo8 = io.tile([128, MB], i8, tag="lo8")
                        xb = io.tile([4, MB * 128], f16, tag="xb")
                        nc.sync.dma_start(out=go[:, :bt],
                                          in_=goth_d[:, t0:t0 + bt])
                        nc.sync.dma_start(out=gs16[:, :bt],
                                          in_=gslf_d[:, t0:t0 + bt])
                        nc.sync.dma_start(out=lo8[:, :bt],
                                          in_=locv_d[:, t0:t0 + bt])
                        nc.sync.dma_start(out=xb[:, :nsl],
                                          in_=xt_d[:, s0:s0 + nsl])
                        lo = io.tile([128, MB], f16, tag="lo")
                        nc.vector.tensor_copy(out=lo[:, :bt], in_=lo8[:, :bt])
                        # self node id = core base + local node id (int16)
                        gs32 = io.tile([128, MB], i32, tag="gs32")
                        nc.vector.tensor_copy(out=gs32[:, :bt],
                                              in_=gs16[:, :bt])
                        gsf = io.tile([128, MB], i32, tag="gsf")
                        nc.vector.tensor_tensor(
                            out=gsf[:, :bt], in0=gs32[:, :bt],
                            in1=cbase[:, 0:1].to_broadcast([128, bt]),
                            op=mybir.AluOpType.add)
                        gp = io.tile([128, 2 * MB, D], f16, tag="gp")
                        # HW DGE handles one offset per partition per
                        # instruction; fan out over the columns. For the
                        # f population the self index is addr_from, for
                        # the t population it is addr_to.
                        for t in range(bt):
                            from_ap = gsf[:, t:t + 1] if pi == 0 \
                                else go[:, t:t + 1]
                            to_ap = go[:, t:t + 1] if pi == 0 \
                                else gsf[:, t:t + 1]
                            nc.gpsimd.indirect_dma_start(
                                out=gp[:, 2 * t, :], out_offset=None,
                                in_=htab_d[:],
                                in_offset=bass.IndirectOffsetOnAxis(
                                    ap=from_ap, axis=0))
                            nc.gpsimd.indirect_dma_start(
                                out=gp[:, 2 * t + 1, :], out_offset=None,
                                in_=htab_d[:],
                                in_offset=bass.IndirectOffsetOnAxis(
                                    ap=to_ap, axis=0))
                        rhs = ap_.tile([128, MB * 128], f16, tag="rhs")
                        for t in range(bt):
                            nc.sync.dma_start_transpose(
                                out=rhs[:, t * 128:(t + 1) * 128],
                                in_=gp[:, 2 * t:2 * t + 2, :])
                        ps0 = ps01.tile([128, MB * 128], f32, tag="ps0")
                        nc.tensor.matmul(out=ps0[:, :nsl], lhsT=wt[p + "w0h"][:],
                                         rhs=rhs[:, :nsl], start=True, stop=False)
                        nc.tensor.matmul(out=ps0[:, :nsl], lhsT=wt[p + "w0x"][:],
                                         rhs=xb[:, :nsl], start=False, stop=True)
                        h1 = ap_.tile([128, MB * 128], f16, tag="h1")
                        nc.scalar.activation(h1[:, :nsl], ps0[:, :nsl], Tanh,
                                             bias=wt[p + "b0"][:, 0:1])
                        ps1 = ps01.tile([128, MB * 128], f32, tag="ps1")
                        nc.tensor.matmul(out=ps1[:, :nsl], lhsT=wt[p + "w1"][:],
                                         rhs=h1[:, :nsl], start=True, stop=True)
                        h2 = ap_.tile([128, MB * 128], f16, tag="h2")
                        nc.scalar.activation(h2[:, :nsl], ps1[:, :nsl], Tanh,
                                             bias=wt[p + "b1"][:, 0:1])
                        psd = psdp.tile([128, MB * D], f32, tag="psd")
                        for t in range(bt):
                            nc.tensor.matmul(out=psd[:, t * D:(t + 1) * D],
                                             lhsT=h2[:, t * 128:(t + 1) * 128],
                                             rhs=wt[p + "w2"][:],
                                             start=True, stop=True)
                        dsb = ap_.tile([128, MB * D], f16, tag="dsb")
                        nc.vector.tensor_tensor(out=dsb[:, :bt * D],
                                                in0=psd[:, :bt * D],
                                                in1=wt[p + "b2r"][:, :bt * D],
                                                op=mybir.AluOpType.add)
                        dtl = ap_.tile([128, MB * D], f16, tag="dtl")
                        nc.scalar.activation(dtl[:, :bt * D], dsb[:, :bt * D],
                                             Tanh)
                        oh = ap_.tile([128, MB, 128], f16, tag="oh")
                        for t in range(bt):
                            nc.vector.tensor_tensor(
                                out=oh[:, t, :],
                                in0=lo[:, t:t + 1].to_broadcast([128, 128]),
                                in1=iota[:],
                                op=mybir.AluOpType.is_equal)
                        for t in range(bt):
                            nc.tensor.matmul(
                                out=win[:],
                                lhsT=oh[:, t, :],
                                rhs=dtl[:, t * D:(t + 1) * D],
                                start=(pi == 0 and bi == 0 and t == 0),
                                stop=(pi == 1 and bi == len(bts) - 1
                                      and t == bt - 1))
                rows = lastrows if w == nwin - 1 else 128
                wout = ap_.tile([128, D], f16, tag="wout")
                nc.scalar.activation(wout[:], win[:], Tanh)
                # int8 quantization: HW converts round-to-nearest-even,
                # max error 1/254 on values in [-1, 1]
                wq = ap_.tile([128, D], i8, tag="wq")
                nc.vector.tensor_scalar(out=wq[:], in0=wout[:], scalar1=127.0,
                                        scalar2=None,
                                        op0=mybir.AluOpType.mult)
                nc.sync.dma_start(out=delta_d[w * 128:w * 128 + rows, :],
                                  in_=wq[0:rows, :])

    # this walrus rejects any compute instruction carrying >1 sem wait;
    # hoist extra waits onto same-engine nops placed just before it.
    if not walrus_fix:
        return nc
    ctr = 0
    for bb in nc.main_func.blocks:
        new = []
        for ins in bb.instructions:
            si = getattr(ins, "sync_info", None)
            if si is not None and si.on_wait and len(si.on_wait) > 1:
                waits = list(si.on_wait)
                si.on_wait = [waits[-1]]
                for wv in waits[:-1]:
                    ctr += 1
                    nop = mybir.InstNoOp(
                        name=f"wsplit-{ctr}", engine=ins.engine, ins=[], outs=[],
                        sync_info=mybir.SyncInfo(on_wait=[wv], on_update=[]))
                    new.append(nop)
            new.append(ins)
        bb.instructions[:] = new
    return nc


def _get_fill_nb():
    if "fill_nb" in _BASS_CACHE:
        return _BASS_CACHE["fill_nb"]
    try:
        import numba
    except Exception:
        _BASS_CACHE["fill_nb"] = None
        return None

    @numba.njit(cache=True)
    def _fill(af32, at32, xu16, goflat, gsflat, lflat, xflat, counters,
              spill_e, spill_pi, ns, nwin, cap, nt_tot, slots):
        ne = af32.size
        nsp = 0
        for e in range(ne):
            for pi in range(2):
                dest = af32[e] if pi == 0 else at32[e]
                other = at32[e] if pi == 0 else af32[e]
                c = dest // ns
                nl = dest - c * ns
                w = nl >> 7
                loc = nl & 127
                idx = (c * nwin + w) * 2 + pi
                r = counters[idx]
                counters[idx] = r + 1
                if r >= cap:
                    spill_e[nsp] = e
                    spill_pi[nsp] = pi
                    nsp += 1
                else:
                    sc = w * (2 * cap) + pi * cap + r
                    tt = sc >> 7
                    pp = sc & 127
                    tbase = (c * 128 + pp) * nt_tot + tt
                    goflat[tbase] = other
                    gsflat[tbase] = nl
                    lflat[tbase] = loc
                    xb = (c * 4) * slots + sc
                    for rr in range(4):
                        xflat[xb + rr * slots] = xu16[e, rr]
        return nsp

    _BASS_CACHE["fill_nb"] = _fill
    return _fill


def _prep_slots(af, at, x_local, ncores, ns, nwin, ktiles):
    """Build per-core padded slot arrays in tile-major layout.
    gidx [ncores, 128, 2*NT] i32 (from/to pairs per tile column),
    locv [ncores, 128, NT] i8 (in-window node offset, -1 = pad),
    xt   [ncores, 4, SLOTS] f16 (x features, slot-major),
    spills = list of (pop, edge_indices) that overflowed window capacity."""
    fill = _get_fill_nb()
    if fill is not None:
        cap = ktiles * 128
        nt_tot = nwin * 2 * ktiles
        slots = nt_tot * 128
        af32 = np.ascontiguousarray(af.astype(np.int32))
        at32 = np.ascontiguousarray(at.astype(np.int32))
        xu16 = np.ascontiguousarray(x_local.astype(np.float16)).view(np.uint16)
        goth = np.zeros((ncores, 128, nt_tot), np.int32)
        gslf = np.zeros((ncores, 128, nt_tot), np.int16)
        locv = np.full((ncores, 128, nt_tot), -1, np.int8)
        xt16 = np.zeros((ncores, 4, slots), np.uint16)
        counters = np.zeros(ncores * nwin * 2, np.int32)
        spill_e = np.empty(af32.size * 2, np.int64)
        spill_pi = np.empty(af32.size * 2, np.int8)
        nsp = fill(af32, at32, xu16, goth.reshape(-1), gslf.reshape(-1),
                   locv.reshape(-1), xt16.reshape(-1), counters, spill_e,
                   spill_pi, ns, nwin, cap, nt_tot, slots)
        spills = []
        for pi in (0, 1):
            sel = spill_e[:nsp][spill_pi[:nsp] == pi]
            if len(sel):
                spills.append((pi, sel))
        return goth, gslf, locv, xt16.view(np.float16), spills
    return _prep_slots_np(af, at, x_local, ncores, ns, nwin, ktiles)


def _prep_slots_np(af, at, x_local, ncores, ns, nwin, ktiles):
    cap = ktiles * 128
    nt_tot = nwin * 2 * ktiles
    slots = nt_tot * 128
    af32 = af.astype(np.int32)
    at32 = at.astype(np.int32)
    goth = np.zeros((ncores, 128, nt_tot), np.int32)
    gslf = np.zeros((ncores, 128, nt_tot), np.int16)
    locv = np.full((ncores, 128, nt_tot), -1, np.int8)
    xt = np.zeros((ncores, 4, slots), np.float16)
    xtv = np.ascontiguousarray(x_local.astype(np.float16))
    goflat = goth.reshape(-1)
    gsflat = gslf.reshape(-1)
    lflat = locv.reshape(-1)
    xflat = xt.reshape(-1)
    spills = []
    for pi, dest in enumerate((af32, at32)):
        other = at32 if pi == 0 else af32
        core = dest // np.int32(ns)
        node_l = dest - core * np.int32(ns)
        w = node_l >> 7
        loc = (node_l & 127).astype(np.int8)
        cw = core * np.int32(nwin) + w
        order = np.argsort(cw, kind="stable").astype(np.int32)
        counts = np.bincount(cw, minlength=ncores * nwin)
        starts = np.concatenate([[0], np.cumsum(counts)[:-1]])
        rank = (np.arange(len(cw), dtype=np.int32)
                - np.repeat(starts, counts).astype(np.int32))
        ok = rank < cap
        if not ok.all():
            spills.append((pi, order[~ok].astype(np.int64)))
            e_ok = order[ok]
            r = rank[ok]
        else:
            e_ok = order
            r = rank
        # within-core slot: window block of 2*cap, population block of cap
        sc = w[e_ok] * np.int32(2 * cap) + np.int32(pi * cap) + r
        T = sc >> 7
        pp = sc & 127
        c_ok = core[e_ok]
        tbase = (c_ok * 128 + pp) * nt_tot + T
        goflat[tbase] = other[e_ok]
        gsflat[tbase] = node_l[e_ok].astype(np.int16)
        lflat[tbase] = loc[e_ok]
        xbase = (c_ok * 4) * slots + sc
        xflat[xbase[:, None] + (np.arange(4, dtype=np.int32) * slots)[None, :]] \
            = xtv[e_ok]
    return goth, gslf, locv, xt, spills


def _prep_weights(inputs):
    const = np.concatenate([np.asarray(inputs["h_global"], np.float32).ravel(),
                            np.asarray(inputs["x_global"], np.float32).ravel(),
                            np.asarray(inputs["t"], np.float32).ravel()])
    wm = {}
    for p in ("f", "t"):
        W0 = np.asarray(inputs[p + "_W0"], np.float32)
        b0 = np.asarray(inputs[p + "_b0"], np.float32)
        b0eff = b0 + const @ W0[132:153]
        wm[p + "w0h"] = np.ascontiguousarray(W0[0:128]).astype(np.float16)
        wm[p + "w0x"] = np.ascontiguousarray(W0[128:132]).astype(np.float16)
        wm[p + "w1"] = np.asarray(inputs[p + "_W1"], np.float32).astype(np.float16)
        wm[p + "w2"] = np.asarray(inputs[p + "_W2"], np.float32).astype(np.float16)
        wm[p + "b0"] = b0eff.reshape(H, 1).astype(np.float32)
        wm[p + "b1"] = np.asarray(inputs[p + "_b1"], np.float32).reshape(H, 1)
        b2 = np.asarray(inputs[p + "_b2"], np.float32)
        wm[p + "b2r"] = np.tile(b2.reshape(1, D), (128, MAXBT)).astype(np.float16)
    wm["iota"] = np.broadcast_to(np.arange(128, dtype=np.float16), (128, 128)).copy()
    return wm


def _fix_spill_nodes(spills, inputs, out):
    """Recompute on host (fp32) every node whose window overflowed device
    capacity; overwrite those rows of `out`. Empty for uniform edge data."""
    if not spills:
        return
    af = np.asarray(inputs["addr_from"]).astype(np.int64)
    at = np.asarray(inputs["addr_to"]).astype(np.int64)
    h = np.asarray(inputs["h_local"], np.float32)
    x = np.asarray(inputs["x_local"], np.float32)
    const = np.concatenate([np.asarray(inputs["h_global"], np.float32).ravel(),
                            np.asarray(inputs["x_global"], np.float32).ravel(),
                            np.asarray(inputs["t"], np.float32).ravel()])
    nodes = np.unique(np.concatenate(
        [(af if pi == 0 else at)[e] for pi, e in spills]))
    node_set = np.zeros(h.shape[0], bool)
    node_set[nodes] = True
    delta = np.zeros((len(nodes), D), np.float32)
    remap = np.full(h.shape[0], -1, np.int64)
    remap[nodes] = np.arange(len(nodes))
    for pi, idx_all in ((0, af), (1, at)):
        p = "f" if pi == 0 else "t"
        edges = np.flatnonzero(node_set[idx_all])
        if not len(edges):
            continue
        inp = np.concatenate([h[af[edges]], h[at[edges]], x[edges],
                              np.broadcast_to(const, (len(edges), 21))], axis=1)
        d = np.tanh(np.tanh(np.tanh(
            inp @ inputs[p + "_W0"] + inputs[p + "_b0"]) @ inputs[p + "_W1"]
            + inputs[p + "_b1"]) @ inputs[p + "_W2"] + inputs[p + "_b2"])
        _scatter_add(delta, remap[idx_all[edges]], d.astype(np.float32))
    out[nodes] = np.tanh(delta)


def _get_exec(nc):
    """Build (once) a cached jitted executor for the bass module: the same
    _bass_exec_p/shard_map lowering run_bass_kernel_spmd uses under axon,
    but with the jitted callable memoized so repeat calls skip retracing."""
    if "exec" in _BASS_CACHE:
        return _BASS_CACHE["exec"]
    import jax
    import concourse.mybir as mybir
    from jax.sharding import Mesh, PartitionSpec
    from jax.experimental.shard_map import shard_map
    from concourse.bass2jax import (_bass_exec_p, install_neuronx_cc_hook,
                                    partition_id_tensor)
    install_neuronx_cc_hook()

    in_names, out_names, out_avals = [], [], []
    pname = nc.partition_id_tensor.name if nc.partition_id_tensor else None
    for alloc in nc.m.functions[0].allocations:
        if not isinstance(alloc, mybir.MemoryLocationSet):
            continue
        name = alloc.memorylocations[0].name
        if alloc.kind == "ExternalInput":
            if name != pname:
                in_names.append(name)
        elif alloc.kind == "ExternalOutput":
            out_names.append(name)
            out_avals.append(jax.core.ShapedArray(
                tuple(alloc.tensor_shape), mybir.dt.np(alloc.dtype)))
    n_params = len(in_names)
    n_outs = len(out_avals)
    all_names = in_names + out_names + ([pname] if pname else [])

    def _body(*args):
        ops = list(args)
        if pname:
            ops.append(partition_id_tensor())
        outs = _bass_exec_p.bind(
            *ops, out_avals=tuple(out_avals), in_names=tuple(all_names),
            out_names=tuple(out_names), lowering_input_output_aliases=(),
            sim_require_finite=True, sim_require_nnan=True, nc=nc)
        return tuple(outs)

    devices = jax.devices()[:NCORES]
    mesh = Mesh(np.asarray(devices), ("core",))
    in_specs = (PartitionSpec("core"),) * (n_params + n_outs)
    out_specs = (PartitionSpec("core"),) * n_outs
    donate = tuple(range(n_params, n_params + n_outs))
    sharded = jax.jit(
        shard_map(_body, mesh=mesh, in_specs=in_specs, out_specs=out_specs,
                  check_rep=False),
        donate_argnums=donate, keep_unused=True)

    from jax.sharding import NamedSharding
    sharding = NamedSharding(mesh, PartitionSpec("core"))
    import jax.numpy as jnp

    # donated output buffers made on-device (zeros never cross the tunnel)
    def _mk_zeros():
        return tuple(jnp.zeros((NCORES * a.shape[0], *a.shape[1:]), a.dtype)
                     for a in out_avals)
    zeros_fn = jax.jit(_mk_zeros, out_shardings=(sharding,) * n_outs)

    ex = (sharded, in_names, out_names, out_avals, sharding, zeros_fn)
    _BASS_CACHE["exec"] = ex
    return ex


def _kernel_bass(addr_from, addr_to, h_local, h_global, x_local, x_global, t,
                 f_W0, f_b0, f_W1, f_b1, f_W2, f_b2,
                 t_W0, t_b0, t_W1, t_b1, t_W2, t_b2, trace=False):
    import sys
    if "/opt/trn_rl_repo" not in sys.path:
        sys.path.insert(0, "/opt/trn_rl_repo")

    inputs = dict(addr_from=addr_from, addr_to=addr_to, h_local=h_local,
                  h_global=h_global, x_local=x_local, x_global=x_global, t=t,
                  f_W0=f_W0, f_b0=f_b0, f_W1=f_W1, f_b1=f_b1, f_W2=f_W2,
                  f_b2=f_b2, t_W0=t_W0, t_b0=t_b0, t_W1=t_W1, t_b1=t_b1,
                  t_W2=t_W2, t_b2=t_b2)
    af = np.asarray(addr_from).astype(np.int64)
    at = np.asarray(addr_to).astype(np.int64)
    h = np.asarray(h_local, np.float32)
    x = np.asarray(x_local, np.float32)

    key = (N, E)
    if key not in _BASS_CACHE:
        _BASS_CACHE[key] = _build_bass(NS, NWIN, LASTROWS, KTILES, N, NCORES)
    nc = _BASS_CACHE[key]
    sharded, in_names, out_names, out_avals, sharding, zeros_fn = _get_exec(nc)

    import jax

    # stage prep-independent inputs first: their h2d transfer overlaps the
    # host-side slot preparation below.
    staged = {}
    wm = _prep_weights(inputs)
    htab = np.ascontiguousarray(h.astype(np.float16))
    staged["hshard"] = jax.device_put(htab, sharding)
    for k, v in wm.items():
        staged[k] = jax.device_put(np.tile(v, (NCORES, 1)), sharding)
    zeros = zeros_fn()

    cbase = (np.arange(NCORES, dtype=np.int32)[:, None, None] * NS
             * np.ones((1, 128, 1), np.int32)).reshape(-1, 1)
    staged["cbase"] = jax.device_put(cbase, sharding)

    goth, gslf, locv, xt, spills = _prep_slots(af, at, x, NCORES, NS, NWIN,
                                               KTILES)
    staged["goth"] = jax.device_put(goth.reshape(-1, goth.shape[-1]), sharding)
    staged["gslf"] = jax.device_put(gslf.reshape(-1, gslf.shape[-1]), sharding)
    staged["locv"] = jax.device_put(locv.reshape(-1, locv.shape[-1]), sharding)
    staged["xt"] = jax.device_put(xt.reshape(-1, xt.shape[-1]), sharding)

    out_arrs = sharded(*[staged[n] for n in in_names], *zeros)
    out = np.asarray(out_arrs[out_names.index("delta")]).astype(np.float32)
    out *= np.float32(1.0 / 127.0)
    _fix_spill_nodes(spills, inputs, out)
    return out


def kernel(**inputs):
    try:
        return _kernel_bass(**inputs)
    except Exception:
        import traceback
        traceback.print_exc()
        return _kernel_numpy(**inputs)


# revision 35
# speedup vs baseline: 33.3573x; 1.0177x over previous
import numpy as np

# nn_LocalDynamics GNN message passing.
#   delta[n] = sum_e tanh(fMLP(inp_e))[addr_from=n] + tanh(tMLP(inp_e))[addr_to=n]
#   out = tanh(delta).  inp_e = [h[from], h[to], x_e, hg, xg, t] (153 dims).
#
# Destination-sharded design: each core owns nodes [c*12500, (c+1)*12500).
# Every edge yields two "slots": an f-slot on the core owning addr_from and a
# t-slot on the core owning addr_to.  Slots are grouped by 128-node windows of
# the owning core; each window holds a fixed KTILES tiles of 128 slots per
# population (f/t), host-padded.  On device, per batch of tiles:
#   AllGather h shards -> indirect-DMA gather of (h[from], h[to]) row pairs ->
#   XBAR transpose to feature-major -> fp16 MLP -> slot-major final layer ->
#   one-hot matmul accumulates the window's delta in PSUM -> tanh -> fp16 out.
# Host recomputes any overflowed windows (empty for uniform edges).

N = 100_000
E = 800_000
D = 64
H = 128
NCORES = 8
NS = N // NCORES            # nodes per core (12500)
NWIN = (NS + 127) // 128    # windows per core (98)
LASTROWS = NS - (NWIN - 1) * 128   # rows in last window (84)
KTILES = 10                 # 128-slot tiles per population per window
NT = NWIN * 2 * KTILES      # tiles per core (1960)
SLOTS = NT * 128            # padded slots per core (250880)
MAXBT = 4                   # max tiles per batch


def _batch_tiles(ktiles):
    out = []
    k = ktiles
    while k > 0:
        out.append(min(MAXBT, k))
        k -= out[-1]
    return out


def _scatter_add(delta, idx, vals):
    o = np.argsort(idx, kind="stable")
    si = idx[o]
    sv = vals[o]
    starts = np.flatnonzero(np.r_[True, si[1:] != si[:-1]])
    sums = np.add.reduceat(sv, starts, axis=0)
    np.add.at(delta, si[starts], sums)


def _kernel_numpy(addr_from, addr_to, h_local, h_global, x_local, x_global, t,
                  f_W0, f_b0, f_W1, f_b1, f_W2, f_b2,
                  t_W0, t_b0, t_W1, t_b1, t_W2, t_b2):
    af = np.asarray(addr_from).astype(np.int64)
    at = np.asarray(addr_to).astype(np.int64)
    h_local = np.asarray(h_local, dtype=np.float32)
    x_local = np.asarray(x_local, dtype=np.float32)
    const = np.concatenate([np.asarray(h_global, np.float32).ravel(),
                            np.asarray(x_global, np.float32).ravel(),
                            np.asarray(t, np.float32).ravel()])
    ne = af.shape[0]
    delta = np.zeros((h_local.shape[0], D), np.float32)
    CH = 100_000
    for s in range(0, ne, CH):
        e = min(s + CH, ne)
        inp = np.concatenate([h_local[af[s:e]], h_local[at[s:e]], x_local[s:e],
                              np.broadcast_to(const, (e - s, 21))], axis=1).astype(np.float32)
        d_f = np.tanh(np.tanh(np.tanh(inp @ f_W0 + f_b0) @ f_W1 + f_b1) @ f_W2 + f_b2)
        d_t = np.tanh(np.tanh(np.tanh(inp @ t_W0 + t_b0) @ t_W1 + t_b1) @ t_W2 + t_b2)
        _scatter_add(delta, af[s:e], d_f.astype(np.float32))
        _scatter_add(delta, at[s:e], d_t.astype(np.float32))
    return np.tanh(delta).astype(np.float32)


_BASS_CACHE = {}


def _build_bass(ns, nwin, lastrows, ktiles, ntot, ncores, walrus_fix=True):
    import concourse.bass as bass
    import concourse.mybir as mybir
    import concourse.tile as tile

    # walrus in this env rejects Drain instructions carrying >1 sem wait;
    # move each wait onto its own sync nop before the drain.
    def _patched(self, tick_clock, wait_clock):
        from concourse.tile import ScopedClock
        nop0 = self.nc.sync.nop(nofuse=True)
        wait_clock.add_sem_waits(nop0.ins, ScopedClock({None: tick_clock.global_clock}))
        si = nop0.ins.sync_info
        if si is not None and si.on_wait and len(si.on_wait) > 1:
            waits = list(si.on_wait)
            si.on_wait = waits[:1]
            for w in waits[1:]:
                n = self.nc.sync.nop(nofuse=True)
                n.ins.sync_info = mybir.SyncInfo(on_wait=[w], on_update=[])
        self.nc.sync.drain()
        self.nc.all_engine_barrier()
        popped = self.nc._tile_sem_poison_stack.pop()
        assert popped is self._sem_poison
        self.nc.clear_and_free_semaphores(list(self.sems.allocated().values()))
        self.nc.all_engine_barrier()

    tile.TileContext._drain_and_barrier = _patched

    f32 = mybir.dt.float32
    f16 = mybir.dt.float16
    i32 = mybir.dt.int32
    i8 = mybir.dt.int8
    nt_tot = nwin * 2 * ktiles
    slots = nt_tot * 128
    bts = _batch_tiles(ktiles)

    i16 = mybir.dt.int16
    nc = bass.Bass(num_devices=ncores)
    goth_d = nc.dram_tensor("goth", [128, nt_tot], i32, kind="ExternalInput")
    gslf_d = nc.dram_tensor("gslf", [128, nt_tot], i16, kind="ExternalInput")
    cbase_d = nc.dram_tensor("cbase", [128, 1], i32, kind="ExternalInput")
    locv_d = nc.dram_tensor("locv", [128, nt_tot], i8, kind="ExternalInput")
    xt_d = nc.dram_tensor("xt", [4, slots], f16, kind="ExternalInput")
    hshard_d = nc.dram_tensor("hshard", [ns, D], f16, kind="ExternalInput")
    hsh_b = nc.dram_tensor("hsh_b", [ns, D], f16)
    htab_d = nc.dram_tensor("hfull", [ntot, D], f16, addr_space="Shared")
    iota_d = nc.dram_tensor("iota", [128, 128], f16, kind="ExternalInput")
    wts = {}
    for p in ("f", "t"):
        wts[p + "w0h"] = nc.dram_tensor(p + "w0h", [128, H], f16, kind="ExternalInput")
        wts[p + "w0x"] = nc.dram_tensor(p + "w0x", [4, H], f16, kind="ExternalInput")
        wts[p + "w1"] = nc.dram_tensor(p + "w1", [H, H], f16, kind="ExternalInput")
        wts[p + "w2"] = nc.dram_tensor(p + "w2", [H, D], f16, kind="ExternalInput")
        wts[p + "b0"] = nc.dram_tensor(p + "b0", [H, 1], f32, kind="ExternalInput")
        wts[p + "b1"] = nc.dram_tensor(p + "b1", [H, 1], f32, kind="ExternalInput")
        wts[p + "b2r"] = nc.dram_tensor(p + "b2r", [128, MAXBT * D], f16, kind="ExternalInput")
    delta_d = nc.dram_tensor("delta", [ns, D], i8, kind="ExternalOutput")

    # all-gather the h shards into a full replicated table before the main
    # body; runs on the gpsimd stream, which also issues the gathers later,
    # so engine program order guarantees completion before first use.
    with nc.Block() as blk, \
         nc.semaphore("ag_dma") as ag_dma, \
         nc.semaphore("ag_cc") as ag_cc:

        @blk.gpsimd
        def _(g):
            g.dma_start(out=hsh_b[:, :], in_=hshard_d[:, :]).then_inc(ag_dma, 16)
            g.wait_ge(ag_dma, 16)
            g.collective_compute(
                "AllGather",
                mybir.AluOpType.bypass,
                replica_groups=[list(range(ncores))],
                ins=[hsh_b.ap().opt()],
                outs=[htab_d.ap().opt()],
            ).then_inc(ag_cc)
            g.wait_ge(ag_cc, 1)

    Tanh = mybir.ActivationFunctionType.Tanh
    MB = MAXBT
    with tile.TileContext(nc) as tc:
        with tc.tile_pool(name="wpool", bufs=1) as wp, \
             tc.tile_pool(name="io", bufs=3) as io, \
             tc.tile_pool(name="act", bufs=2) as ap_, \
             tc.tile_pool(name="ps01", bufs=1, space="PSUM") as ps01, \
             tc.tile_pool(name="psd", bufs=2, space="PSUM") as psdp, \
             tc.tile_pool(name="win", bufs=2, space="PSUM") as winp:
            wt = {}
            for k, dr in wts.items():
                tl = wp.tile(list(dr.shape), dr.dtype, tag="w" + k)
                nc.sync.dma_start(out=tl[:], in_=dr[:])
                wt[k] = tl
            iota = wp.tile([128, 128], f16, tag="iota")
            nc.sync.dma_start(out=iota[:], in_=iota_d[:])
            cbase = wp.tile([128, 1], i32, tag="cbase")
            nc.sync.dma_start(out=cbase[:], in_=cbase_d[:])

            for w in range(nwin):
                win = winp.tile([128, D], f32, tag="win")
                for pi, p in enumerate(("f", "t")):
                    tbase = (w * 2 + pi) * ktiles
                    off = 0
                    for bi, bt in enumerate(bts):
                        t0 = tbase + off
                        s0 = t0 * 128
                        nsl = bt * 128
                        off += bt
                        go = io.tile([128, MB], i32, tag="go")
                        gs16 = io.tile([128, MB], i16, tag="gs16")
                        lo8 = io.tile([128, MB], i8, tag="lo8")
                        xb = io.tile([4, MB * 128], f16, tag="xb")
                        nc.sync.dma_start(out=go[:, :bt],
                                          in_=goth_d[:, t0:t0 + bt])
                        nc.sync.dma_start(out=gs16[:, :bt],
                                          in_=gslf_d[:, t0:t0 + bt])
                        nc.sync.dma_start(out=lo8[:, :bt],
                                          in_=locv_d[:, t0:t0 + bt])
                        nc.sync.dma_start(out=xb[:, :nsl],
                                          in_=xt_d[:, s0:s0 + nsl])
                        lo = io.tile([128, MB], f16, tag="lo")
                        nc.vector.tensor_copy(out=lo[:, :bt], in_=lo8[:, :bt])
                        # self node id = core base + local node id (int16)
                        gs32 = io.tile([128, MB], i32, tag="gs32")
                        nc.vector.tensor_copy(out=gs32[:, :bt],
                                              in_=gs16[:, :bt])
                        gsf = io.tile([128, MB], i32, tag="gsf")
                        nc.vector.tensor_tensor(
                            out=gsf[:, :bt], in0=gs32[:, :bt],
                            in1=cbase[:, 0:1].to_broadcast([128, bt]),
                            op=mybir.AluOpType.add)
                        gp = io.tile([128, 2 * MB, D], f16, tag="gp")
                        # HW DGE handles one offset per partition per
                        # instruction; fan out over the columns. For the
                        # f population the self index is addr_from, for
                        # the t population it is addr_to.
                        for t in range(bt):
                            from_ap = gsf[:, t:t + 1] if pi == 0 \
                                else go[:, t:t + 1]
                            to_ap = go[:, t:t + 1] if pi == 0 \
                                else gsf[:, t:t + 1]
                            nc.gpsimd.indirect_dma_start(
                                out=gp[:, 2 * t, :], out_offset=None,
                                in_=htab_d[:],
                                in_offset=bass.IndirectOffsetOnAxis(
                                    ap=from_ap, axis=0))
                            nc.gpsimd.indirect_dma_start(
                                out=gp[:, 2 * t + 1, :], out_offset=None,
                                in_=htab_d[:],
                                in_offset=bass.IndirectOffsetOnAxis(
                                    ap=to_ap, axis=0))
                        rhs = ap_.tile([128, MB * 128], f16, tag="rhs")
                        for t in range(bt):
                            nc.sync.dma_start_transpose(
                                out=rhs[:, t * 128:(t + 1) * 128],
                                in_=gp[:, 2 * t:2 * t + 2, :])
                        ps0 = ps01.tile([128, MB * 128], f32, tag="ps0")
                        nc.tensor.matmul(out=ps0[:, :nsl], lhsT=wt[p + "w0h"][:],
                                         rhs=rhs[:, :nsl], start=True, stop=False)
                        nc.tensor.matmul(out=ps0[:, :nsl], lhsT=wt[p + "w0x"][:],
                                         rhs=xb[:, :nsl], start=False, stop=True)
                        h1 = ap_.tile([128, MB * 128], f16, tag="h1")
                        nc.scalar.activation(h1[:, :nsl], ps0[:, :nsl], Tanh,
                                             bias=wt[p + "b0"][:, 0:1])
                        ps1 = ps01.tile([128, MB * 128], f32, tag="ps1")
                        nc.tensor.matmul(out=ps1[:, :nsl], lhsT=wt[p + "w1"][:],
                                         rhs=h1[:, :nsl], start=True, stop=True)
                        h2 = ap_.tile([128, MB * 128], f16, tag="h2")
                        nc.scalar.activation(h2[:, :nsl], ps1[:, :nsl], Tanh,
                                             bias=wt[p + "b1"][:, 0:1])
                        psd = psdp.tile([128, MB * D], f32, tag="psd")
                        for t in range(bt):
                            nc.tensor.matmul(out=psd[:, t * D:(t + 1) * D],
                                             lhsT=h2[:, t * 128:(t + 1) * 128],
                                             rhs=wt[p + "w2"][:],
                                             start=True, stop=True)
                        dsb = ap_.tile([128, MB * D], f16, tag="dsb")
                        nc.vector.tensor_tensor(out=dsb[:, :bt * D],
                                                in0=psd[:, :bt * D],
                                                in1=wt[p + "b2r"][:, :bt * D],
                                                op=mybir.AluOpType.add)
                        dtl = ap_.tile([128, MB * D], f16, tag="dtl")
                        nc.scalar.activation(dtl[:, :bt * D], dsb[:, :bt * D],
                                             Tanh)
                        oh = ap_.tile([128, MB, 128], f16, tag="oh")
                        for t in range(bt):
                            nc.vector.tensor_tensor(
                                out=oh[:, t, :],
                                in0=lo[:, t:t + 1].to_broadcast([128, 128]),
                                in1=iota[:],
                                op=mybir.AluOpType.is_equal)
                        for t in range(bt):
                            nc.tensor.matmul(
                                out=win[:],
                                lhsT=oh[:, t, :],
                                rhs=dtl[:, t * D:(t + 1) * D],
                                start=(pi == 0 and bi == 0 and t == 0),
                                stop=(pi == 1 and bi == len(bts) - 1
                                      and t == bt - 1))
                rows = lastrows if w == nwin - 1 else 128
                wout = ap_.tile([128, D], f16, tag="wout")
                nc.scalar.activation(wout[:], win[:], Tanh)
                # int8 quantization: HW converts round-to-nearest-even,
                # max error 1/254 on values in [-1, 1]
                wq = ap_.tile([128, D], i8, tag="wq")
                nc.vector.tensor_scalar(out=wq[:], in0=wout[:], scalar1=127.0,
                                        scalar2=None,
                                        op0=mybir.AluOpType.mult)
                nc.sync.dma_start(out=delta_d[w * 128:w * 128 + rows, :],
                                  in_=wq[0:rows, :])

    # this walrus rejects any compute instruction carrying >1 sem wait;
    # hoist extra waits onto same-engine nops placed just before it.
    if not walrus_fix:
        return nc
    ctr = 0
    for bb in nc.main_func.blocks:
        new = []
        for ins in bb.instructions:
            si = getattr(ins, "sync_info", None)
            if si is not None and si.on_wait and len(si.on_wait) > 1:
                waits = list(si.on_wait)
                si.on_wait = [waits[-1]]
                for wv in waits[:-1]:
                    ctr += 1
                    nop = mybir.InstNoOp(
                        name=f"wsplit-{ctr}", engine=ins.engine, ins=[], outs=[],
                        sync_info=mybir.SyncInfo(on_wait=[wv], on_update=[]))
                    new.append(nop)
            new.append(ins)
        bb.instructions[:] = new
    return nc


def _get_fill_nb():
    if "fill_nb" in _BASS_CACHE:
        return _BASS_CACHE["fill_nb"]
    try:
        import numba
    except Exception:
        _BASS_CACHE["fill_nb"] = None
        return None

    @numba.njit(cache=True)
    def _fill(af32, at32, xu16, goflat, gsflat, lflat, xflat, counters,
              spill_e, spill_pi, ns, nwin, cap, nt_tot, slots):
        ne = af32.size
        nsp = 0
        for e in range(ne):
            for pi in range(2):
                dest = af32[e] if pi == 0 else at32[e]
                other = at32[e] if pi == 0 else af32[e]
                c = dest // ns
                nl = dest - c * ns
                w = nl >> 7
                loc = nl & 127
                idx = (c * nwin + w) * 2 + pi
                r = counters[idx]
                counters[idx] = r + 1
                if r >= cap:
                    spill_e[nsp] = e
                    spill_pi[nsp] = pi
                    nsp += 1
                else:
                    sc = w * (2 * cap) + pi * cap + r
                    tt = sc >> 7
                    pp = sc & 127
                    tbase = (c * 128 + pp) * nt_tot + tt
                    goflat[tbase] = other
                    gsflat[tbase] = nl
                    lflat[tbase] = loc
                    xb = (c * 4) * slots + sc
                    for rr in range(4):
                        xflat[xb + rr * slots] = xu16[e, rr]
        return nsp

    _BASS_CACHE["fill_nb"] = _fill
    return _fill


def _prep_slots(af, at, x_local, ncores, ns, nwin, ktiles):
    """Build per-core padded slot arrays in tile-major layout.
    gidx [ncores, 128, 2*NT] i32 (from/to pairs per tile column),
    locv [ncores, 128, NT] i8 (in-window node offset, -1 = pad),
    xt   [ncores, 4, SLOTS] f16 (x features, slot-major),
    spills = list of (pop, edge_indices) that overflowed window capacity."""
    fill = _get_fill_nb()
    if fill is not None:
        cap = ktiles * 128
        nt_tot = nwin * 2 * ktiles
        slots = nt_tot * 128
        af32 = np.ascontiguousarray(af.astype(np.int32))
        at32 = np.ascontiguousarray(at.astype(np.int32))
        xu16 = np.ascontiguousarray(x_local.astype(np.float16)).view(np.uint16)
        bufs = _BASS_CACHE.get("prep_bufs")
        if bufs is None or bufs[0].shape[0] != ncores or \
                bufs[0].shape[2] != nt_tot:
            bufs = (np.zeros((ncores, 128, nt_tot), np.int32),
                    np.zeros((ncores, 128, nt_tot), np.int16),
                    np.empty((ncores, 128, nt_tot), np.int8),
                    np.zeros((ncores, 4, slots), np.uint16),
                    np.empty(ncores * nwin * 2, np.int32),
                    np.empty(af32.size * 2, np.int64),
                    np.empty(af32.size * 2, np.int8))
            _BASS_CACHE["prep_bufs"] = bufs
        goth, gslf, locv, xt16, counters, spill_e, spill_pi = bufs
        # stale pad entries in goth/gslf/xt are harmless (their one-hot row
        # is zero and gathered rows stay in-bounds); only locv steers the
        # scatter and the counters must reset.
        locv.fill(-1)
        counters.fill(0)
        nsp = fill(af32, at32, xu16, goth.reshape(-1), gslf.reshape(-1),
                   locv.reshape(-1), xt16.reshape(-1), counters, spill_e,
                   spill_pi, ns, nwin, cap, nt_tot, slots)
        spills = []
        for pi in (0, 1):
            sel = spill_e[:nsp][spill_pi[:nsp] == pi]
            if len(sel):
                spills.append((pi, sel))
        return goth, gslf, locv, xt16.view(np.float16), spills
    return _prep_slots_np(af, at, x_local, ncores, ns, nwin, ktiles)


def _prep_slots_np(af, at, x_local, ncores, ns, nwin, ktiles):
    cap = ktiles * 128
    nt_tot = nwin * 2 * ktiles
    slots = nt_tot * 128
    af32 = af.astype(np.int32)
    at32 = at.astype(np.int32)
    goth = np.zeros((ncores, 128, nt_tot), np.int32)
    gslf = np.zeros((ncores, 128, nt_tot), np.int16)
    locv = np.full((ncores, 128, nt_tot), -1, np.int8)
    xt = np.zeros((ncores, 4, slots), np.float16)
    xtv = np.ascontiguousarray(x_local.astype(np.float16))
    goflat = goth.reshape(-1)
    gsflat = gslf.reshape(-1)
    lflat = locv.reshape(-1)
    xflat = xt.reshape(-1)
    spills = []
    for pi, dest in enumerate((af32, at32)):
        other = at32 if pi == 0 else af32
        core = dest // np.int32(ns)
        node_l = dest - core * np.int32(ns)
        w = node_l >> 7
        loc = (node_l & 127).astype(np.int8)
        cw = core * np.int32(nwin) + w
        order = np.argsort(cw, kind="stable").astype(np.int32)
        counts = np.bincount(cw, minlength=ncores * nwin)
        starts = np.concatenate([[0], np.cumsum(counts)[:-1]])
        rank = (np.arange(len(cw), dtype=np.int32)
                - np.repeat(starts, counts).astype(np.int32))
        ok = rank < cap
        if not ok.all():
            spills.append((pi, order[~ok].astype(np.int64)))
            e_ok = order[ok]
            r = rank[ok]
        else:
            e_ok = order
            r = rank
        # within-core slot: window block of 2*cap, population block of cap
        sc = w[e_ok] * np.int32(2 * cap) + np.int32(pi * cap) + r
        T = sc >> 7
        pp = sc & 127
        c_ok = core[e_ok]
        tbase = (c_ok * 128 + pp) * nt_tot + T
        goflat[tbase] = other[e_ok]
        gsflat[tbase] = node_l[e_ok].astype(np.int16)
        lflat[tbase] = loc[e_ok]
        xbase = (c_ok * 4) * slots + sc
        xflat[xbase[:, None] + (np.arange(4, dtype=np.int32) * slots)[None, :]] \
            = xtv[e_ok]
    return goth, gslf, locv, xt, spills


def _prep_weights(inputs):
    const = np.concatenate([np.asarray(inputs["h_global"], np.float32).ravel(),
                            np.asarray(inputs["x_global"], np.float32).ravel(),
                            np.asarray(inputs["t"], np.float32).ravel()])
    wm = {}
    for p in ("f", "t"):
        W0 = np.asarray(inputs[p + "_W0"], np.float32)
        b0 = np.asarray(inputs[p + "_b0"], np.float32)
        b0eff = b0 + const @ W0[132:153]
        wm[p + "w0h"] = np.ascontiguousarray(W0[0:128]).astype(np.float16)
        wm[p + "w0x"] = np.ascontiguousarray(W0[128:132]).astype(np.float16)
        wm[p + "w1"] = np.asarray(inputs[p + "_W1"], np.float32).astype(np.float16)
        wm[p + "w2"] = np.asarray(inputs[p + "_W2"], np.float32).astype(np.float16)
        wm[p + "b0"] = b0eff.reshape(H, 1).astype(np.float32)
        wm[p + "b1"] = np.asarray(inputs[p + "_b1"], np.float32).reshape(H, 1)
        b2 = np.asarray(inputs[p + "_b2"], np.float32)
        wm[p + "b2r"] = np.tile(b2.reshape(1, D), (128, MAXBT)).astype(np.float16)
    wm["iota"] = np.broadcast_to(np.arange(128, dtype=np.float16), (128, 128)).copy()
    return wm


def _fix_spill_nodes(spills, inputs, out):
    """Recompute on host (fp32) every node whose window overflowed device
    capacity; overwrite those rows of `out`. Empty for uniform edge data."""
    if not spills:
        return
    af = np.asarray(inputs["addr_from"]).astype(np.int64)
    at = np.asarray(inputs["addr_to"]).astype(np.int64)
    h = np.asarray(inputs["h_local"], np.float32)
    x = np.asarray(inputs["x_local"], np.float32)
    const = np.concatenate([np.asarray(inputs["h_global"], np.float32).ravel(),
                            np.asarray(inputs["x_global"], np.float32).ravel(),
                            np.asarray(inputs["t"], np.float32).ravel()])
    nodes = np.unique(np.concatenate(
        [(af if pi == 0 else at)[e] for pi, e in spills]))
    node_set = np.zeros(h.shape[0], bool)
    node_set[nodes] = True
    delta = np.zeros((len(nodes), D), np.float32)
    remap = np.full(h.shape[0], -1, np.int64)
    remap[nodes] = np.arange(len(nodes))
    for pi, idx_all in ((0, af), (1, at)):
        p = "f" if pi == 0 else "t"
        edges = np.flatnonzero(node_set[idx_all])
        if not len(edges):
            continue
        inp = np.concatenate([h[af[edges]], h[at[edges]], x[edges],
                              np.broadcast_to(const, (len(edges), 21))], axis=1)
        d = np.tanh(np.tanh(np.tanh(
            inp @ inputs[p + "_W0"] + inputs[p + "_b0"]) @ inputs[p + "_W1"]
            + inputs[p + "_b1"]) @ inputs[p + "_W2"] + inputs[p + "_b2"])
        _scatter_add(delta, remap[idx_all[edges]], d.astype(np.float32))
    out[nodes] = np.tanh(delta)


def _get_exec(nc):
    """Build (once) a cached jitted executor for the bass module: the same
    _bass_exec_p/shard_map lowering run_bass_kernel_spmd uses under axon,
    but with the jitted callable memoized so repeat calls skip retracing."""
    if "exec" in _BASS_CACHE:
        return _BASS_CACHE["exec"]
    import jax
    import concourse.mybir as mybir
    from jax.sharding import Mesh, PartitionSpec
    from jax.experimental.shard_map import shard_map
    from concourse.bass2jax import (_bass_exec_p, install_neuronx_cc_hook,
                                    partition_id_tensor)
    install_neuronx_cc_hook()

    in_names, out_names, out_avals = [], [], []
    pname = nc.partition_id_tensor.name if nc.partition_id_tensor else None
    for alloc in nc.m.functions[0].allocations:
        if not isinstance(alloc, mybir.MemoryLocationSet):
            continue
        name = alloc.memorylocations[0].name
        if alloc.kind == "ExternalInput":
            if name != pname:
                in_names.append(name)
        elif alloc.kind == "ExternalOutput":
            out_names.append(name)
            out_avals.append(jax.core.ShapedArray(
                tuple(alloc.tensor_shape), mybir.dt.np(alloc.dtype)))
    n_params = len(in_names)
    n_outs = len(out_avals)
    all_names = in_names + out_names + ([pname] if pname else [])

    def _body(*args):
        ops = list(args)
        if pname:
            ops.append(partition_id_tensor())
        outs = _bass_exec_p.bind(
            *ops, out_avals=tuple(out_avals), in_names=tuple(all_names),
            out_names=tuple(out_names), lowering_input_output_aliases=(),
            sim_require_finite=True, sim_require_nnan=True, nc=nc)
        return tuple(outs)

    devices = jax.devices()[:NCORES]
    mesh = Mesh(np.asarray(devices), ("core",))
    in_specs = (PartitionSpec("core"),) * (n_params + n_outs)
    out_specs = (PartitionSpec("core"),) * n_outs
    donate = tuple(range(n_params, n_params + n_outs))
    sharded = jax.jit(
        shard_map(_body, mesh=mesh, in_specs=in_specs, out_specs=out_specs,
                  check_rep=False),
        donate_argnums=donate, keep_unused=True)

    from jax.sharding import NamedSharding
    sharding = NamedSharding(mesh, PartitionSpec("core"))
    import jax.numpy as jnp

    # donated output buffers made on-device (zeros never cross the tunnel)
    def _mk_zeros():
        return tuple(jnp.zeros((NCORES * a.shape[0], *a.shape[1:]), a.dtype)
                     for a in out_avals)
    zeros_fn = jax.jit(_mk_zeros, out_shardings=(sharding,) * n_outs)

    ex = (sharded, in_names, out_names, out_avals, sharding, zeros_fn)
    _BASS_CACHE["exec"] = ex
    return ex


def _kernel_bass(addr_from, addr_to, h_local, h_global, x_local, x_global, t,
                 f_W0, f_b0, f_W1, f_b1, f_W2, f_b2,
                 t_W0, t_b0, t_W1, t_b1, t_W2, t_b2, trace=False):
    import sys
    if "/opt/trn_rl_repo" not in sys.path:
        sys.path.insert(0, "/opt/trn_rl_repo")

    inputs = dict(addr_from=addr_from, addr_to=addr_to, h_local=h_local,
                  h_global=h_global, x_local=x_local, x_global=x_global, t=t,
                  f_W0=f_W0, f_b0=f_b0, f_W1=f_W1, f_b1=f_b1, f_W2=f_W2,
                  f_b2=f_b2, t_W0=t_W0, t_b0=t_b0, t_W1=t_W1, t_b1=t_b1,
                  t_W2=t_W2, t_b2=t_b2)
    af = np.asarray(addr_from).astype(np.int64)
    at = np.asarray(addr_to).astype(np.int64)
    h = np.asarray(h_local, np.float32)
    x = np.asarray(x_local, np.float32)

    key = (N, E)
    if key not in _BASS_CACHE:
        _BASS_CACHE[key] = _build_bass(NS, NWIN, LASTROWS, KTILES, N, NCORES)
    nc = _BASS_CACHE[key]
    sharded, in_names, out_names, out_avals, sharding, zeros_fn = _get_exec(nc)

    import jax

    # stage prep-independent inputs first: their h2d transfer overlaps the
    # host-side slot preparation below.
    staged = {}
    wm = _prep_weights(inputs)
    htab = np.ascontiguousarray(h.astype(np.float16))
    staged["hshard"] = jax.device_put(htab, sharding)
    for k, v in wm.items():
        staged[k] = jax.device_put(np.tile(v, (NCORES, 1)), sharding)
    zeros = zeros_fn()

    cbase = (np.arange(NCORES, dtype=np.int32)[:, None, None] * NS
             * np.ones((1, 128, 1), np.int32)).reshape(-1, 1)
    staged["cbase"] = jax.device_put(cbase, sharding)

    goth, gslf, locv, xt, spills = _prep_slots(af, at, x, NCORES, NS, NWIN,
                                               KTILES)
    staged["goth"] = jax.device_put(goth.reshape(-1, goth.shape[-1]), sharding)
    staged["gslf"] = jax.device_put(gslf.reshape(-1, gslf.shape[-1]), sharding)
    staged["locv"] = jax.device_put(locv.reshape(-1, locv.shape[-1]), sharding)
    staged["xt"] = jax.device_put(xt.reshape(-1, xt.shape[-1]), sharding)

    out_arrs = sharded(*[staged[n] for n in in_names], *zeros)
    out = np.asarray(out_arrs[out_names.index("delta")]).astype(np.float32)
    out *= np.float32(1.0 / 127.0)
    _fix_spill_nodes(spills, inputs, out)
    return out


def kernel(**inputs):
    try:
        return _kernel_bass(**inputs)
    except Exception:
        import traceback
        traceback.print_exc()
        return _kernel_numpy(**inputs)


# revision 39
# speedup vs baseline: 33.7929x; 1.0131x over previous
import numpy as np

# nn_LocalDynamics GNN message passing.
#   delta[n] = sum_e tanh(fMLP(inp_e))[addr_from=n] + tanh(tMLP(inp_e))[addr_to=n]
#   out = tanh(delta).  inp_e = [h[from], h[to], x_e, hg, xg, t] (153 dims).
#
# Destination-sharded design: each core owns nodes [c*12500, (c+1)*12500).
# Every edge yields two "slots": an f-slot on the core owning addr_from and a
# t-slot on the core owning addr_to.  Slots are grouped by 128-node windows of
# the owning core; each window holds a fixed KTILES tiles of 128 slots per
# population (f/t), host-padded.  On device, per batch of tiles:
#   AllGather h shards -> indirect-DMA gather of (h[from], h[to]) row pairs ->
#   XBAR transpose to feature-major -> fp16 MLP -> slot-major final layer ->
#   one-hot matmul accumulates the window's delta in PSUM -> tanh -> fp16 out.
# Host recomputes any overflowed windows (empty for uniform edges).

N = 100_000
E = 800_000
D = 64
H = 128
NCORES = 8
NS = N // NCORES            # nodes per core (12500)
NWIN = (NS + 127) // 128    # windows per core (98)
LASTROWS = NS - (NWIN - 1) * 128   # rows in last window (84)
KTILES = 10                 # 128-slot tiles per population per window
NT = NWIN * 2 * KTILES      # tiles per core (1960)
SLOTS = NT * 128            # padded slots per core (250880)
MAXBT = 4                   # max tiles per batch


def _batch_tiles(ktiles):
    out = []
    k = ktiles
    while k > 0:
        out.append(min(MAXBT, k))
        k -= out[-1]
    return out


def _scatter_add(delta, idx, vals):
    o = np.argsort(idx, kind="stable")
    si = idx[o]
    sv = vals[o]
    starts = np.flatnonzero(np.r_[True, si[1:] != si[:-1]])
    sums = np.add.reduceat(sv, starts, axis=0)
    np.add.at(delta, si[starts], sums)


def _kernel_numpy(addr_from, addr_to, h_local, h_global, x_local, x_global, t,
                  f_W0, f_b0, f_W1, f_b1, f_W2, f_b2,
                  t_W0, t_b0, t_W1, t_b1, t_W2, t_b2):
    af = np.asarray(addr_from).astype(np.int64)
    at = np.asarray(addr_to).astype(np.int64)
    h_local = np.asarray(h_local, dtype=np.float32)
    x_local = np.asarray(x_local, dtype=np.float32)
    const = np.concatenate([np.asarray(h_global, np.float32).ravel(),
                            np.asarray(x_global, np.float32).ravel(),
                            np.asarray(t, np.float32).ravel()])
    ne = af.shape[0]
    delta = np.zeros((h_local.shape[0], D), np.float32)
    CH = 100_000
    for s in range(0, ne, CH):
        e = min(s + CH, ne)
        inp = np.concatenate([h_local[af[s:e]], h_local[at[s:e]], x_local[s:e],
                              np.broadcast_to(const, (e - s, 21))], axis=1).astype(np.float32)
        d_f = np.tanh(np.tanh(np.tanh(inp @ f_W0 + f_b0) @ f_W1 + f_b1) @ f_W2 + f_b2)
        d_t = np.tanh(np.tanh(np.tanh(inp @ t_W0 + t_b0) @ t_W1 + t_b1) @ t_W2 + t_b2)
        _scatter_add(delta, af[s:e], d_f.astype(np.float32))
        _scatter_add(delta, at[s:e], d_t.astype(np.float32))
    return np.tanh(delta).astype(np.float32)


_BASS_CACHE = {}


def _build_bass(ns, nwin, lastrows, ktiles, ntot, ncores, walrus_fix=True):
    import concourse.bass as bass
    import concourse.mybir as mybir
    import concourse.tile as tile

    # walrus in this env rejects Drain instructions carrying >1 sem wait;
    # move each wait onto its own sync nop before the drain.
    def _patched(self, tick_clock, wait_clock):
        from concourse.tile import ScopedClock
        nop0 = self.nc.sync.nop(nofuse=True)
        wait_clock.add_sem_waits(nop0.ins, ScopedClock({None: tick_clock.global_clock}))
        si = nop0.ins.sync_info
        if si is not None and si.on_wait and len(si.on_wait) > 1:
            waits = list(si.on_wait)
            si.on_wait = waits[:1]
            for w in waits[1:]:
                n = self.nc.sync.nop(nofuse=True)
                n.ins.sync_info = mybir.SyncInfo(on_wait=[w], on_update=[])
        self.nc.sync.drain()
        self.nc.all_engine_barrier()
        popped = self.nc._tile_sem_poison_stack.pop()
        assert popped is self._sem_poison
        self.nc.clear_and_free_semaphores(list(self.sems.allocated().values()))
        self.nc.all_engine_barrier()

    tile.TileContext._drain_and_barrier = _patched

    f32 = mybir.dt.float32
    f16 = mybir.dt.float16
    i32 = mybir.dt.int32
    i8 = mybir.dt.int8
    nt_tot = nwin * 2 * ktiles
    slots = nt_tot * 128
    bts = _batch_tiles(ktiles)

    i16 = mybir.dt.int16
    nc = bass.Bass(num_devices=ncores)
    goth_d = nc.dram_tensor("goth", [128, nt_tot], i32, kind="ExternalInput")
    gslf_d = nc.dram_tensor("gslf", [128, nt_tot], i16, kind="ExternalInput")
    cbase_d = nc.dram_tensor("cbase", [128, 1], i32, kind="ExternalInput")
    locv_d = nc.dram_tensor("locv", [128, nt_tot], i8, kind="ExternalInput")
    xt_d = nc.dram_tensor("xt", [4, slots], f16, kind="ExternalInput")
    hshard_d = nc.dram_tensor("hshard", [ns, D], f16, kind="ExternalInput")
    hsh_b = nc.dram_tensor("hsh_b", [ns, D], f16)
    htab_d = nc.dram_tensor("hfull", [ntot, D], f16, addr_space="Shared")
    iota_d = nc.dram_tensor("iota", [128, 128], f16, kind="ExternalInput")
    wts = {}
    for p in ("f", "t"):
        wts[p + "w0h"] = nc.dram_tensor(p + "w0h", [128, H], f16, kind="ExternalInput")
        wts[p + "w0x"] = nc.dram_tensor(p + "w0x", [4, H], f16, kind="ExternalInput")
        wts[p + "w1"] = nc.dram_tensor(p + "w1", [H, H], f16, kind="ExternalInput")
        wts[p + "w2"] = nc.dram_tensor(p + "w2", [H, D], f16, kind="ExternalInput")
        wts[p + "b0"] = nc.dram_tensor(p + "b0", [H, 1], f32, kind="ExternalInput")
        wts[p + "b1"] = nc.dram_tensor(p + "b1", [H, 1], f32, kind="ExternalInput")
        wts[p + "b2r"] = nc.dram_tensor(p + "b2r", [128, MAXBT * D], f16, kind="ExternalInput")
    delta_d = nc.dram_tensor("delta", [ns, D], i8, kind="ExternalOutput")

    # all-gather the h shards into a full replicated table before the main
    # body; runs on the gpsimd stream, which also issues the gathers later,
    # so engine program order guarantees completion before first use.
    with nc.Block() as blk, \
         nc.semaphore("ag_dma") as ag_dma, \
         nc.semaphore("ag_cc") as ag_cc:

        @blk.gpsimd
        def _(g):
            g.dma_start(out=hsh_b[:, :], in_=hshard_d[:, :]).then_inc(ag_dma, 16)
            g.wait_ge(ag_dma, 16)
            g.collective_compute(
                "AllGather",
                mybir.AluOpType.bypass,
                replica_groups=[list(range(ncores))],
                ins=[hsh_b.ap().opt()],
                outs=[htab_d.ap().opt()],
            ).then_inc(ag_cc)
            g.wait_ge(ag_cc, 1)

    Tanh = mybir.ActivationFunctionType.Tanh
    MB = MAXBT
    with tile.TileContext(nc) as tc:
        with tc.tile_pool(name="wpool", bufs=1) as wp, \
             tc.tile_pool(name="io", bufs=3) as io, \
             tc.tile_pool(name="act", bufs=2) as ap_, \
             tc.tile_pool(name="ps01", bufs=1, space="PSUM") as ps01, \
             tc.tile_pool(name="psd", bufs=2, space="PSUM") as psdp, \
             tc.tile_pool(name="win", bufs=2, space="PSUM") as winp:
            wt = {}
            for k, dr in wts.items():
                tl = wp.tile(list(dr.shape), dr.dtype, tag="w" + k)
                nc.sync.dma_start(out=tl[:], in_=dr[:])
                wt[k] = tl
            iota = wp.tile([128, 128], f16, tag="iota")
            nc.sync.dma_start(out=iota[:], in_=iota_d[:])
            cbase = wp.tile([128, 1], i32, tag="cbase")
            nc.sync.dma_start(out=cbase[:], in_=cbase_d[:])

            for w in range(nwin):
                win = winp.tile([128, D], f32, tag="win")
                for pi, p in enumerate(("f", "t")):
                    tbase = (w * 2 + pi) * ktiles
                    off = 0
                    for bi, bt in enumerate(bts):
                        t0 = tbase + off
                        s0 = t0 * 128
                        nsl = bt * 128
                        off += bt
                        go = io.tile([128, MB], i32, tag="go")
                        gs16 = io.tile([128, MB], i16, tag="gs16")
                        lo8 = io.tile([128, MB], i8, tag="lo8")
                        xb = io.tile([4, MB * 128], f16, tag="xb")
                        nc.sync.dma_start(out=go[:, :bt],
                                          in_=goth_d[:, t0:t0 + bt])
                        nc.sync.dma_start(out=gs16[:, :bt],
                                          in_=gslf_d[:, t0:t0 + bt])
                        nc.sync.dma_start(out=lo8[:, :bt],
                                          in_=locv_d[:, t0:t0 + bt])
                        nc.sync.dma_start(out=xb[:, :nsl],
                                          in_=xt_d[:, s0:s0 + nsl])
                        lo = io.tile([128, MB], f16, tag="lo")
                        nc.vector.tensor_copy(out=lo[:, :bt], in_=lo8[:, :bt])
                        # self node id = core base + local node id (int16)
                        gs32 = io.tile([128, MB], i32, tag="gs32")
                        nc.vector.tensor_copy(out=gs32[:, :bt],
                                              in_=gs16[:, :bt])
                        gsf = io.tile([128, MB], i32, tag="gsf")
                        nc.vector.tensor_tensor(
                            out=gsf[:, :bt], in0=gs32[:, :bt],
                            in1=cbase[:, 0:1].to_broadcast([128, bt]),
                            op=mybir.AluOpType.add)
                        gp = io.tile([128, 2 * MB, D], f16, tag="gp")
                        # HW DGE handles one offset per partition per
                        # instruction; fan out over the columns. For the
                        # f population the self index is addr_from, for
                        # the t population it is addr_to.
                        for t in range(bt):
                            from_ap = gsf[:, t:t + 1] if pi == 0 \
                                else go[:, t:t + 1]
                            to_ap = go[:, t:t + 1] if pi == 0 \
                                else gsf[:, t:t + 1]
                            nc.gpsimd.indirect_dma_start(
                                out=gp[:, 2 * t, :], out_offset=None,
                                in_=htab_d[:],
                                in_offset=bass.IndirectOffsetOnAxis(
                                    ap=from_ap, axis=0))
                            nc.gpsimd.indirect_dma_start(
                                out=gp[:, 2 * t + 1, :], out_offset=None,
                                in_=htab_d[:],
                                in_offset=bass.IndirectOffsetOnAxis(
                                    ap=to_ap, axis=0))
                        rhs = ap_.tile([128, MB * 128], f16, tag="rhs")
                        for t in range(bt):
                            nc.sync.dma_start_transpose(
                                out=rhs[:, t * 128:(t + 1) * 128],
                                in_=gp[:, 2 * t:2 * t + 2, :])
                        ps0 = ps01.tile([128, MB * 128], f32, tag="ps0")
                        nc.tensor.matmul(out=ps0[:, :nsl], lhsT=wt[p + "w0h"][:],
                                         rhs=rhs[:, :nsl], start=True, stop=False)
                        nc.tensor.matmul(out=ps0[:, :nsl], lhsT=wt[p + "w0x"][:],
                                         rhs=xb[:, :nsl], start=False, stop=True)
                        h1 = ap_.tile([128, MB * 128], f16, tag="h1")
                        nc.scalar.activation(h1[:, :nsl], ps0[:, :nsl], Tanh,
                                             bias=wt[p + "b0"][:, 0:1])
                        ps1 = ps01.tile([128, MB * 128], f32, tag="ps1")
                        nc.tensor.matmul(out=ps1[:, :nsl], lhsT=wt[p + "w1"][:],
                                         rhs=h1[:, :nsl], start=True, stop=True)
                        h2 = ap_.tile([128, MB * 128], f16, tag="h2")
                        nc.scalar.activation(h2[:, :nsl], ps1[:, :nsl], Tanh,
                                             bias=wt[p + "b1"][:, 0:1])
                        psd = psdp.tile([128, MB * D], f32, tag="psd")
                        for t in range(bt):
                            nc.tensor.matmul(out=psd[:, t * D:(t + 1) * D],
                                             lhsT=h2[:, t * 128:(t + 1) * 128],
                                             rhs=wt[p + "w2"][:],
                                             start=True, stop=True)
                        dsb = ap_.tile([128, MB * D], f16, tag="dsb")
                        nc.vector.tensor_tensor(out=dsb[:, :bt * D],
                                                in0=psd[:, :bt * D],
                                                in1=wt[p + "b2r"][:, :bt * D],
                                                op=mybir.AluOpType.add)
                        dtl = ap_.tile([128, MB * D], f16, tag="dtl")
                        nc.scalar.activation(dtl[:, :bt * D], dsb[:, :bt * D],
                                             Tanh)
                        oh = ap_.tile([128, MB, 128], f16, tag="oh")
                        for t in range(bt):
                            nc.vector.tensor_tensor(
                                out=oh[:, t, :],
                                in0=lo[:, t:t + 1].to_broadcast([128, 128]),
                                in1=iota[:],
                                op=mybir.AluOpType.is_equal)
                        for t in range(bt):
                            nc.tensor.matmul(
                                out=win[:],
                                lhsT=oh[:, t, :],
                                rhs=dtl[:, t * D:(t + 1) * D],
                                start=(pi == 0 and bi == 0 and t == 0),
                                stop=(pi == 1 and bi == len(bts) - 1
                                      and t == bt - 1))
                rows = lastrows if w == nwin - 1 else 128
                wout = ap_.tile([128, D], f16, tag="wout")
                nc.scalar.activation(wout[:], win[:], Tanh)
                # int8 quantization: HW converts round-to-nearest-even,
                # max error 1/254 on values in [-1, 1]
                wq = ap_.tile([128, D], i8, tag="wq")
                nc.vector.tensor_scalar(out=wq[:], in0=wout[:], scalar1=127.0,
                                        scalar2=None,
                                        op0=mybir.AluOpType.mult)
                nc.sync.dma_start(out=delta_d[w * 128:w * 128 + rows, :],
                                  in_=wq[0:rows, :])

    # this walrus rejects any compute instruction carrying >1 sem wait;
    # hoist extra waits onto same-engine nops placed just before it.
    if not walrus_fix:
        return nc
    ctr = 0
    for bb in nc.main_func.blocks:
        new = []
        for ins in bb.instructions:
            si = getattr(ins, "sync_info", None)
            if si is not None and si.on_wait and len(si.on_wait) > 1:
                waits = list(si.on_wait)
                si.on_wait = [waits[-1]]
                for wv in waits[:-1]:
                    ctr += 1
                    nop = mybir.InstNoOp(
                        name=f"wsplit-{ctr}", engine=ins.engine, ins=[], outs=[],
                        sync_info=mybir.SyncInfo(on_wait=[wv], on_update=[]))
                    new.append(nop)
            new.append(ins)
        bb.instructions[:] = new
    return nc


def _get_fill_nb():
    if "fill_nb" in _BASS_CACHE:
        return _BASS_CACHE["fill_nb"]
    try:
        import numba
    except Exception:
        _BASS_CACHE["fill_nb"] = None
        return None

    @numba.njit(cache=True)
    def _fill(af32, at32, xu16, goflat, gsflat, lflat, xflat, counters,
              spill_e, spill_pi, ns, nwin, cap, nt_tot, slots):
        ne = af32.size
        nsp = 0
        for e in range(ne):
            for pi in range(2):
                dest = af32[e] if pi == 0 else at32[e]
                other = at32[e] if pi == 0 else af32[e]
                c = dest // ns
                nl = dest - c * ns
                w = nl >> 7
                loc = nl & 127
                idx = (c * nwin + w) * 2 + pi
                r = counters[idx]
                counters[idx] = r + 1
                if r >= cap:
                    spill_e[nsp] = e
                    spill_pi[nsp] = pi
                    nsp += 1
                else:
                    sc = w * (2 * cap) + pi * cap + r
                    tt = sc >> 7
                    pp = sc & 127
                    tbase = (c * 128 + pp) * nt_tot + tt
                    goflat[tbase] = other
                    gsflat[tbase] = nl
                    lflat[tbase] = loc
                    xb = (c * 4) * slots + sc
                    for rr in range(4):
                        xflat[xb + rr * slots] = xu16[e, rr]
        return nsp

    _BASS_CACHE["fill_nb"] = _fill
    return _fill


def _prep_slots(af, at, x_local, ncores, ns, nwin, ktiles):
    """Build per-core padded slot arrays in tile-major layout.
    gidx [ncores, 128, 2*NT] i32 (from/to pairs per tile column),
    locv [ncores, 128, NT] i8 (in-window node offset, -1 = pad),
    xt   [ncores, 4, SLOTS] f16 (x features, slot-major),
    spills = list of (pop, edge_indices) that overflowed window capacity."""
    fill = _get_fill_nb()
    if fill is not None:
        cap = ktiles * 128
        nt_tot = nwin * 2 * ktiles
        slots = nt_tot * 128
        af32 = np.ascontiguousarray(af.astype(np.int32))
        at32 = np.ascontiguousarray(at.astype(np.int32))
        xu16 = np.ascontiguousarray(x_local.astype(np.float16)).view(np.uint16)
        bufs = _BASS_CACHE.get("prep_bufs")
        if bufs is None or bufs[0].shape[0] != ncores or \
                bufs[0].shape[2] != nt_tot:
            bufs = (np.zeros((ncores, 128, nt_tot), np.int32),
                    np.zeros((ncores, 128, nt_tot), np.int16),
                    np.empty((ncores, 128, nt_tot), np.int8),
                    np.zeros((ncores, 4, slots), np.uint16),
                    np.empty(ncores * nwin * 2, np.int32),
                    np.empty(af32.size * 2, np.int64),
                    np.empty(af32.size * 2, np.int8))
            _BASS_CACHE["prep_bufs"] = bufs
        goth, gslf, locv, xt16, counters, spill_e, spill_pi = bufs
        # stale pad entries in goth/gslf/xt are harmless (their one-hot row
        # is zero and gathered rows stay in-bounds); only locv steers the
        # scatter and the counters must reset.
        locv.fill(-1)
        counters.fill(0)
        nsp = fill(af32, at32, xu16, goth.reshape(-1), gslf.reshape(-1),
                   locv.reshape(-1), xt16.reshape(-1), counters, spill_e,
                   spill_pi, ns, nwin, cap, nt_tot, slots)
        spills = []
        for pi in (0, 1):
            sel = spill_e[:nsp][spill_pi[:nsp] == pi]
            if len(sel):
                spills.append((pi, sel))
        return goth, gslf, locv, xt16.view(np.float16), spills
    return _prep_slots_np(af, at, x_local, ncores, ns, nwin, ktiles)


def _prep_slots_np(af, at, x_local, ncores, ns, nwin, ktiles):
    cap = ktiles * 128
    nt_tot = nwin * 2 * ktiles
    slots = nt_tot * 128
    af32 = af.astype(np.int32)
    at32 = at.astype(np.int32)
    goth = np.zeros((ncores, 128, nt_tot), np.int32)
    gslf = np.zeros((ncores, 128, nt_tot), np.int16)
    locv = np.full((ncores, 128, nt_tot), -1, np.int8)
    xt = np.zeros((ncores, 4, slots), np.float16)
    xtv = np.ascontiguousarray(x_local.astype(np.float16))
    goflat = goth.reshape(-1)
    gsflat = gslf.reshape(-1)
    lflat = locv.reshape(-1)
    xflat = xt.reshape(-1)
    spills = []
    for pi, dest in enumerate((af32, at32)):
        other = at32 if pi == 0 else af32
        core = dest // np.int32(ns)
        node_l = dest - core * np.int32(ns)
        w = node_l >> 7
        loc = (node_l & 127).astype(np.int8)
        cw = core * np.int32(nwin) + w
        order = np.argsort(cw, kind="stable").astype(np.int32)
        counts = np.bincount(cw, minlength=ncores * nwin)
        starts = np.concatenate([[0], np.cumsum(counts)[:-1]])
        rank = (np.arange(len(cw), dtype=np.int32)
                - np.repeat(starts, counts).astype(np.int32))
        ok = rank < cap
        if not ok.all():
            spills.append((pi, order[~ok].astype(np.int64)))
            e_ok = order[ok]
            r = rank[ok]
        else:
            e_ok = order
            r = rank
        # within-core slot: window block of 2*cap, population block of cap
        sc = w[e_ok] * np.int32(2 * cap) + np.int32(pi * cap) + r
        T = sc >> 7
        pp = sc & 127
        c_ok = core[e_ok]
        tbase = (c_ok * 128 + pp) * nt_tot + T
        goflat[tbase] = other[e_ok]
        gsflat[tbase] = node_l[e_ok].astype(np.int16)
        lflat[tbase] = loc[e_ok]
        xbase = (c_ok * 4) * slots + sc
        xflat[xbase[:, None] + (np.arange(4, dtype=np.int32) * slots)[None, :]] \
            = xtv[e_ok]
    return goth, gslf, locv, xt, spills


def _prep_weights(inputs):
    const = np.concatenate([np.asarray(inputs["h_global"], np.float32).ravel(),
                            np.asarray(inputs["x_global"], np.float32).ravel(),
                            np.asarray(inputs["t"], np.float32).ravel()])
    wm = {}
    for p in ("f", "t"):
        W0 = np.asarray(inputs[p + "_W0"], np.float32)
        b0 = np.asarray(inputs[p + "_b0"], np.float32)
        b0eff = b0 + const @ W0[132:153]
        wm[p + "w0h"] = np.ascontiguousarray(W0[0:128]).astype(np.float16)
        wm[p + "w0x"] = np.ascontiguousarray(W0[128:132]).astype(np.float16)
        wm[p + "w1"] = np.asarray(inputs[p + "_W1"], np.float32).astype(np.float16)
        wm[p + "w2"] = np.asarray(inputs[p + "_W2"], np.float32).astype(np.float16)
        wm[p + "b0"] = b0eff.reshape(H, 1).astype(np.float32)
        wm[p + "b1"] = np.asarray(inputs[p + "_b1"], np.float32).reshape(H, 1)
        b2 = np.asarray(inputs[p + "_b2"], np.float32)
        wm[p + "b2r"] = np.tile(b2.reshape(1, D), (128, MAXBT)).astype(np.float16)
    wm["iota"] = np.broadcast_to(np.arange(128, dtype=np.float16), (128, 128)).copy()
    return wm


def _fix_spill_nodes(spills, inputs, out):
    """Recompute on host (fp32) every node whose window overflowed device
    capacity; overwrite those rows of `out`. Empty for uniform edge data."""
    if not spills:
        return
    af = np.asarray(inputs["addr_from"]).astype(np.int64)
    at = np.asarray(inputs["addr_to"]).astype(np.int64)
    h = np.asarray(inputs["h_local"], np.float32)
    x = np.asarray(inputs["x_local"], np.float32)
    const = np.concatenate([np.asarray(inputs["h_global"], np.float32).ravel(),
                            np.asarray(inputs["x_global"], np.float32).ravel(),
                            np.asarray(inputs["t"], np.float32).ravel()])
    nodes = np.unique(np.concatenate(
        [(af if pi == 0 else at)[e] for pi, e in spills]))
    node_set = np.zeros(h.shape[0], bool)
    node_set[nodes] = True
    delta = np.zeros((len(nodes), D), np.float32)
    remap = np.full(h.shape[0], -1, np.int64)
    remap[nodes] = np.arange(len(nodes))
    for pi, idx_all in ((0, af), (1, at)):
        p = "f" if pi == 0 else "t"
        edges = np.flatnonzero(node_set[idx_all])
        if not len(edges):
            continue
        inp = np.concatenate([h[af[edges]], h[at[edges]], x[edges],
                              np.broadcast_to(const, (len(edges), 21))], axis=1)
        d = np.tanh(np.tanh(np.tanh(
            inp @ inputs[p + "_W0"] + inputs[p + "_b0"]) @ inputs[p + "_W1"]
            + inputs[p + "_b1"]) @ inputs[p + "_W2"] + inputs[p + "_b2"])
        _scatter_add(delta, remap[idx_all[edges]], d.astype(np.float32))
    out[nodes] = np.tanh(delta)


def _get_exec(nc, key):
    """Build (once) a cached jitted executor for the bass module: the same
    _bass_exec_p/shard_map lowering run_bass_kernel_spmd uses under axon,
    but with the jitted callable memoized so repeat calls skip retracing."""
    if key in _BASS_CACHE:
        return _BASS_CACHE[key]
    import jax
    import concourse.mybir as mybir
    from jax.sharding import Mesh, PartitionSpec
    from jax.experimental.shard_map import shard_map
    from concourse.bass2jax import (_bass_exec_p, install_neuronx_cc_hook,
                                    partition_id_tensor)
    install_neuronx_cc_hook()

    in_names, out_names, out_avals = [], [], []
    pname = nc.partition_id_tensor.name if nc.partition_id_tensor else None
    for alloc in nc.m.functions[0].allocations:
        if not isinstance(alloc, mybir.MemoryLocationSet):
            continue
        name = alloc.memorylocations[0].name
        if alloc.kind == "ExternalInput":
            if name != pname:
                in_names.append(name)
        elif alloc.kind == "ExternalOutput":
            out_names.append(name)
            out_avals.append(jax.core.ShapedArray(
                tuple(alloc.tensor_shape), mybir.dt.np(alloc.dtype)))
    n_params = len(in_names)
    n_outs = len(out_avals)
    all_names = in_names + out_names + ([pname] if pname else [])

    def _body(*args):
        ops = list(args)
        if pname:
            ops.append(partition_id_tensor())
        outs = _bass_exec_p.bind(
            *ops, out_avals=tuple(out_avals), in_names=tuple(all_names),
            out_names=tuple(out_names), lowering_input_output_aliases=(),
            sim_require_finite=True, sim_require_nnan=True, nc=nc)
        return tuple(outs)

    devices = jax.devices()[:NCORES]
    mesh = Mesh(np.asarray(devices), ("core",))
    in_specs = (PartitionSpec("core"),) * (n_params + n_outs)
    out_specs = (PartitionSpec("core"),) * n_outs
    donate = tuple(range(n_params, n_params + n_outs))
    sharded = jax.jit(
        shard_map(_body, mesh=mesh, in_specs=in_specs, out_specs=out_specs,
                  check_rep=False),
        donate_argnums=donate, keep_unused=True)

    from jax.sharding import NamedSharding
    sharding = NamedSharding(mesh, PartitionSpec("core"))
    import jax.numpy as jnp

    # donated output buffers made on-device (zeros never cross the tunnel)
    def _mk_zeros():
        return tuple(jnp.zeros((NCORES * a.shape[0], *a.shape[1:]), a.dtype)
                     for a in out_avals)
    zeros_fn = jax.jit(_mk_zeros, out_shardings=(sharding,) * n_outs)

    ex = (sharded, in_names, out_names, out_avals, sharding, zeros_fn)
    _BASS_CACHE[key] = ex
    return ex


def _kernel_bass(addr_from, addr_to, h_local, h_global, x_local, x_global, t,
                 f_W0, f_b0, f_W1, f_b1, f_W2, f_b2,
                 t_W0, t_b0, t_W1, t_b1, t_W2, t_b2, trace=False):
    import sys
    if "/opt/trn_rl_repo" not in sys.path:
        sys.path.insert(0, "/opt/trn_rl_repo")

    inputs = dict(addr_from=addr_from, addr_to=addr_to, h_local=h_local,
                  h_global=h_global, x_local=x_local, x_global=x_global, t=t,
                  f_W0=f_W0, f_b0=f_b0, f_W1=f_W1, f_b1=f_b1, f_W2=f_W2,
                  f_b2=f_b2, t_W0=t_W0, t_b0=t_b0, t_W1=t_W1, t_b1=t_b1,
                  t_W2=t_W2, t_b2=t_b2)
    af = np.asarray(addr_from).astype(np.int64)
    at = np.asarray(addr_to).astype(np.int64)
    h = np.asarray(h_local, np.float32)
    x = np.asarray(x_local, np.float32)

    # size the per-window tile capacity from the actual data: K tiles of 128
    # slots covering the fullest (core, window, population) group, so padding
    # is minimal and the spill path stays empty
    af32 = af.astype(np.int32)
    at32 = at.astype(np.int32)
    wmax = 0
    for dest in (af32, at32):
        cw = (dest // NS) * NWIN + ((dest % NS) >> 7)
        wmax = max(wmax, int(np.bincount(cw, minlength=NCORES * NWIN).max()))
    ktiles = max(1, min(-(-wmax // 128), 4 * KTILES))

    key = (N, E, ktiles)
    if key not in _BASS_CACHE:
        _BASS_CACHE[key] = _build_bass(NS, NWIN, LASTROWS, ktiles, N, NCORES)
    nc = _BASS_CACHE[key]
    sharded, in_names, out_names, out_avals, sharding, zeros_fn = \
        _get_exec(nc, ("exec", ktiles))

    import jax

    # stage prep-independent inputs first: their h2d transfer overlaps the
    # host-side slot preparation below.
    staged = {}
    wm = _prep_weights(inputs)
    htab = np.ascontiguousarray(h.astype(np.float16))
    staged["hshard"] = jax.device_put(htab, sharding)
    for k, v in wm.items():
        staged[k] = jax.device_put(np.tile(v, (NCORES, 1)), sharding)
    zeros = zeros_fn()

    cbase = (np.arange(NCORES, dtype=np.int32)[:, None, None] * NS
             * np.ones((1, 128, 1), np.int32)).reshape(-1, 1)
    staged["cbase"] = jax.device_put(cbase, sharding)

    goth, gslf, locv, xt, spills = _prep_slots(af, at, x, NCORES, NS, NWIN,
                                               ktiles)
    staged["goth"] = jax.device_put(goth.reshape(-1, goth.shape[-1]), sharding)
    staged["gslf"] = jax.device_put(gslf.reshape(-1, gslf.shape[-1]), sharding)
    staged["locv"] = jax.device_put(locv.reshape(-1, locv.shape[-1]), sharding)
    staged["xt"] = jax.device_put(xt.reshape(-1, xt.shape[-1]), sharding)

    out_arrs = sharded(*[staged[n] for n in in_names], *zeros)
    out = np.asarray(out_arrs[out_names.index("delta")]).astype(np.float32)
    out *= np.float32(1.0 / 127.0)
    _fix_spill_nodes(spills, inputs, out)
    return out


def kernel(**inputs):
    try:
        return _kernel_bass(**inputs)
    except Exception:
        import traceback
        traceback.print_exc()
        return _kernel_numpy(**inputs)
